# revision 1
# baseline (speedup 1.0000x reference)
"""Trainium2 Bass kernel for nn_AlignModule_full (8 NeuronCores, data-parallel).

Reference computation: two 1x1 convs -> concat -> 3x3 conv + BN + ReLU ->
3x3 conv -> flow -> bilinear grid_sample warp of t2_pred, where output
channel (n, ch) is warped with flow[(3n+ch) % 4] (torch flow.repeat
semantics faithfully ported by the reference).

Sharding: core c = (q, h), q = c//2 flow batch, h = c%2 row half.
Each core computes flow(q, rows 64h..64h+64) from batch-q features, then
warps the 19 (n, ch) images with (3n+ch)%4 == q for its row half, using
only its own flow. Zero cross-core communication.

Warp implementation: per-pixel bilinear gather via gpsimd ap_gather with a
host-built d=8 interleaved source: each index fetches the full 2x2 patch
for TWO image slots at once (19 images = 16 lanes x 2 slots).
"""
import sys

for _p in ('/opt/trn_rl_repo',):
    if _p not in sys.path:
        sys.path.append(_p)

import numpy as np
import ml_dtypes

import concourse.bass as bass
import concourse.bacc as bacc
import concourse.mybir as mybir
import concourse.tile as tile

F32 = mybir.dt.float32
BF16 = mybir.dt.bfloat16
I16 = mybir.dt.int16
AF = mybir.ActivationFunctionType
ALU = mybir.AluOpType

H, W, CIN, T, CCLS, NB = 128, 256, 256, 64, 19, 4
SLAB_R = 68          # feature slab rows
WS = 258             # padded width for t/x buffers
XR = 66              # x rows total
XH = 36              # x rows per partition-half (A: 0..36, B: 30..66)
YS, XS = 76, 26      # gather slab rows/cols per (group, call=col-half)
LNUM = YS * XS       # base positions per partition
DCH = 8              # interleave chunk: 2 slots x (2x2 patch)
NIDX = 1024          # gather indices per group per call
ROWB = 6             # slab row margin before first output row of the call
COLB = 5             # slab col margin before group col block

BF = ml_dtypes.bfloat16


def img_list(q):
    return [(n, ch) for n in range(NB) for ch in range(CCLS)
            if (3 * n + ch) % 4 == q]


def build_nc():
    nc = bacc.Bacc(None, target_bir_lowering=False, debug=False)
    P = nc.declare_dram_parameter
    f1_d = P("f1", [2, 128, SLAB_R, W], BF16, isOutput=False)
    f2_d = P("f2", [2, 128, SLAB_R, W], BF16, isOutput=False)
    wd_d = P("wd", [128, 2, 2, T], BF16, isOutput=False)
    wf1_d = P("wf1", [128, 9, T], BF16, isOutput=False)
    wf2_d = P("wf2", [128, 2, 9, 2], BF16, isOutput=False)
    bn_d = P("bn", [128, 2, 1], F32, isOutput=False)
    mask_d = P("mask", [128, 2, 1], F32, isOutput=False)
    bx_d = P("bx", [128, 128], F32, isOutput=False)
    by_d = P("by", [128, 128], F32, isOutput=False)
    ylo_d = P("ylo", [128, 128], F32, isOutput=False)
    yhi_d = P("yhi", [128, 128], F32, isOutput=False)
    xlo_d = P("xlo", [128, 128], F32, isOutput=False)
    xhi_d = P("xhi", [128, 128], F32, isOutput=False)
    emat_d = P("emat", [8, 128], BF16, isOutput=False)
    dsrc_d = P("dsrc", [2, 128, LNUM * DCH], BF16, isOutput=False)
    out_d = P("out", [CCLS, 64, W], F32, isOutput=True)

    flow_dramh = [nc.dram_tensor("flow_t0", [2, W, 32], F32),
                  nc.dram_tensor("flow_t1", [2, W, 32], F32)]  # (ch, col, row-half)
    w_dram = nc.dram_tensor("w_dram", [4, 128 * 128], BF16)

    with tile.TileContext(nc) as tc:
        with (
            tc.tile_pool(name="stream", bufs=3) as sp,
            tc.tile_pool(name="big", bufs=1) as bp,
            tc.tile_pool(name="psA", bufs=2, space="PSUM") as pp,
        ):
            # ---- constants ----
            wd_s = bp.tile([128, 2, 2, T], BF16, tag="wd")
            wf1_s = bp.tile([128, 9, T], BF16, tag="wf1")
            wf2_s = bp.tile([128, 2, 9, 2], BF16, tag="wf2")
            bn_s = bp.tile([128, 2, 1], F32, tag="bn")
            mask_s = bp.tile([128, 2, 1], F32, tag="mask")
            xlo_s = bp.tile([128, 128], F32, tag="xlo")
            xhi_s = bp.tile([128, 128], F32, tag="xhi")
            emat_s = bp.tile([8, 128], BF16, tag="emat")
            bx_s = bp.tile([128, 128], F32, tag="bx")
            by_s = bp.tile([128, 128], F32, tag="by")
            ylo_s = bp.tile([128, 128], F32, tag="ylo")
            yhi_s = bp.tile([128, 128], F32, tag="yhi")
            for t_, d_ in ((wd_s, wd_d), (wf1_s, wf1_d), (wf2_s, wf2_d),
                           (bn_s, bn_d), (mask_s, mask_d), (xlo_s, xlo_d),
                           (xhi_s, xhi_d),
                           (emat_s, emat_d), (bx_s, bx_d), (by_s, by_d),
                           (ylo_s, ylo_d), (yhi_s, yhi_d)):
                nc.scalar.dma_start(t_[:], d_[:])

            # ---- big shared tiles ----
            t_cat = bp.tile([128, SLAB_R * WS], BF16, tag="tcat_gat")
            dsrc = bp.tile([128, LNUM * DCH], BF16, tag="dsrc")
            dsrc2 = bp.tile([128, LNUM * DCH], BF16, tag="dsrc2")
            with tc.tile_wait_until(0.12):
                nc.gpsimd.dma_start(dsrc[:, 0:LNUM * DCH // 2], dsrc_d[0, :, 0:LNUM * DCH // 2])
                nc.sync.dma_start(dsrc[:, LNUM * DCH // 2:], dsrc_d[0, :, LNUM * DCH // 2:])
            with tc.tile_wait_until(0.14):
                nc.gpsimd.dma_start(dsrc2[:, 0:LNUM * DCH // 2], dsrc_d[1, :, 0:LNUM * DCH // 2])
                nc.sync.dma_start(dsrc2[:, LNUM * DCH // 2:], dsrc_d[1, :, LNUM * DCH // 2:])
            x_sb = bp.tile([128, XH * WS], BF16, tag="x_w4")

            t3 = t_cat[:].rearrange("p (r c) -> p r c", r=SLAB_R, c=WS)
            nc.vector.memset(t3[:, :, 0:1], 0.0)
            nc.vector.memset(t3[:, :, 257:258], 0.0)

            # ---- phases 1+2 interleaved: 1x1 convs feed 3x3 conv ----
            def p1_tile(it):
                r0 = 2 * it
                ps = pp.tile([128, 2 * W], F32, tag="pst", name="pst")
                fs = []
                for ck in range(2):
                    fa = sp.tile([128, 2 * W], BF16, tag="fa", name="fa")
                    fb = sp.tile([128, 2 * W], BF16, tag="fb", name="fb")
                    nc.sync.dma_start(fa[:], f1_d[ck, :, r0:r0 + 2, :])
                    nc.sync.dma_start(fb[:], f2_d[ck, :, r0:r0 + 2, :])
                    fs.append((fa, fb))
                for ck in range(2):
                    nc.tensor.matmul(ps[0:T, :], wd_s[:, 0, ck, :], fs[ck][0][:],
                                     start=(ck == 0), stop=(ck == 1))
                for ck in range(2):
                    nc.tensor.matmul(ps[T:128, :], wd_s[:, 1, ck, :], fs[ck][1][:],
                                     start=(ck == 0), stop=(ck == 1))
                dst = bass.AP(tensor=t_cat.tensor, offset=r0 * WS + 1,
                              ap=[[SLAB_R * WS, 128], [WS, 2], [1, W]])
                nc.vector.tensor_copy(dst, ps[:].rearrange("p (r c) -> p r c",
                                                           r=2, c=W))

            x3 = x_sb[:].rearrange("p (r c) -> p r c", r=XH, c=WS)
            nc.vector.memset(x3[:, :, 0:1], 0.0)
            nc.vector.memset(x3[:, :, 257:258], 0.0)

            def p2_iter(it):
                jA = 2 * it
                jB = 30 + 2 * it
                ps = pp.tile([128, 2 * W], F32, tag="psx", name="psx")
                for tap in range(9):
                    dy, dx = tap // 3, tap % 3
                    rhsA = bass.AP(tensor=t_cat.tensor,
                                   offset=(jA + dy) * WS + dx,
                                   ap=[[SLAB_R * WS, 128], [WS, 2], [1, W]])
                    rhsB = bass.AP(tensor=t_cat.tensor,
                                   offset=(jB + dy) * WS + dx,
                                   ap=[[SLAB_R * WS, 128], [WS, 2], [1, W]])
                    nc.tensor.matmul(ps[0:T, :], wf1_s[:, tap, :], rhsA,
                                     start=(tap == 0), stop=(tap == 8),
                                     tile_position=(0, 0),
                                     skip_group_check=True)
                    nc.tensor.matmul(ps[T:128, :], wf1_s[:, tap, :], rhsB,
                                     start=(tap == 0), stop=(tap == 8),
                                     tile_position=(0, 64),
                                     skip_group_check=True)
                dstA = bass.AP(tensor=x_sb.tensor, offset=jA * WS + 1,
                               ap=[[XH * WS, T], [WS, 2], [1, W]])
                dstB = bass.AP(tensor=x_sb.tensor,
                               offset=T * (XH * WS) + jA * WS + 1,
                               ap=[[XH * WS, T], [WS, 2], [1, W]])
                nc.scalar.activation(dstA,
                                     ps[0:T].rearrange("p (r c) -> p r c", r=2, c=W),
                                     AF.Relu, bias=bn_s[0:T, 1], scale=bn_s[0:T, 0])
                nc.scalar.activation(dstB,
                                     ps[T:128].rearrange("p (r c) -> p r c", r=2, c=W),
                                     AF.Relu, bias=bn_s[T:128, 1], scale=bn_s[T:128, 0])

            for it in range(18):
                p1_tile(it)
            for it in range(18):
                p2_iter(it)
                if 18 + it < SLAB_R // 2:
                    p1_tile(18 + it)
            nc.vector.tensor_scalar_mul(x3[0:T, 0, :], x3[0:T, 0, :], mask_s[0:T, 0])
            nc.vector.tensor_scalar_mul(x3[T:128, 35, :], x3[T:128, 35, :],
                                        mask_s[T:128, 1])

            # ---- phase 3: 3x3 conv 64->2 (plain, half-aware rhs) ----
            for it in range(32):
                i0 = 2 * it
                hf = 0 if i0 < 34 else 1
                ps = pp.tile([2, 2 * W], F32, tag="psf", name="psf", bufs=4)
                for tap in range(9):
                    dy, dx = tap // 3, tap % 3
                    base = i0 + dy - 30 * hf
                    rhs = bass.AP(tensor=x_sb.tensor,
                                  offset=base * WS + dx,
                                  ap=[[XH * WS, 128], [WS, 2], [1, W]])
                    nc.tensor.matmul(ps[:], wf2_s[:, hf, tap, :], rhs,
                                     start=(tap == 0), stop=(tap == 8))
                bt = sp.tile([2, 2 * W], F32, tag="bt", name="bt", bufs=8)
                bt_v = bass.AP(tensor=bt.tensor, offset=0,
                               ap=[[2 * W, 2], [1, 2], [2, W]])
                nc.vector.tensor_copy(bt_v,
                                      ps[:].rearrange("p (r c) -> p r c", r=2, c=W))
                dst = bass.AP(tensor=flow_dramh[i0 // 32], offset=i0 % 32,
                              ap=[[W * 32, 2], [32, W], [1, 2]])
                (nc.scalar if it % 2 == 0 else nc.sync).dma_start(dst, bt[:])

            # ---- phase 4/5: flow -> CL + index math + gathers, by row half ----
            cl_fx = bp.tile([128, 128], F32, tag="clfx")
            cl_fy = bp.tile([128, 128], F32, tag="clfy")

            def cl(tag):
                tt = bp.tile([128, 128], F32, tag=tag, name=tag)
                return tt

            ix = cl("ix"); iy = cl("iy"); tmp = cl("tmp")
            x0i = bp.tile([128, 128], I16, tag="x0i")
            y0i = bp.tile([128, 128], I16, tag="y0i")
            x0f = cl("x0f"); y0f = cl("y0f")
            ef = cl("ef")
            eidx = bp.tile([128, 128], I16, tag="eidx")
            gatall = bp.tile([128, 2 * NIDX * DCH], BF16, tag="tcat_gat")
            _qs = [nc.sync, nc.scalar]
            _qi = 0

            def cl_load(rh):

                for ch, dtile in ((0, cl_fx), (1, cl_fy)):
                    for G in range(8):
                        for w in range(2):
                            dst = bass.AP(tensor=dtile.tensor,
                                          offset=(16 * G) * 128 + 64 * w + 32 * rh,
                                          ap=[[128, 16], [1, 32]])
                            srcp = bass.AP(
                                tensor=flow_dramh[rh],
                                offset=ch * W * 32 + (32 * G + 16 * w) * 32,
                                ap=[[32, 16], [1, 32]])
                            _qs[(ch + G + w) % 2].dma_start(dst, srcp)

            def idx_math(sl):
                V = nc.vector
                V.tensor_scalar_mul(ix[:, sl], cl_fx[:, sl], 0.5)
                V.tensor_tensor(ix[:, sl], ix[:, sl], bx_s[:, sl], ALU.add)
                V.tensor_scalar_mul(iy[:, sl], cl_fy[:, sl], 0.5)
                V.tensor_tensor(iy[:, sl], iy[:, sl], by_s[:, sl], ALU.add)
                V.tensor_copy(x0i[:, sl], ix[:, sl])
                V.tensor_copy(x0f[:, sl], x0i[:, sl])
                V.tensor_tensor(tmp[:, sl], x0f[:, sl], ix[:, sl], ALU.is_gt)
                V.tensor_tensor(x0f[:, sl], x0f[:, sl], tmp[:, sl], ALU.subtract)
                V.tensor_copy(y0i[:, sl], iy[:, sl])
                V.tensor_copy(y0f[:, sl], y0i[:, sl])
                V.tensor_tensor(tmp[:, sl], y0f[:, sl], iy[:, sl], ALU.is_gt)
                V.tensor_tensor(y0f[:, sl], y0f[:, sl], tmp[:, sl], ALU.subtract)
                V.tensor_scalar_mul(ef[:, sl], y0f[:, sl], float(XS))
                V.tensor_tensor(ef[:, sl], ef[:, sl], x0f[:, sl], ALU.add)
                V.tensor_scalar(ef[:, sl], ef[:, sl], 0.0, float(LNUM - XS - 2),
                                ALU.max, ALU.min)
                V.tensor_copy(eidx[:, sl], ef[:, sl])

            for rh in range(2):
                cl_load(rh)
                for w in range(2):
                    sl = slice(64 * w + 32 * rh, 64 * w + 32 * rh + 32)
                    idx_math(sl)
                    ds = dsrc if w == 0 else dsrc2
                    off = w * (NIDX * DCH) + rh * 4096
                    nc.gpsimd.ap_gather(
                        gatall[:, off:off + 4096], ds[:],
                        eidx[:, sl],
                        channels=128, num_elems=LNUM, d=DCH, num_idxs=512)

            # ---- weights math (full tensors) ----
            fx = cl("fx"); fy = cl("fy")
            nc.vector.tensor_tensor(fx[:], ix[:], x0f[:], ALU.subtract)
            nc.vector.tensor_tensor(fy[:], iy[:], y0f[:], ALU.subtract)
            vx0 = cl("vx0"); vx1 = cl("vx1"); vy0 = cl("vy0"); vy1 = cl("vy1")
            xp1 = cl("xp1"); yp1 = cl("yp1")
            nc.vector.tensor_scalar_add(xp1[:], x0f[:], 1.0)
            nc.vector.tensor_scalar_add(yp1[:], y0f[:], 1.0)
            for vt, src_f in ((vx0, x0f), (vx1, xp1)):
                nc.vector.tensor_tensor(vt[:], src_f[:], xlo_s[:], ALU.is_ge)
                nc.vector.tensor_tensor(tmp[:], src_f[:], xhi_s[:], ALU.is_le)
                nc.vector.tensor_tensor(vt[:], vt[:], tmp[:], ALU.mult)
            for vt, src_f in ((vy0, y0f), (vy1, yp1)):
                nc.vector.tensor_tensor(vt[:], src_f[:], ylo_s[:], ALU.is_ge)
                nc.vector.tensor_tensor(tmp[:], src_f[:], yhi_s[:], ALU.is_le)
                nc.vector.tensor_tensor(vt[:], vt[:], tmp[:], ALU.mult)
            gx0 = cl("gx0"); gx1 = cl("gx1"); gy0 = cl("gy0"); gy1 = cl("gy1")
            nc.vector.tensor_scalar(tmp[:], fx[:], -1.0, 1.0, ALU.mult, ALU.add)
            nc.vector.tensor_tensor(gx0[:], tmp[:], vx0[:], ALU.mult)
            nc.vector.tensor_tensor(gx1[:], fx[:], vx1[:], ALU.mult)
            nc.vector.tensor_scalar(tmp[:], fy[:], -1.0, 1.0, ALU.mult, ALU.add)
            nc.vector.tensor_tensor(gy0[:], tmp[:], vy0[:], ALU.mult)
            nc.vector.tensor_tensor(gy1[:], fy[:], vy1[:], ALU.mult)
            ws_ = []
            for _wn in ("w00", "w01", "w10", "w11"):
                _wt = bp.tile([128, 128], BF16, tag=_wn, name=_wn)
                ws_.append(_wt)
            nc.vector.tensor_tensor(ws_[0][:], gx0[:], gy0[:], ALU.mult)
            nc.vector.tensor_tensor(ws_[1][:], gx1[:], gy0[:], ALU.mult)
            nc.vector.tensor_tensor(ws_[2][:], gx0[:], gy1[:], ALU.mult)
            nc.vector.tensor_tensor(ws_[3][:], gx1[:], gy1[:], ALU.mult)
            # ---- phase 6: weight planes -> dram -> w_g -> W4 (l-replicated) ----
            for s in range(4):
                nc.sync.dma_start(
                    bass.AP(tensor=w_dram, offset=s * 16384,
                            ap=[[128, 128], [1, 128]]),
                    ws_[s][:])
            w_g = bp.tile([8, 4, 2048], BF16, tag="wg")
            for s in range(4):
                nc.sync.dma_start(
                    w_g[:, s, :],
                    bass.AP(tensor=w_dram, offset=s * 16384,
                            ap=[[2048, 8], [1, 2048]]))
            # W4 [128, 4, 2048] j-ordered (j = 32r + 16w + m over full 64 rows)
            w4 = bp.tile([128, 4 * 2048], F32, tag="x_w4")
            for s in range(4):
                for c4 in range(4):
                    pw = pp.tile([128, 512], F32, tag="pst", name="pw")
                    nc.tensor.matmul(pw[:], emat_s[:], w_g[:, s, 512 * c4:512 * (c4 + 1)],
                                     start=True, stop=True)
                    # pw free = (m-part: 128,4)(r: 2,64)(w: 1,2) for m in [4c4, 4c4+4)
                    dstw = bass.AP(tensor=w4.tensor,
                                   offset=s * 2048 + 4 * c4,
                                   ap=[[4 * 2048, 128], [1, 4], [1024, 2], [16, 64]])
                    nc.scalar.copy(
                        dstw,
                        pw[:].rearrange("p (m w r) -> p m w r", m=4, w=2, r=64))

            pls = [bp.tile([128, NIDX], F32, tag=f"pl{s}", name=f"pl{s}") for s in range(4)]
            bb_s = [bp.tile([128, NIDX], F32, tag=f"bb{i}", name=f"bb{i}")
                    for i in range(2)]
            for call in range(2):
                g = gatall[:, call * NIDX * DCH:(call + 1) * NIDX * DCH]
                for slot in range(2):
                    for s in range(4):
                        g_v = bass.AP(tensor=gatall.tensor,
                                      offset=call * NIDX * DCH + 4 * slot + s,
                                      ap=[[2 * NIDX * DCH, 128], [DCH, NIDX]])
                        nc.vector.tensor_tensor(
                            pls[s][:], g_v,
                            w4[:, (s * 2048 + 1024 * call):(s * 2048 + 1024 * call + NIDX)],
                            ALU.mult)
                    bb = bb_s[slot]
                    nc.vector.tensor_tensor(pls[0][:], pls[0][:], pls[1][:], ALU.add)
                    nc.vector.tensor_tensor(pls[2][:], pls[2][:], pls[3][:], ALU.add)
                    nc.vector.tensor_tensor(bb[:], pls[0][:], pls[2][:], ALU.add)
                    nl = 16 if slot == 0 else 3
                    for G in range(8):
                        dst = bass.AP(
                            tensor=out_d,
                            offset=(16 * slot) * 64 * W + 32 * G + 16 * call,
                            ap=[[64 * W, nl], [W, 64], [1, 16]])
                        srcb = bass.AP(
                            tensor=bb.tensor, offset=(16 * G) * NIDX,
                            ap=[[NIDX, nl], [16, 64], [1, 16]])
                        nc.scalar.dma_start(dst, srcb)
    nc.finalize()
    return nc


# ======================= host-side prep =======================

def _feat_slab(feat_b, h):
    """feat_b (256, 128, 256) f32 -> (2, 128, 68, 256) bf16 slab for half h."""
    r0 = 64 * h - 2
    slab = np.zeros((CIN, SLAB_R, W), np.float32)
    lo, hi = max(r0, 0), min(r0 + SLAB_R, H)
    slab[:, lo - r0:hi - r0, :] = feat_b[:, lo:hi, :]
    return np.ascontiguousarray(
        slab.reshape(2, 128, SLAB_R, W).astype(BF))


def _host_constants(q, h):
    R0 = 64 * h
    # CL layout: p = 16G + m, f = 64w + r; pixel (row R0+r, col 32G+16w+m)
    p = np.arange(128)[:, None]
    f = np.arange(128)[None, :]
    G = p // 16
    m = p % 16
    r = f % 64
    w = f // 64
    col = 32 * G + 16 * w + m
    row = R0 + r
    ix_base = col + col / (W - 1.0) - 0.5
    iy_base = row + row / (H - 1.0) - 0.5
    colbase = 32 * G + 16 * w - COLB
    rowbase = R0 - ROWB
    bx = np.broadcast_to(ix_base - colbase, (128, 128)).astype(np.float32).copy()
    by = np.broadcast_to(iy_base - rowbase, (128, 128)).astype(np.float32).copy()
    xlo = np.broadcast_to(0.0 - colbase, (128, 128)).astype(np.float32).copy()
    xhi = np.broadcast_to((W - 1.0) - colbase, (128, 128)).astype(np.float32).copy()
    ylo = np.full((128, 128), 0.0 - rowbase, np.float32)
    yhi = np.full((128, 128), (H - 1.0) - rowbase, np.float32)
    return bx, by, xlo, xhi, ylo, yhi


def _dsrc_build(pred_imgs, h):
    """pred_imgs: (19, 128, 256) f32. Returns (2, 128, LNUM*8) f32 gather
    source; call = col-half w, slab = rows [R0-6, R0+70) x 26-col band."""
    R0 = 64 * h
    padded = np.zeros((CCLS, H + 16, W + 16), np.float32)
    padded[:, 8:8 + H, 8:8 + W] = pred_imgs
    out = np.zeros((2, 128, LNUM, DCH), np.float32)
    rowbase = R0 - ROWB
    for call in range(2):
        for G in range(8):
            colbase = 32 * G + 16 * call - COLB
            for l in range(16):
                for slot in range(2):
                    img = l + 16 * slot
                    if img >= CCLS:
                        img = l
                    for j2 in range(2):
                        for j1 in range(2):
                            win = padded[img,
                                         8 + rowbase + j2: 8 + rowbase + j2 + YS,
                                         8 + colbase + j1: 8 + colbase + j1 + XS]
                            out[call, 16 * G + l, :, 4 * slot + 2 * j2 + j1] = \
                                win.reshape(-1)
    return out.reshape(2, 128, LNUM * DCH)


def make_inputs(core, t1_feature, t2_feature, t2_pred, w_down1, w_down2,
                w_flow1, bn_gamma, bn_beta, bn_mean, bn_var, w_flow2):
    q, h = core // 2, core % 2
    f1 = _feat_slab(t1_feature[q], h)
    f2 = _feat_slab(t2_feature[q], h)
    wd = np.stack([
        np.stack([w_down1[:, 128 * k:128 * (k + 1), 0, 0].T for k in range(2)]),
        np.stack([w_down2[:, 128 * k:128 * (k + 1), 0, 0].T for k in range(2)]),
    ]).transpose(2, 0, 1, 3).astype(BF).copy()        # (128,2,2,64)
    wf1 = np.stack([w_flow1[:, :, t // 3, t % 3].T for t in range(9)],
                   axis=1).astype(BF).copy()          # (128,9,64)
    wf2h = np.stack([w_flow2[:, :, t // 3, t % 3].T for t in range(9)],
                    axis=1).astype(BF)                # (64,9,2)
    z = np.zeros_like(wf2h)
    wf2 = np.stack([np.concatenate([wf2h, z], axis=0),
                    np.concatenate([z, wf2h], axis=0)],
                   axis=1).copy()                     # (128,2,9,2)
    scale = bn_gamma / np.sqrt(bn_var + 1e-5)
    bias = bn_beta - bn_mean * scale
    bn1 = np.stack([scale, bias], axis=1).reshape(T, 2, 1).astype(np.float32)
    bn = np.concatenate([bn1, bn1], axis=0)           # (128,2,1)
    mask = np.ones((128, 2, 1), np.float32)
    if h == 0:
        mask[0:T, 0] = 0.0   # x row 0 (half A) = image row -1
    else:
        mask[T:128, 1] = 0.0  # x half-B row 35 = x row 65 = image row 128
    bx, by, xlo, xhi, ylo, yhi = _host_constants(q, h)
    imgs = img_list(q)
    pred_imgs = np.stack([t2_pred[n, ch] for (n, ch) in imgs])
    dsrc = _dsrc_build(pred_imgs, h)
    emat = np.zeros((8, 128), BF)
    for Gi in range(8):
        emat[Gi, 16 * Gi:16 * (Gi + 1)] = 1.0
    return {
        "f1": f1, "f2": f2, "wd": wd, "wf1": wf1, "wf2": wf2,
        "bn": bn, "mask": mask, "bx": bx, "by": by, "ylo": ylo, "yhi": yhi,
        "xlo": xlo, "xhi": xhi, "emat": emat, "dsrc": dsrc.astype(BF),
    }


_NC_CACHE = {}


def kernel(**inputs):
    from concourse.bass_utils import run_bass_kernel_spmd
    if "nc" not in _NC_CACHE:
        _NC_CACHE["nc"] = build_nc()
    nc = _NC_CACHE["nc"]
    in_maps = [make_inputs(c, **inputs) for c in range(8)]
    res = run_bass_kernel_spmd(nc, in_maps, list(range(8)))
    out = np.zeros((NB, CCLS, H, W), np.float32)
    for c in range(8):
        q, h = c // 2, c % 2
        o = res.results[c]["out"]
        for i, (n, ch) in enumerate(img_list(q)):
            out[n, ch, 64 * h:64 * (h + 1), :] = o[i]
    return out



# revision 16
# speedup vs baseline: 1.0580x; 1.0580x over previous
"""Trainium2 Bass kernel for nn_AlignModule_full (8 NeuronCores, data-parallel).

Reference computation: two 1x1 convs -> concat -> 3x3 conv + BN + ReLU ->
3x3 conv -> flow -> bilinear grid_sample warp of t2_pred, where output
channel (n, ch) is warped with flow[(3n+ch) % 4] (torch flow.repeat
semantics faithfully ported by the reference).

Sharding: core c = (q, h), q = c//2 flow batch, h = c%2 row half.
Each core computes flow(q, rows 64h..64h+64) from batch-q features, then
warps the 19 (n, ch) images with (3n+ch)%4 == q for its row half, using
only its own flow. Zero cross-core communication.

Warp implementation: per-pixel bilinear gather via gpsimd ap_gather with a
host-built d=8 interleaved source: each index fetches the full 2x2 patch
for TWO image slots at once (19 images = 16 lanes x 2 slots).
"""
import sys

for _p in ('/opt/trn_rl_repo',):
    if _p not in sys.path:
        sys.path.append(_p)

import numpy as np
import ml_dtypes

import concourse.bass as bass
import concourse.bacc as bacc
import concourse.mybir as mybir
import concourse.tile as tile

F32 = mybir.dt.float32
BF16 = mybir.dt.bfloat16
I16 = mybir.dt.int16
AF = mybir.ActivationFunctionType
ALU = mybir.AluOpType

H, W, CIN, T, CCLS, NB = 128, 256, 256, 64, 19, 4
SLAB_R = 68          # feature slab rows
WS = 258             # padded width for t/x buffers
XR = 66              # x rows total
XH = 36              # x rows per partition-half (A: 0..36, B: 30..66)
YS, XS = 76, 26      # gather slab rows/cols per (group, call=col-half)
LNUM = YS * XS       # base positions per partition
DCH = 8              # interleave chunk: 2 slots x (2x2 patch)
NIDX = 1024          # gather indices per group per call
ROWB = 6             # slab row margin before first output row of the call
COLB = 5             # slab col margin before group col block

BF = ml_dtypes.bfloat16


def img_list(q):
    return [(n, ch) for n in range(NB) for ch in range(CCLS)
            if (3 * n + ch) % 4 == q]


def build_nc():
    nc = bacc.Bacc(None, target_bir_lowering=False, debug=False)
    P = nc.declare_dram_parameter
    f1_d = P("f1", [2, 128, SLAB_R, W], BF16, isOutput=False)
    f2_d = P("f2", [2, 128, SLAB_R, W], BF16, isOutput=False)
    wd_d = P("wd", [128, 2, 2, T], BF16, isOutput=False)
    wf1_d = P("wf1", [128, 9, T], BF16, isOutput=False)
    wf2_d = P("wf2", [128, 2, 9, 2], BF16, isOutput=False)
    bn_d = P("bn", [128, 2, 1], F32, isOutput=False)
    mask_d = P("mask", [128, 2, 1], F32, isOutput=False)
    bx_d = P("bx", [128, 128], F32, isOutput=False)
    by_d = P("by", [128, 128], F32, isOutput=False)
    ylo_d = P("ylo", [128, 128], F32, isOutput=False)
    yhi_d = P("yhi", [128, 128], F32, isOutput=False)
    xlo_d = P("xlo", [128, 128], F32, isOutput=False)
    xhi_d = P("xhi", [128, 128], F32, isOutput=False)
    emat_d = P("emat", [8, 128], BF16, isOutput=False)
    dsrc_d = P("dsrc", [2, 128, LNUM * DCH], BF16, isOutput=False)
    out_d = P("out", [CCLS, 64, W], F32, isOutput=True)

    flow_dramh = [nc.dram_tensor("flow_t0", [2, W, 32], F32),
                  nc.dram_tensor("flow_t1", [2, W, 32], F32)]  # (ch, col, row-half)

    NRB = 4               # feature rows per DMA batch
    NBATCH = (SLAB_R + NRB - 1) // NRB  # 12 (last batch 2 rows)

    with tile.TileContext(nc) as tc:
        with (
            tc.tile_pool(name="stream", bufs=3) as sp,
            tc.tile_pool(name="big", bufs=1) as bp,
            tc.tile_pool(name="psA", bufs=2, space="PSUM") as pp,
        ):
            # ---- feature batches: [128, feat, ck, NRB, W] bf16, 2-deep ring
            fts = {}

            def load_batch(b):
                r0 = NRB * b
                nr = min(NRB, SLAB_R - r0)
                ft = bp.tile([128, 2, 2, NRB, W], BF16, tag="fbatch",
                             name=f"fb{b}", bufs=2)
                for fi, fd in ((0, f1_d), (1, f2_d)):
                    src = bass.AP(tensor=fd, offset=r0 * W,
                                  ap=[[SLAB_R * W, 128], [128 * SLAB_R * W, 2],
                                      [W, nr], [1, W]])
                    (nc.sync if fi == 0 else nc.scalar).dma_start(
                        ft[:, fi, :, 0:nr, :], src)
                fts[b] = ft

            load_batch(0)

            # ---- constants (spread across both HWDGE queues) ----
            wd_s = bp.tile([128, 2, 2, T], BF16, tag="wd")
            wf1_s = bp.tile([128, 9, T], BF16, tag="wf1")
            wf2_s = bp.tile([128, 2, 9, 2], BF16, tag="wf2")
            bn_s = bp.tile([128, 2, 1], F32, tag="bn")
            mask_s = bp.tile([128, 2, 1], F32, tag="mask")
            xlo_s = bp.tile([128, 128], F32, tag="xlo")
            xhi_s = bp.tile([128, 128], F32, tag="xhi")
            emat_s = bp.tile([8, 128], BF16, tag="emat")
            bx_s = bp.tile([128, 128], F32, tag="bx")
            by_s = bp.tile([128, 128], F32, tag="by")
            ylo_s = bp.tile([128, 128], F32, tag="ylo")
            yhi_s = bp.tile([128, 128], F32, tag="yhi")
            for i, (t_, d_) in enumerate((
                    (wd_s, wd_d), (wf1_s, wf1_d), (wf2_s, wf2_d),
                    (bn_s, bn_d), (mask_s, mask_d), (xlo_s, xlo_d),
                    (xhi_s, xhi_d),
                    (emat_s, emat_d), (bx_s, bx_d), (by_s, by_d),
                    (ylo_s, ylo_d), (yhi_s, yhi_d))):
                (nc.sync if i % 2 else nc.scalar).dma_start(t_[:], d_[:])

            # ---- big shared tiles; gather sources loaded up front (SWDGE) ----
            t_cat = bp.tile([128, SLAB_R * WS], BF16, tag="tcat_gat")
            dsrc = bp.tile([128, LNUM * DCH], BF16, tag="dsrc")
            dsrc2 = bp.tile([128, LNUM * DCH], BF16, tag="dsrc2")
            nc.gpsimd.dma_start(dsrc[:], dsrc_d[0, :, :])
            nc.gpsimd.dma_start(dsrc2[:], dsrc_d[1, :, :])
            x_sb = bp.tile([128, XH * WS], BF16, tag="x_w4")

            load_batch(1)

            t3 = t_cat[:].rearrange("p (r c) -> p r c", r=SLAB_R, c=WS)
            nc.vector.memset(t3[:, :, 0:1], 0.0)
            nc.vector.memset(t3[:, :, 257:258], 0.0)

            # ---- phases 1+2 interleaved: 1x1 convs feed 3x3 conv ----
            def p1_tile(it):
                r0 = 2 * it
                b, rr = r0 // NRB, r0 % NRB
                if rr == 0 and b + 1 < NBATCH and (b + 1) not in fts:
                    load_batch(b + 1)
                ft = fts[b]
                ps = pp.tile([128, 2 * W], F32, tag="pst", name="pst")
                for ck in range(2):
                    nc.tensor.matmul(ps[0:T, :], wd_s[:, 0, ck, :],
                                     ft[:, 0, ck, rr:rr + 2, :],
                                     start=(ck == 0), stop=(ck == 1),
                                     tile_position=(0, 0),
                                     skip_group_check=True)
                    nc.tensor.matmul(ps[T:128, :], wd_s[:, 1, ck, :],
                                     ft[:, 1, ck, rr:rr + 2, :],
                                     start=(ck == 0), stop=(ck == 1),
                                     tile_position=(0, 64),
                                     skip_group_check=True)
                dst = bass.AP(tensor=t_cat.tensor, offset=r0 * WS + 1,
                              ap=[[SLAB_R * WS, 128], [WS, 2], [1, W]])
                nc.vector.tensor_copy(dst, ps[:].rearrange("p (r c) -> p r c",
                                                           r=2, c=W))

            x3 = x_sb[:].rearrange("p (r c) -> p r c", r=XH, c=WS)
            nc.vector.memset(x3[:, :, 0:1], 0.0)
            nc.vector.memset(x3[:, :, 257:258], 0.0)

            def p2_iter(it):
                jA = 2 * it
                jB = 30 + 2 * it
                ps = pp.tile([128, 2 * W], F32, tag="psx", name="psx")
                for tap in range(9):
                    dy, dx = tap // 3, tap % 3
                    rhsA = bass.AP(tensor=t_cat.tensor,
                                   offset=(jA + dy) * WS + dx,
                                   ap=[[SLAB_R * WS, 128], [WS, 2], [1, W]])
                    rhsB = bass.AP(tensor=t_cat.tensor,
                                   offset=(jB + dy) * WS + dx,
                                   ap=[[SLAB_R * WS, 128], [WS, 2], [1, W]])
                    nc.tensor.matmul(ps[0:T, :], wf1_s[:, tap, :], rhsA,
                                     start=(tap == 0), stop=(tap == 8),
                                     tile_position=(0, 0),
                                     skip_group_check=True)
                    nc.tensor.matmul(ps[T:128, :], wf1_s[:, tap, :], rhsB,
                                     start=(tap == 0), stop=(tap == 8),
                                     tile_position=(0, 64),
                                     skip_group_check=True)
                dstA = bass.AP(tensor=x_sb.tensor, offset=jA * WS + 1,
                               ap=[[XH * WS, T], [WS, 2], [1, W]])
                dstB = bass.AP(tensor=x_sb.tensor,
                               offset=T * (XH * WS) + jA * WS + 1,
                               ap=[[XH * WS, T], [WS, 2], [1, W]])
                nc.scalar.activation(dstA,
                                     ps[0:T].rearrange("p (r c) -> p r c", r=2, c=W),
                                     AF.Relu, bias=bn_s[0:T, 1], scale=bn_s[0:T, 0])
                nc.scalar.activation(dstB,
                                     ps[T:128].rearrange("p (r c) -> p r c", r=2, c=W),
                                     AF.Relu, bias=bn_s[T:128, 1], scale=bn_s[T:128, 0])

            for it in range(18):
                p1_tile(it)
            for it in range(18):
                p2_iter(it)
                if 18 + it < SLAB_R // 2:
                    p1_tile(18 + it)
            nc.vector.tensor_scalar_mul(x3[0:T, 0, :], x3[0:T, 0, :], mask_s[0:T, 0])
            nc.vector.tensor_scalar_mul(x3[T:128, 35, :], x3[T:128, 35, :],
                                        mask_s[T:128, 1])

            # ---- phase 3: 3x3 conv 64->2, two tiles concurrent via PE
            # column strips.  First 8 pairs cover flow rows 0..31 (rh0) so
            # the warp pipeline for rh0 can start while rh1 still computes.
            def p3_pair(iA, iB):
                # PE column strips: out PSUM start partition must equal the
                # tile-position column, so pos-1 writes partitions 64:66.
                tiles = [(iA, 0)] + ([(iB, 1)] if iB is not None else [])
                pss = []
                for i0, pos in tiles:
                    t_ = pp.tile([128, 2 * W], F32,
                                 tag=("psf" if pos == 0 else "psfB"),
                                 name="psf", bufs=2)
                    pss.append(t_[64 * pos:64 * pos + 2])
                for tap in range(9):
                    dy, dx = tap // 3, tap % 3
                    for (i0, pos), ps in zip(tiles, pss):
                        hf = 0 if i0 < 34 else 1
                        base = i0 + dy - 30 * hf
                        rhs = bass.AP(tensor=x_sb.tensor,
                                      offset=base * WS + dx,
                                      ap=[[XH * WS, 128], [WS, 2], [1, W]])
                        nc.tensor.matmul(ps, wf2_s[:, hf, tap, :], rhs,
                                         start=(tap == 0), stop=(tap == 8),
                                         tile_position=(0, 64 * pos),
                                         skip_group_check=True)
                for (i0, pos), ps in zip(tiles, pss):
                    bt = sp.tile([2, 2 * W], F32, tag="bt", name="bt", bufs=8)
                    bt_v = bass.AP(tensor=bt.tensor, offset=0,
                                   ap=[[2 * W, 2], [1, 2], [2, W]])
                    src = ps.rearrange("p (r c) -> p r c", r=2, c=W)
                    if pos == 0:
                        nc.vector.tensor_copy(bt_v, src)
                    else:
                        nc.scalar.copy(bt_v, src)
                    dst = bass.AP(tensor=flow_dramh[i0 // 32], offset=i0 % 32,
                                  ap=[[W * 32, 2], [32, W], [1, 2]])
                    (nc.sync if pos == 0 else nc.scalar).dma_start(dst, bt[:])

            p3_pairs_rh0 = [(2 * i, 16 + 2 * i) for i in range(8)]
            p3_pairs_rh1 = [(32, 34)] + [(36 + 4 * i, 38 + 4 * i)
                                         for i in range(7)]

            # ---- phase 4/5: flow -> CL + index math + gathers, by row half ----
            cl_fx = bp.tile([128, 128], F32, tag="clfx")
            cl_fy = bp.tile([128, 128], F32, tag="clfy")

            def cl(tag):
                tt = bp.tile([128, 128], F32, tag=tag, name=tag)
                return tt

            ix = cl("ix"); iy = cl("iy"); tmp = cl("tmp")
            x0i = bp.tile([128, 128], I16, tag="x0i")
            y0i = bp.tile([128, 128], I16, tag="y0i")
            x0f = cl("x0f"); y0f = cl("y0f")
            ef = cl("ef")
            eidx = bp.tile([128, 128], I16, tag="eidx")
            gatall = bp.tile([128, 2 * NIDX * DCH], BF16, tag="tcat_gat")
            _qs = [nc.sync, nc.scalar]
            _qi = 0

            def cl_load(rh):

                for ch, dtile in ((0, cl_fx), (1, cl_fy)):
                    for G in range(8):
                        for w in range(2):
                            dst = bass.AP(tensor=dtile.tensor,
                                          offset=(16 * G) * 128 + 64 * w + 32 * rh,
                                          ap=[[128, 16], [1, 32]])
                            srcp = bass.AP(
                                tensor=flow_dramh[rh],
                                offset=ch * W * 32 + (32 * G + 16 * w) * 32,
                                ap=[[32, 16], [1, 32]])
                            _qs[(ch + G + w) % 2].dma_start(dst, srcp)

            def idx_math(sl):
                V = nc.vector
                V.tensor_scalar_mul(ix[:, sl], cl_fx[:, sl], 0.5)
                V.tensor_tensor(ix[:, sl], ix[:, sl], bx_s[:, sl], ALU.add)
                V.tensor_scalar_mul(iy[:, sl], cl_fy[:, sl], 0.5)
                V.tensor_tensor(iy[:, sl], iy[:, sl], by_s[:, sl], ALU.add)
                V.tensor_copy(x0i[:, sl], ix[:, sl])
                V.tensor_copy(x0f[:, sl], x0i[:, sl])
                V.tensor_tensor(tmp[:, sl], x0f[:, sl], ix[:, sl], ALU.is_gt)
                V.tensor_tensor(x0f[:, sl], x0f[:, sl], tmp[:, sl], ALU.subtract)
                V.tensor_copy(y0i[:, sl], iy[:, sl])
                V.tensor_copy(y0f[:, sl], y0i[:, sl])
                V.tensor_tensor(tmp[:, sl], y0f[:, sl], iy[:, sl], ALU.is_gt)
                V.tensor_tensor(y0f[:, sl], y0f[:, sl], tmp[:, sl], ALU.subtract)
                V.tensor_scalar_mul(ef[:, sl], y0f[:, sl], float(XS))
                V.tensor_tensor(ef[:, sl], ef[:, sl], x0f[:, sl], ALU.add)
                V.tensor_scalar(ef[:, sl], ef[:, sl], 0.0, float(LNUM - XS - 2),
                                ALU.max, ALU.min)
                V.tensor_copy(eidx[:, sl], ef[:, sl])

            def warp_front(rh):
                cl_load(rh)
                for w in range(2):
                    sl = slice(64 * w + 32 * rh, 64 * w + 32 * rh + 32)
                    idx_math(sl)
                    ds = dsrc if w == 0 else dsrc2
                    off = w * (NIDX * DCH) + rh * 4096
                    nc.gpsimd.ap_gather(
                        gatall[:, off:off + 4096], ds[:],
                        eidx[:, sl],
                        channels=128, num_elems=LNUM, d=DCH, num_idxs=512)

            for a, b in p3_pairs_rh0:
                p3_pair(a, b)
            warp_front(0)
            for a, b in p3_pairs_rh1:
                p3_pair(a, b)
            warp_front(1)

            # ---- weights math (full tensors) ----
            fx = cl("fx"); fy = cl("fy")
            nc.vector.tensor_tensor(fx[:], ix[:], x0f[:], ALU.subtract)
            nc.vector.tensor_tensor(fy[:], iy[:], y0f[:], ALU.subtract)
            vx0 = cl("vx0"); vx1 = cl("vx1"); vy0 = cl("vy0"); vy1 = cl("vy1")
            xp1 = cl("ix"); yp1 = cl("iy")   # reuse dead buffers (WAR-tracked)
            nc.vector.tensor_scalar_add(xp1[:], x0f[:], 1.0)
            nc.vector.tensor_scalar_add(yp1[:], y0f[:], 1.0)
            for vt, src_f in ((vx0, x0f), (vx1, xp1)):
                nc.vector.tensor_tensor(vt[:], src_f[:], xlo_s[:], ALU.is_ge)
                nc.vector.tensor_tensor(tmp[:], src_f[:], xhi_s[:], ALU.is_le)
                nc.vector.tensor_tensor(vt[:], vt[:], tmp[:], ALU.mult)
            for vt, src_f in ((vy0, y0f), (vy1, yp1)):
                nc.vector.tensor_tensor(vt[:], src_f[:], ylo_s[:], ALU.is_ge)
                nc.vector.tensor_tensor(tmp[:], src_f[:], yhi_s[:], ALU.is_le)
                nc.vector.tensor_tensor(vt[:], vt[:], tmp[:], ALU.mult)
            gx0 = cl("x0f"); gx1 = cl("y0f"); gy0 = cl("vx0"); gy1 = cl("vx1")
            nc.vector.tensor_scalar(tmp[:], fx[:], -1.0, 1.0, ALU.mult, ALU.add)
            nc.vector.tensor_tensor(gx0[:], tmp[:], vx0[:], ALU.mult)
            nc.vector.tensor_tensor(gx1[:], fx[:], vx1[:], ALU.mult)
            nc.vector.tensor_scalar(tmp[:], fy[:], -1.0, 1.0, ALU.mult, ALU.add)
            nc.vector.tensor_tensor(gy0[:], tmp[:], vy0[:], ALU.mult)
            nc.vector.tensor_tensor(gy1[:], fy[:], vy1[:], ALU.mult)
            wsall = bp.tile([128, 4, 128], BF16, tag="wsall")
            nc.vector.tensor_tensor(wsall[:, 0, :], gx0[:], gy0[:], ALU.mult)
            nc.vector.tensor_tensor(wsall[:, 1, :], gx1[:], gy0[:], ALU.mult)
            nc.vector.tensor_tensor(wsall[:, 2, :], gx0[:], gy1[:], ALU.mult)
            nc.vector.tensor_tensor(wsall[:, 3, :], gx1[:], gy1[:], ALU.mult)
            # ---- phase 6: weight planes -> w_g (SBUF->SBUF) -> W4 ----
            w_g = bp.tile([8, 4, 2048], BF16, tag="wg")
            for s in range(4):
                (nc.sync if s % 2 else nc.scalar).dma_start(
                    bass.AP(tensor=w_g.tensor, offset=s * 2048,
                            ap=[[4 * 2048, 8], [128, 16], [1, 128]]),
                    wsall[:, s, :])
            # W4 [128, 4, 2048] j-ordered (j = 32r + 16w + m over full 64 rows)
            w4 = bp.tile([128, 4 * 2048], BF16, tag="x_w4")
            for s in range(4):
                for c4 in range(4):
                    pw = pp.tile([128, 512], F32, tag="pst", name="pw")
                    nc.tensor.matmul(pw[:], emat_s[:], w_g[:, s, 512 * c4:512 * (c4 + 1)],
                                     start=True, stop=True)
                    # pw free = (m-part: 128,4)(r: 2,64)(w: 1,2) for m in [4c4, 4c4+4)
                    dstw = bass.AP(tensor=w4.tensor,
                                   offset=s * 2048 + 4 * c4,
                                   ap=[[4 * 2048, 128], [1, 4], [1024, 2], [16, 64]])
                    nc.scalar.copy(
                        dstw,
                        pw[:].rearrange("p (m w r) -> p m w r", m=4, w=2, r=64))

            # Two independent combine chains: call 0 on VectorE, call 1 on
            # GpSimd, each with its own scratch so they run concurrently.
            # pls reuse the gather-source buffers (dead after the last gather)
            pls_c = [bp.tile([128, 4, NIDX], BF16, tag=("dsrc", "dsrc2")[c],
                             name=f"pls{c}")
                     for c in range(2)]
            bb_c = [bp.tile([128, 2, NIDX], F32, tag=f"bbc{c}", name=f"bbc{c}")
                    for c in range(2)]
            for call in range(2):
                eng = nc.vector if call == 0 else nc.gpsimd
                pls, bbt = pls_c[call], bb_c[call]
                for slot in range(2):
                    for s in range(4):
                        g_v = bass.AP(tensor=gatall.tensor,
                                      offset=call * NIDX * DCH + 4 * slot + s,
                                      ap=[[2 * NIDX * DCH, 128], [DCH, NIDX]])
                        eng.tensor_tensor(
                            pls[:, s, :], g_v,
                            w4[:, (s * 2048 + 1024 * call):(s * 2048 + 1024 * call + NIDX)],
                            ALU.mult)
                    eng.tensor_tensor(pls[:, 0, :], pls[:, 0, :], pls[:, 1, :],
                                      ALU.add)
                    eng.tensor_tensor(pls[:, 2, :], pls[:, 2, :], pls[:, 3, :],
                                      ALU.add)
                    eng.tensor_tensor(bbt[:, slot, :], pls[:, 0, :], pls[:, 2, :],
                                      ALU.add)
                    nl = 16 if slot == 0 else 3
                    for G in range(8):
                        dst = bass.AP(
                            tensor=out_d,
                            offset=(16 * slot) * 64 * W + 32 * G + 16 * call,
                            ap=[[64 * W, nl], [W, 64], [1, 16]])
                        srcb = bass.AP(
                            tensor=bbt.tensor,
                            offset=(16 * G) * (2 * NIDX) + slot * NIDX,
                            ap=[[2 * NIDX, nl], [16, 64], [1, 16]])
                        (nc.sync if call == 0 else nc.scalar).dma_start(dst, srcb)
    nc.finalize()
    return nc


# ======================= host-side prep =======================

def _feat_slab(feat_b, h):
    """feat_b (256, 128, 256) f32 -> (2, 128, 68, 256) bf16 slab for half h."""
    r0 = 64 * h - 2
    slab = np.zeros((CIN, SLAB_R, W), np.float32)
    lo, hi = max(r0, 0), min(r0 + SLAB_R, H)
    slab[:, lo - r0:hi - r0, :] = feat_b[:, lo:hi, :]
    return np.ascontiguousarray(
        slab.reshape(2, 128, SLAB_R, W).astype(BF))


def _host_constants(q, h):
    R0 = 64 * h
    # CL layout: p = 16G + m, f = 64w + r; pixel (row R0+r, col 32G+16w+m)
    p = np.arange(128)[:, None]
    f = np.arange(128)[None, :]
    G = p // 16
    m = p % 16
    r = f % 64
    w = f // 64
    col = 32 * G + 16 * w + m
    row = R0 + r
    ix_base = col + col / (W - 1.0) - 0.5
    iy_base = row + row / (H - 1.0) - 0.5
    colbase = 32 * G + 16 * w - COLB
    rowbase = R0 - ROWB
    bx = np.broadcast_to(ix_base - colbase, (128, 128)).astype(np.float32).copy()
    by = np.broadcast_to(iy_base - rowbase, (128, 128)).astype(np.float32).copy()
    xlo = np.broadcast_to(0.0 - colbase, (128, 128)).astype(np.float32).copy()
    xhi = np.broadcast_to((W - 1.0) - colbase, (128, 128)).astype(np.float32).copy()
    ylo = np.full((128, 128), 0.0 - rowbase, np.float32)
    yhi = np.full((128, 128), (H - 1.0) - rowbase, np.float32)
    return bx, by, xlo, xhi, ylo, yhi


def _dsrc_build(pred_imgs, h):
    """pred_imgs: (19, 128, 256) f32. Returns (2, 128, LNUM*8) f32 gather
    source; call = col-half w, slab = rows [R0-6, R0+70) x 26-col band."""
    R0 = 64 * h
    padded = np.zeros((CCLS, H + 16, W + 16), np.float32)
    padded[:, 8:8 + H, 8:8 + W] = pred_imgs
    out = np.zeros((2, 128, LNUM, DCH), np.float32)
    rowbase = R0 - ROWB
    for call in range(2):
        for G in range(8):
            colbase = 32 * G + 16 * call - COLB
            for l in range(16):
                for slot in range(2):
                    img = l + 16 * slot
                    if img >= CCLS:
                        img = l
                    for j2 in range(2):
                        for j1 in range(2):
                            win = padded[img,
                                         8 + rowbase + j2: 8 + rowbase + j2 + YS,
                                         8 + colbase + j1: 8 + colbase + j1 + XS]
                            out[call, 16 * G + l, :, 4 * slot + 2 * j2 + j1] = \
                                win.reshape(-1)
    return out.reshape(2, 128, LNUM * DCH)


def make_inputs(core, t1_feature, t2_feature, t2_pred, w_down1, w_down2,
                w_flow1, bn_gamma, bn_beta, bn_mean, bn_var, w_flow2):
    q, h = core // 2, core % 2
    f1 = _feat_slab(t1_feature[q], h)
    f2 = _feat_slab(t2_feature[q], h)
    wd = np.stack([
        np.stack([w_down1[:, 128 * k:128 * (k + 1), 0, 0].T for k in range(2)]),
        np.stack([w_down2[:, 128 * k:128 * (k + 1), 0, 0].T for k in range(2)]),
    ]).transpose(2, 0, 1, 3).astype(BF).copy()        # (128,2,2,64)
    wf1 = np.stack([w_flow1[:, :, t // 3, t % 3].T for t in range(9)],
                   axis=1).astype(BF).copy()          # (128,9,64)
    wf2h = np.stack([w_flow2[:, :, t // 3, t % 3].T for t in range(9)],
                    axis=1).astype(BF)                # (64,9,2)
    z = np.zeros_like(wf2h)
    wf2 = np.stack([np.concatenate([wf2h, z], axis=0),
                    np.concatenate([z, wf2h], axis=0)],
                   axis=1).copy()                     # (128,2,9,2)
    scale = bn_gamma / np.sqrt(bn_var + 1e-5)
    bias = bn_beta - bn_mean * scale
    bn1 = np.stack([scale, bias], axis=1).reshape(T, 2, 1).astype(np.float32)
    bn = np.concatenate([bn1, bn1], axis=0)           # (128,2,1)
    mask = np.ones((128, 2, 1), np.float32)
    if h == 0:
        mask[0:T, 0] = 0.0   # x row 0 (half A) = image row -1
    else:
        mask[T:128, 1] = 0.0  # x half-B row 35 = x row 65 = image row 128
    bx, by, xlo, xhi, ylo, yhi = _host_constants(q, h)
    imgs = img_list(q)
    pred_imgs = np.stack([t2_pred[n, ch] for (n, ch) in imgs])
    dsrc = _dsrc_build(pred_imgs, h)
    emat = np.zeros((8, 128), BF)
    for Gi in range(8):
        emat[Gi, 16 * Gi:16 * (Gi + 1)] = 1.0
    return {
        "f1": f1, "f2": f2, "wd": wd, "wf1": wf1, "wf2": wf2,
        "bn": bn, "mask": mask, "bx": bx, "by": by, "ylo": ylo, "yhi": yhi,
        "xlo": xlo, "xhi": xhi, "emat": emat, "dsrc": dsrc.astype(BF),
    }


_NC_CACHE = {}


def kernel(**inputs):
    from concourse.bass_utils import run_bass_kernel_spmd
    if "nc" not in _NC_CACHE:
        _NC_CACHE["nc"] = build_nc()
    nc = _NC_CACHE["nc"]
    in_maps = [make_inputs(c, **inputs) for c in range(8)]
    res = run_bass_kernel_spmd(nc, in_maps, list(range(8)))
    out = np.zeros((NB, CCLS, H, W), np.float32)
    for c in range(8):
        q, h = c // 2, c % 2
        o = res.results[c]["out"]
        for i, (n, ch) in enumerate(img_list(q)):
            out[n, ch, 64 * h:64 * (h + 1), :] = o[i]
    return out



# revision 25
# speedup vs baseline: 1.1471x; 1.0842x over previous
"""Trainium2 Bass kernel for nn_AlignModule_full (8 NeuronCores, data-parallel).

Reference computation: two 1x1 convs -> concat -> 3x3 conv + BN + ReLU ->
3x3 conv -> flow -> bilinear grid_sample warp of t2_pred, where output
channel (n, ch) is warped with flow[(3n+ch) % 4] (torch flow.repeat
semantics faithfully ported by the reference).

Sharding: core c = (q, h), q = c//2 flow batch, h = c%2 row half.
Each core computes flow(q, rows 64h..64h+64) from batch-q features, then
warps the 19 (n, ch) images with (3n+ch)%4 == q for its row half, using
only its own flow. Zero cross-core communication.

Warp implementation: per-pixel bilinear gather via gpsimd ap_gather with a
host-built d=8 interleaved source: each index fetches the full 2x2 patch
for TWO image slots at once (19 images = 16 lanes x 2 slots).
"""
import sys

for _p in ('/opt/trn_rl_repo',):
    if _p not in sys.path:
        sys.path.append(_p)

import numpy as np
import ml_dtypes

import concourse.bass as bass
import concourse.bacc as bacc
import concourse.mybir as mybir
import concourse.tile as tile

F32 = mybir.dt.float32
BF16 = mybir.dt.bfloat16
I16 = mybir.dt.int16
AF = mybir.ActivationFunctionType
ALU = mybir.AluOpType

H, W, CIN, T, CCLS, NB = 128, 256, 256, 64, 19, 4
SLAB_R = 68          # feature slab rows
WS = 258             # padded width for t/x buffers
XR = 66              # x rows total
XH = 36              # x rows per partition-half (A: 0..36, B: 30..66)
YS, XS = 76, 26      # gather slab rows/cols per (group, call=col-half)
LNUM = YS * XS       # base positions per partition
DCH = 8              # interleave chunk: 2 slots x (2x2 patch)
NIDX = 1024          # gather indices per group per call
ROWB = 6             # slab row margin before first output row of the call
COLB = 5             # slab col margin before group col block

BF = ml_dtypes.bfloat16


def img_list(q):
    return [(n, ch) for n in range(NB) for ch in range(CCLS)
            if (3 * n + ch) % 4 == q]


def build_nc():
    nc = bacc.Bacc(None, target_bir_lowering=False, debug=False)
    P = nc.declare_dram_parameter
    f1_d = P("f1", [2, 128, SLAB_R, W], BF16, isOutput=False)
    f2_d = P("f2", [2, 128, SLAB_R, W], BF16, isOutput=False)
    wd_d = P("wd", [128, 2, 2, T], BF16, isOutput=False)
    wf1_d = P("wf1", [128, 9, T], BF16, isOutput=False)
    wf2_d = P("wf2", [128, 2, 9, 2], BF16, isOutput=False)
    bn_d = P("bn", [128, 2, 1], F32, isOutput=False)
    mask_d = P("mask", [128, 2, 1], F32, isOutput=False)
    bx_d = P("bx", [128, 128], F32, isOutput=False)
    by_d = P("by", [128, 128], F32, isOutput=False)
    ylo_d = P("ylo", [128, 128], F32, isOutput=False)
    yhi_d = P("yhi", [128, 128], F32, isOutput=False)
    xlo_d = P("xlo", [128, 128], F32, isOutput=False)
    xhi_d = P("xhi", [128, 128], F32, isOutput=False)
    emat_d = P("emat", [8, 128], BF16, isOutput=False)
    dsrc_d = P("dsrc", [2, 128, LNUM * DCH], BF16, isOutput=False)
    out_d = P("out", [CCLS, 64, W], F32, isOutput=True)

    flow_dramh = [nc.dram_tensor("flow_t0", [2, W, 32], BF16),
                  nc.dram_tensor("flow_t1", [2, W, 32], BF16)]  # (ch, col, row-half)

    NRB = 4               # feature rows per DMA batch
    NBATCH = (SLAB_R + NRB - 1) // NRB  # 12 (last batch 2 rows)

    with tile.TileContext(nc) as tc:
        with (
            tc.tile_pool(name="stream", bufs=3) as sp,
            tc.tile_pool(name="big", bufs=1) as bp,
            tc.tile_pool(name="psA", bufs=2, space="PSUM") as pp,
        ):
            # ---- feature batches: [128, feat, ck, NRB, W] bf16, 2-deep ring
            fts = {}

            def load_batch(b):
                r0 = NRB * b
                nr = min(NRB, SLAB_R - r0)
                ft = bp.tile([128, 2, 2, NRB, W], BF16, tag="fbatch",
                             name=f"fb{b}", bufs=2)
                for fi, fd in ((0, f1_d), (1, f2_d)):
                    src = bass.AP(tensor=fd, offset=r0 * W,
                                  ap=[[SLAB_R * W, 128], [128 * SLAB_R * W, 2],
                                      [W, nr], [1, W]])
                    (nc.sync if fi == 0 else nc.scalar).dma_start(
                        ft[:, fi, :, 0:nr, :], src)
                fts[b] = ft

            load_batch(0)

            # ---- constants (spread across both HWDGE queues) ----
            wd_s = bp.tile([128, 2, 2, T], BF16, tag="wd")
            wf1_s = bp.tile([128, 9, T], BF16, tag="wf1")
            wf2_s = bp.tile([128, 2, 9, 2], BF16, tag="wf2")
            bn_s = bp.tile([128, 2, 1], F32, tag="bn")
            mask_s = bp.tile([128, 2, 1], F32, tag="mask")
            xlo_s = bp.tile([128, 128], F32, tag="xlo")
            xhi_s = bp.tile([128, 128], F32, tag="xhi")
            emat_s = bp.tile([8, 128], BF16, tag="emat")
            bx_s = bp.tile([128, 128], F32, tag="bx")
            by_s = bp.tile([128, 128], F32, tag="by")
            ylo_s = bp.tile([128, 128], F32, tag="ylo")
            yhi_s = bp.tile([128, 128], F32, tag="yhi")
            for i, (t_, d_) in enumerate((
                    (wd_s, wd_d), (wf1_s, wf1_d), (wf2_s, wf2_d),
                    (bn_s, bn_d), (mask_s, mask_d), (xlo_s, xlo_d),
                    (xhi_s, xhi_d),
                    (emat_s, emat_d), (bx_s, bx_d), (by_s, by_d),
                    (ylo_s, ylo_d), (yhi_s, yhi_d))):
                (nc.sync if i % 2 else nc.scalar).dma_start(t_[:], d_[:])

            # ---- big shared tiles; gather sources loaded up front (SWDGE) ----
            t_cat = bp.tile([128, SLAB_R * WS], BF16, tag="tcat_gat")
            dsrc = bp.tile([128, LNUM * DCH], BF16, tag="dsrc")
            dsrc2 = bp.tile([128, LNUM * DCH], BF16, tag="dsrc2")
            nc.gpsimd.dma_start(dsrc[:], dsrc_d[0, :, :])
            # dsrc2 is loaded later (mid-P2) to keep HBM free for features.
            x_sb = bp.tile([128, XH * WS], BF16, tag="x_w4")

            load_batch(1)

            t3 = t_cat[:].rearrange("p (r c) -> p r c", r=SLAB_R, c=WS)
            nc.vector.memset(t3[:, :, 0:1], 0.0)
            nc.vector.memset(t3[:, :, 257:258], 0.0)

            # ---- phases 1+2 interleaved: 1x1 convs feed 3x3 conv ----
            def p1_tile(it):
                r0 = 2 * it
                b, rr = r0 // NRB, r0 % NRB
                if rr == 0 and b + 1 < NBATCH and (b + 1) not in fts:
                    load_batch(b + 1)
                ft = fts[b]
                ps = pp.tile([128, 2 * W], F32, tag="pst", name="pst")
                for ck in range(2):
                    nc.tensor.matmul(ps[0:T, :], wd_s[:, 0, ck, :],
                                     ft[:, 0, ck, rr:rr + 2, :],
                                     start=(ck == 0), stop=(ck == 1),
                                     tile_position=(0, 0),
                                     skip_group_check=True)
                    nc.tensor.matmul(ps[T:128, :], wd_s[:, 1, ck, :],
                                     ft[:, 1, ck, rr:rr + 2, :],
                                     start=(ck == 0), stop=(ck == 1),
                                     tile_position=(0, 64),
                                     skip_group_check=True)
                dst = bass.AP(tensor=t_cat.tensor, offset=r0 * WS + 1,
                              ap=[[SLAB_R * WS, 128], [WS, 2], [1, W]])
                nc.vector.tensor_copy(dst, ps[:].rearrange("p (r c) -> p r c",
                                                           r=2, c=W))

            x3 = x_sb[:].rearrange("p (r c) -> p r c", r=XH, c=WS)
            nc.vector.memset(x3[:, :, 0:1], 0.0)
            nc.vector.memset(x3[:, :, 257:258], 0.0)

            def p2_iter(it):
                jA = 2 * it
                jB = 30 + 2 * it
                ps = pp.tile([128, 2 * W], F32, tag="psx", name="psx")
                for tap in range(9):
                    dy, dx = tap // 3, tap % 3
                    rhsA = bass.AP(tensor=t_cat.tensor,
                                   offset=(jA + dy) * WS + dx,
                                   ap=[[SLAB_R * WS, 128], [WS, 2], [1, W]])
                    rhsB = bass.AP(tensor=t_cat.tensor,
                                   offset=(jB + dy) * WS + dx,
                                   ap=[[SLAB_R * WS, 128], [WS, 2], [1, W]])
                    nc.tensor.matmul(ps[0:T, :], wf1_s[:, tap, :], rhsA,
                                     start=(tap == 0), stop=(tap == 8),
                                     tile_position=(0, 0),
                                     skip_group_check=True)
                    nc.tensor.matmul(ps[T:128, :], wf1_s[:, tap, :], rhsB,
                                     start=(tap == 0), stop=(tap == 8),
                                     tile_position=(0, 64),
                                     skip_group_check=True)
                dstA = bass.AP(tensor=x_sb.tensor, offset=jA * WS + 1,
                               ap=[[XH * WS, T], [WS, 2], [1, W]])
                dstB = bass.AP(tensor=x_sb.tensor,
                               offset=T * (XH * WS) + jA * WS + 1,
                               ap=[[XH * WS, T], [WS, 2], [1, W]])
                nc.scalar.activation(dstA,
                                     ps[0:T].rearrange("p (r c) -> p r c", r=2, c=W),
                                     AF.Relu, bias=bn_s[0:T, 1], scale=bn_s[0:T, 0])
                nc.scalar.activation(dstB,
                                     ps[T:128].rearrange("p (r c) -> p r c", r=2, c=W),
                                     AF.Relu, bias=bn_s[T:128, 1], scale=bn_s[T:128, 0])

            for it in range(18):
                p1_tile(it)
            nc.gpsimd.dma_start(dsrc2[:], dsrc_d[1, :, :])
            for it in range(18):
                p2_iter(it)
                if 18 + it < SLAB_R // 2:
                    p1_tile(18 + it)
            nc.vector.tensor_scalar_mul(x3[0:T, 0, :], x3[0:T, 0, :], mask_s[0:T, 0])
            nc.vector.tensor_scalar_mul(x3[T:128, 35, :], x3[T:128, 35, :],
                                        mask_s[T:128, 1])

            # ---- phase 3: 3x3 conv 64->2, two tiles concurrent via PE
            # column strips.  First 8 pairs cover flow rows 0..31 (rh0) so
            # the warp pipeline for rh0 can start while rh1 still computes.
            def p3_pair(iA, iB):
                # PE column strips: out PSUM start partition must equal the
                # tile-position column, so pos-1 writes partitions 64:66.
                tiles = [(iA, 0)] + ([(iB, 1)] if iB is not None else [])
                pss = []
                for i0, pos in tiles:
                    t_ = pp.tile([128, 2 * W], F32,
                                 tag=("psf" if pos == 0 else "psfB"),
                                 name="psf", bufs=2)
                    pss.append(t_[64 * pos:64 * pos + 2])
                for tap in range(9):
                    dy, dx = tap // 3, tap % 3
                    for (i0, pos), ps in zip(tiles, pss):
                        hf = 0 if i0 < 34 else 1
                        base = i0 + dy - 30 * hf
                        rhs = bass.AP(tensor=x_sb.tensor,
                                      offset=base * WS + dx,
                                      ap=[[XH * WS, 128], [WS, 2], [1, W]])
                        nc.tensor.matmul(ps, wf2_s[:, hf, tap, :], rhs,
                                         start=(tap == 0), stop=(tap == 8),
                                         tile_position=(0, 64 * pos),
                                         skip_group_check=True)
                for (i0, pos), ps in zip(tiles, pss):
                    # stage rows into the per-half SBUF accumulator; the
                    # DRAM write happens once per row-half (2 descriptors)
                    bt_v = bass.AP(tensor=btbig[i0 // 32].tensor,
                                   offset=i0 % 32,
                                   ap=[[8192, 2], [1, 2], [32, W]])
                    src = ps.rearrange("p (r c) -> p r c", r=2, c=W)
                    nc.vector.tensor_copy(bt_v, src)

            p3_pairs_rh0 = [(2 * i, 16 + 2 * i) for i in range(8)]
            p3_pairs_rh1 = [(32, 34)] + [(36 + 4 * i, 38 + 4 * i)
                                         for i in range(7)]
            # one buffer, reused for rh1 after rh0's DRAM write (WAR-tracked)
            btbig = {0: bp.tile([2, W * 32], BF16, tag="btbig", name="btb0"),
                     1: bp.tile([2, W * 32], BF16, tag="btbig", name="btb1")}

            # ---- phase 4/5: flow -> CL + index math + gathers, by row half ----
            cl_fx = bp.tile([128, 128], BF16, tag="clfx")
            cl_fy = bp.tile([128, 128], BF16, tag="clfy")

            def cl(tag):
                tt = bp.tile([128, 128], F32, tag=tag, name=tag)
                return tt

            ix = cl("ix"); iy = cl("iy"); tmp = cl("tmp")
            x0i = bp.tile([128, 128], I16, tag="x0i")
            y0i = bp.tile([128, 128], I16, tag="y0i")
            x0f = cl("x0f"); y0f = cl("y0f")
            ef = cl("ef")
            eidx = bp.tile([128, 128], I16, tag="eidx")
            gatall = bp.tile([128, 2 * NIDX * DCH], BF16, tag="tcat_gat")
            _qs = [nc.sync, nc.scalar]
            _qi = 0

            def cl_load(rh):

                for ch, dtile in ((0, cl_fx), (1, cl_fy)):
                    for G in range(8):
                        for w in range(2):
                            dst = bass.AP(tensor=dtile.tensor,
                                          offset=(16 * G) * 128 + 64 * w + 32 * rh,
                                          ap=[[128, 16], [1, 32]])
                            srcp = bass.AP(
                                tensor=flow_dramh[rh],
                                offset=ch * W * 32 + (32 * G + 16 * w) * 32,
                                ap=[[32, 16], [1, 32]])
                            _qs[(ch + G + w) % 2].dma_start(dst, srcp)

            def idx_math(sl):
                V = nc.vector
                V.tensor_scalar_mul(ix[:, sl], cl_fx[:, sl], 0.5)
                V.tensor_tensor(ix[:, sl], ix[:, sl], bx_s[:, sl], ALU.add)
                V.tensor_scalar_mul(iy[:, sl], cl_fy[:, sl], 0.5)
                V.tensor_tensor(iy[:, sl], iy[:, sl], by_s[:, sl], ALU.add)
                V.tensor_copy(x0i[:, sl], ix[:, sl])
                V.tensor_copy(x0f[:, sl], x0i[:, sl])
                V.tensor_tensor(tmp[:, sl], x0f[:, sl], ix[:, sl], ALU.is_gt)
                V.tensor_tensor(x0f[:, sl], x0f[:, sl], tmp[:, sl], ALU.subtract)
                V.tensor_copy(y0i[:, sl], iy[:, sl])
                V.tensor_copy(y0f[:, sl], y0i[:, sl])
                V.tensor_tensor(tmp[:, sl], y0f[:, sl], iy[:, sl], ALU.is_gt)
                V.tensor_tensor(y0f[:, sl], y0f[:, sl], tmp[:, sl], ALU.subtract)
                V.tensor_scalar_mul(ef[:, sl], y0f[:, sl], float(XS))
                V.tensor_tensor(ef[:, sl], ef[:, sl], x0f[:, sl], ALU.add)
                V.tensor_scalar(ef[:, sl], ef[:, sl], 0.0, float(LNUM - XS - 2),
                                ALU.max, ALU.min)
                V.tensor_copy(eidx[:, sl], ef[:, sl])

            def warp_front(rh):
                nc.sync.dma_start(flow_dramh[rh][:], btbig[rh][:])
                cl_load(rh)
                for w in range(2):
                    sl = slice(64 * w + 32 * rh, 64 * w + 32 * rh + 32)
                    idx_math(sl)
                    ds = dsrc if w == 0 else dsrc2
                    off = w * (NIDX * DCH) + rh * 4096
                    nc.gpsimd.ap_gather(
                        gatall[:, off:off + 4096], ds[:],
                        eidx[:, sl],
                        channels=128, num_elems=LNUM, d=DCH, num_idxs=512)

            for a, b in p3_pairs_rh0:
                p3_pair(a, b)
            warp_front(0)
            for a, b in p3_pairs_rh1:
                p3_pair(a, b)
            warp_front(1)

            # ---- weights math (full tensors) ----
            fx = cl("fx"); fy = cl("fy")
            nc.vector.tensor_tensor(fx[:], ix[:], x0f[:], ALU.subtract)
            nc.vector.tensor_tensor(fy[:], iy[:], y0f[:], ALU.subtract)
            vx0 = cl("vx0"); vx1 = cl("vx1"); vy0 = cl("vy0"); vy1 = cl("vy1")
            xp1 = cl("ix"); yp1 = cl("iy")   # reuse dead buffers (WAR-tracked)
            nc.vector.tensor_scalar_add(xp1[:], x0f[:], 1.0)
            nc.vector.tensor_scalar_add(yp1[:], y0f[:], 1.0)
            for vt, src_f in ((vx0, x0f), (vx1, xp1)):
                nc.vector.tensor_tensor(vt[:], src_f[:], xlo_s[:], ALU.is_ge)
                nc.vector.tensor_tensor(tmp[:], src_f[:], xhi_s[:], ALU.is_le)
                nc.vector.tensor_tensor(vt[:], vt[:], tmp[:], ALU.mult)
            for vt, src_f in ((vy0, y0f), (vy1, yp1)):
                nc.vector.tensor_tensor(vt[:], src_f[:], ylo_s[:], ALU.is_ge)
                nc.vector.tensor_tensor(tmp[:], src_f[:], yhi_s[:], ALU.is_le)
                nc.vector.tensor_tensor(vt[:], vt[:], tmp[:], ALU.mult)
            gx0 = cl("x0f"); gx1 = cl("y0f"); gy0 = cl("vx0"); gy1 = cl("vx1")
            nc.vector.tensor_scalar(tmp[:], fx[:], -1.0, 1.0, ALU.mult, ALU.add)
            nc.vector.tensor_tensor(gx0[:], tmp[:], vx0[:], ALU.mult)
            nc.vector.tensor_tensor(gx1[:], fx[:], vx1[:], ALU.mult)
            nc.vector.tensor_scalar(tmp[:], fy[:], -1.0, 1.0, ALU.mult, ALU.add)
            nc.vector.tensor_tensor(gy0[:], tmp[:], vy0[:], ALU.mult)
            nc.vector.tensor_tensor(gy1[:], fy[:], vy1[:], ALU.mult)
            wsall = bp.tile([128, 4, 128], BF16, tag="wsall")
            nc.vector.tensor_tensor(wsall[:, 0, :], gx0[:], gy0[:], ALU.mult)
            nc.vector.tensor_tensor(wsall[:, 1, :], gx1[:], gy0[:], ALU.mult)
            nc.vector.tensor_tensor(wsall[:, 2, :], gx0[:], gy1[:], ALU.mult)
            nc.vector.tensor_tensor(wsall[:, 3, :], gx1[:], gy1[:], ALU.mult)
            # ---- phase 6: weight planes -> w_g (SBUF->SBUF) -> W4 ----
            w_g = bp.tile([8, 4, 2048], BF16, tag="wg")
            for s in range(4):
                (nc.sync if s % 2 else nc.scalar).dma_start(
                    bass.AP(tensor=w_g.tensor, offset=s * 2048,
                            ap=[[4 * 2048, 8], [128, 16], [1, 128]]),
                    wsall[:, s, :])
            # W4 [128, 4, 2048] j-ordered (j = 32r + 16w + m over full 64 rows)
            w4 = bp.tile([128, 4 * 2048], F32, tag="x_w4")
            for s in range(4):
                for c4 in range(4):
                    pw = pp.tile([128, 512], F32, tag="pst", name="pw")
                    nc.tensor.matmul(pw[:], emat_s[:], w_g[:, s, 512 * c4:512 * (c4 + 1)],
                                     start=True, stop=True)
                    # pw free = (m-part: 128,4)(r: 2,64)(w: 1,2) for m in [4c4, 4c4+4)
                    dstw = bass.AP(tensor=w4.tensor,
                                   offset=s * 2048 + 4 * c4,
                                   ap=[[4 * 2048, 128], [1, 4], [1024, 2], [16, 64]])
                    src_w = pw[:].rearrange("p (m w r) -> p m w r", m=4, w=2, r=64)
                    if c4 % 2 == 0:
                        nc.scalar.copy(dstw, src_w)
                    else:
                        nc.vector.tensor_copy(dstw, src_w)

            # Two independent combine chains: call 0 on VectorE, call 1 on
            # GpSimd, each with its own scratch so they run concurrently.
            # pls reuse the gather-source buffers (dead after the last gather)
            pls_c = [bp.tile([128, 4, NIDX], BF16, tag=("dsrc", "dsrc2")[c],
                             name=f"pls{c}")
                     for c in range(2)]
            bb_c = [bp.tile([128, 2, NIDX], F32, tag=f"bbc{c}", name=f"bbc{c}")
                    for c in range(2)]
            for call in range(2):
                eng = nc.vector if call == 0 else nc.gpsimd
                pls, bbt = pls_c[call], bb_c[call]
                for slot in range(2):
                    for s in range(4):
                        g_v = bass.AP(tensor=gatall.tensor,
                                      offset=call * NIDX * DCH + 4 * slot + s,
                                      ap=[[2 * NIDX * DCH, 128], [DCH, NIDX]])
                        eng.tensor_tensor(
                            pls[:, s, :], g_v,
                            w4[:, (s * 2048 + 1024 * call):(s * 2048 + 1024 * call + NIDX)],
                            ALU.mult)
                    eng.tensor_tensor(pls[:, 0, :], pls[:, 0, :], pls[:, 1, :],
                                      ALU.add)
                    eng.tensor_tensor(pls[:, 2, :], pls[:, 2, :], pls[:, 3, :],
                                      ALU.add)
                    eng.tensor_tensor(bbt[:, slot, :], pls[:, 0, :], pls[:, 2, :],
                                      ALU.add)
                    nl = 16 if slot == 0 else 3
                    for G in range(8):
                        dst = bass.AP(
                            tensor=out_d,
                            offset=(16 * slot) * 64 * W + 32 * G + 16 * call,
                            ap=[[64 * W, nl], [W, 64], [1, 16]])
                        srcb = bass.AP(
                            tensor=bbt.tensor,
                            offset=(16 * G) * (2 * NIDX) + slot * NIDX,
                            ap=[[2 * NIDX, nl], [16, 64], [1, 16]])
                        (nc.sync if call == 0 else nc.scalar).dma_start(dst, srcb)
    nc.finalize()
    return nc


# ======================= host-side prep =======================

def _feat_slab(feat_b, h):
    """feat_b (256, 128, 256) f32 -> (2, 128, 68, 256) bf16 slab for half h."""
    r0 = 64 * h - 2
    slab = np.zeros((CIN, SLAB_R, W), np.float32)
    lo, hi = max(r0, 0), min(r0 + SLAB_R, H)
    slab[:, lo - r0:hi - r0, :] = feat_b[:, lo:hi, :]
    return np.ascontiguousarray(
        slab.reshape(2, 128, SLAB_R, W).astype(BF))


def _host_constants(q, h):
    R0 = 64 * h
    # CL layout: p = 16G + m, f = 64w + r; pixel (row R0+r, col 32G+16w+m)
    p = np.arange(128)[:, None]
    f = np.arange(128)[None, :]
    G = p // 16
    m = p % 16
    r = f % 64
    w = f // 64
    col = 32 * G + 16 * w + m
    row = R0 + r
    ix_base = col + col / (W - 1.0) - 0.5
    iy_base = row + row / (H - 1.0) - 0.5
    colbase = 32 * G + 16 * w - COLB
    rowbase = R0 - ROWB
    bx = np.broadcast_to(ix_base - colbase, (128, 128)).astype(np.float32).copy()
    by = np.broadcast_to(iy_base - rowbase, (128, 128)).astype(np.float32).copy()
    xlo = np.broadcast_to(0.0 - colbase, (128, 128)).astype(np.float32).copy()
    xhi = np.broadcast_to((W - 1.0) - colbase, (128, 128)).astype(np.float32).copy()
    ylo = np.full((128, 128), 0.0 - rowbase, np.float32)
    yhi = np.full((128, 128), (H - 1.0) - rowbase, np.float32)
    return bx, by, xlo, xhi, ylo, yhi


def _dsrc_build(pred_imgs, h):
    """pred_imgs: (19, 128, 256) f32. Returns (2, 128, LNUM*8) f32 gather
    source; call = col-half w, slab = rows [R0-6, R0+70) x 26-col band."""
    R0 = 64 * h
    padded = np.zeros((CCLS, H + 16, W + 16), np.float32)
    padded[:, 8:8 + H, 8:8 + W] = pred_imgs
    out = np.zeros((2, 128, LNUM, DCH), np.float32)
    rowbase = R0 - ROWB
    for call in range(2):
        for G in range(8):
            colbase = 32 * G + 16 * call - COLB
            for l in range(16):
                for slot in range(2):
                    img = l + 16 * slot
                    if img >= CCLS:
                        img = l
                    for j2 in range(2):
                        for j1 in range(2):
                            win = padded[img,
                                         8 + rowbase + j2: 8 + rowbase + j2 + YS,
                                         8 + colbase + j1: 8 + colbase + j1 + XS]
                            out[call, 16 * G + l, :, 4 * slot + 2 * j2 + j1] = \
                                win.reshape(-1)
    return out.reshape(2, 128, LNUM * DCH)


def make_inputs(core, t1_feature, t2_feature, t2_pred, w_down1, w_down2,
                w_flow1, bn_gamma, bn_beta, bn_mean, bn_var, w_flow2):
    q, h = core // 2, core % 2
    f1 = _feat_slab(t1_feature[q], h)
    f2 = _feat_slab(t2_feature[q], h)
    wd = np.stack([
        np.stack([w_down1[:, 128 * k:128 * (k + 1), 0, 0].T for k in range(2)]),
        np.stack([w_down2[:, 128 * k:128 * (k + 1), 0, 0].T for k in range(2)]),
    ]).transpose(2, 0, 1, 3).astype(BF).copy()        # (128,2,2,64)
    wf1 = np.stack([w_flow1[:, :, t // 3, t % 3].T for t in range(9)],
                   axis=1).astype(BF).copy()          # (128,9,64)
    wf2h = np.stack([w_flow2[:, :, t // 3, t % 3].T for t in range(9)],
                    axis=1).astype(BF)                # (64,9,2)
    z = np.zeros_like(wf2h)
    wf2 = np.stack([np.concatenate([wf2h, z], axis=0),
                    np.concatenate([z, wf2h], axis=0)],
                   axis=1).copy()                     # (128,2,9,2)
    scale = bn_gamma / np.sqrt(bn_var + 1e-5)
    bias = bn_beta - bn_mean * scale
    bn1 = np.stack([scale, bias], axis=1).reshape(T, 2, 1).astype(np.float32)
    bn = np.concatenate([bn1, bn1], axis=0)           # (128,2,1)
    mask = np.ones((128, 2, 1), np.float32)
    if h == 0:
        mask[0:T, 0] = 0.0   # x row 0 (half A) = image row -1
    else:
        mask[T:128, 1] = 0.0  # x half-B row 35 = x row 65 = image row 128
    bx, by, xlo, xhi, ylo, yhi = _host_constants(q, h)
    imgs = img_list(q)
    pred_imgs = np.stack([t2_pred[n, ch] for (n, ch) in imgs])
    dsrc = _dsrc_build(pred_imgs, h)
    emat = np.zeros((8, 128), BF)
    for Gi in range(8):
        emat[Gi, 16 * Gi:16 * (Gi + 1)] = 1.0
    return {
        "f1": f1, "f2": f2, "wd": wd, "wf1": wf1, "wf2": wf2,
        "bn": bn, "mask": mask, "bx": bx, "by": by, "ylo": ylo, "yhi": yhi,
        "xlo": xlo, "xhi": xhi, "emat": emat, "dsrc": dsrc.astype(BF),
    }


_NC_CACHE = {}


def kernel(**inputs):
    from concourse.bass_utils import run_bass_kernel_spmd
    if "nc" not in _NC_CACHE:
        _NC_CACHE["nc"] = build_nc()
    nc = _NC_CACHE["nc"]
    in_maps = [make_inputs(c, **inputs) for c in range(8)]
    res = run_bass_kernel_spmd(nc, in_maps, list(range(8)))
    out = np.zeros((NB, CCLS, H, W), np.float32)
    for c in range(8):
        q, h = c // 2, c % 2
        o = res.results[c]["out"]
        for i, (n, ch) in enumerate(img_list(q)):
            out[n, ch, 64 * h:64 * (h + 1), :] = o[i]
    return out



# revision 33
# speedup vs baseline: 1.2067x; 1.0520x over previous
"""Trainium2 Bass kernel for nn_AlignModule_full (8 NeuronCores, data-parallel).

Reference computation: two 1x1 convs -> concat -> 3x3 conv + BN + ReLU ->
3x3 conv -> flow -> bilinear grid_sample warp of t2_pred, where output
channel (n, ch) is warped with flow[(3n+ch) % 4] (torch flow.repeat
semantics faithfully ported by the reference).

Sharding: core c = (q, h), q = c//2 flow batch, h = c%2 row half.
Each core computes flow(q, rows 64h..64h+64) from batch-q features, then
warps the 19 (n, ch) images with (3n+ch)%4 == q for its row half, using
only its own flow. Zero cross-core communication.

Warp implementation: per-pixel bilinear gather via gpsimd ap_gather with a
host-built d=8 interleaved source: each index fetches the full 2x2 patch
for TWO image slots at once (19 images = 16 lanes x 2 slots).
"""
import sys

for _p in ('/opt/trn_rl_repo',):
    if _p not in sys.path:
        sys.path.append(_p)

import numpy as np
import ml_dtypes

import concourse.bass as bass
import concourse.bacc as bacc
import concourse.mybir as mybir
import concourse.tile as tile

F32 = mybir.dt.float32
BF16 = mybir.dt.bfloat16
I16 = mybir.dt.int16
AF = mybir.ActivationFunctionType
ALU = mybir.AluOpType

H, W, CIN, T, CCLS, NB = 128, 256, 256, 64, 19, 4
SLAB_R = 68          # feature slab rows
WS = 258             # padded width for t/x buffers
XR = 66              # x rows total
XH = 36              # x rows per partition-half (A: 0..36, B: 30..66)
YS, XS = 76, 26      # gather slab rows/cols per (group, call=col-half)
LNUM = YS * XS       # base positions per partition
DCH = 8              # interleave chunk: 2 slots x (2x2 patch)
NIDX = 1024          # gather indices per group per call
ROWB = 6             # slab row margin before first output row of the call
COLB = 5             # slab col margin before group col block

BF = ml_dtypes.bfloat16


def img_list(q):
    return [(n, ch) for n in range(NB) for ch in range(CCLS)
            if (3 * n + ch) % 4 == q]


def build_nc():
    nc = bacc.Bacc(None, target_bir_lowering=False, debug=False)
    P = nc.declare_dram_parameter
    f1_d = P("f1", [2, 128, SLAB_R, W], BF16, isOutput=False)
    f2_d = P("f2", [2, 128, SLAB_R, W], BF16, isOutput=False)
    wd_d = P("wd", [128, 2, 2, T], BF16, isOutput=False)
    wf1_d = P("wf1", [128, 9, T], BF16, isOutput=False)
    wf2_d = P("wf2", [128, 2, 9, 2], BF16, isOutput=False)
    bn_d = P("bn", [128, 2, 1], F32, isOutput=False)
    mask_d = P("mask", [128, 2, 1], F32, isOutput=False)
    bx_d = P("bx", [128, 128], F32, isOutput=False)
    by_d = P("by", [128, 128], F32, isOutput=False)
    ylo_d = P("ylo", [128, 128], F32, isOutput=False)
    yhi_d = P("yhi", [128, 128], F32, isOutput=False)
    xlo_d = P("xlo", [128, 128], F32, isOutput=False)
    xhi_d = P("xhi", [128, 128], F32, isOutput=False)
    emat_d = P("emat", [8, 128], BF16, isOutput=False)
    dsrc_d = P("dsrc", [2, 128, LNUM * DCH], BF16, isOutput=False)
    out_d = P("out", [CCLS, 64, W], F32, isOutput=True)

    flow_dramh = [nc.dram_tensor("flow_t0", [2, W, 32], BF16),
                  nc.dram_tensor("flow_t1", [2, W, 32], BF16)]  # (ch, col, row-half)

    NRB = 4               # feature rows per DMA batch
    NBATCH = (SLAB_R + NRB - 1) // NRB  # 12 (last batch 2 rows)

    with tile.TileContext(nc) as tc:
        with (
            tc.tile_pool(name="stream", bufs=3) as sp,
            tc.tile_pool(name="big", bufs=1) as bp,
            tc.tile_pool(name="psA", bufs=2, space="PSUM") as pp,
        ):
            # ---- feature batches: [128, feat, ck, NRB, W] bf16, 2-deep ring
            fts = {}

            def load_batch(b):
                r0 = NRB * b
                nr = min(NRB, SLAB_R - r0)
                ft = bp.tile([128, 2, 2, NRB, W], BF16, tag="fbatch",
                             name=f"fb{b}", bufs=2)
                for fi, fd in ((0, f1_d), (1, f2_d)):
                    src = bass.AP(tensor=fd, offset=r0 * W,
                                  ap=[[SLAB_R * W, 128], [128 * SLAB_R * W, 2],
                                      [W, nr], [1, W]])
                    (nc.sync if fi == 0 else nc.scalar).dma_start(
                        ft[:, fi, :, 0:nr, :], src)
                fts[b] = ft

            load_batch(0)

            # ---- constants (spread across both HWDGE queues) ----
            wd_s = bp.tile([128, 2, 2, T], BF16, tag="wd")
            wf1_s = bp.tile([128, 9, T], BF16, tag="wf1")
            wf2_s = bp.tile([128, 2, 9, 2], BF16, tag="wf2")
            bn_s = bp.tile([128, 2, 1], F32, tag="bn")
            mask_s = bp.tile([128, 2, 1], F32, tag="mask")
            xlo_s = bp.tile([128, 128], F32, tag="xlo")
            xhi_s = bp.tile([128, 128], F32, tag="xhi")
            emat_s = bp.tile([8, 128], BF16, tag="emat")
            bx_s = bp.tile([128, 128], F32, tag="bx")
            by_s = bp.tile([128, 128], F32, tag="by")
            ylo_s = bp.tile([128, 128], F32, tag="ylo")
            yhi_s = bp.tile([128, 128], F32, tag="yhi")
            for i, (t_, d_) in enumerate((
                    (wd_s, wd_d), (wf1_s, wf1_d), (wf2_s, wf2_d),
                    (bn_s, bn_d), (mask_s, mask_d), (xlo_s, xlo_d),
                    (xhi_s, xhi_d),
                    (emat_s, emat_d), (bx_s, bx_d), (by_s, by_d),
                    (ylo_s, ylo_d), (yhi_s, yhi_d))):
                (nc.sync if i % 2 else nc.scalar).dma_start(t_[:], d_[:])

            # ---- big shared tiles; gather sources loaded up front (SWDGE) ----
            t_cat = bp.tile([128, SLAB_R * WS], BF16, tag="tcat_gat")
            dsrc = bp.tile([128, LNUM * DCH], BF16, tag="dsrc")
            dsrc2 = bp.tile([128, LNUM * DCH], BF16, tag="dsrc2")
            # dsrc loads happen post-P1-solo on the HWDGE queues so startup
            # HBM bandwidth is all features.  GpSimd runs only ap_gather +
            # tensor_tensor, and a dummy gather below pre-loads the gather
            # ucode lib so no IRAM swap lands on the critical path.
            x_sb = bp.tile([128, XH * WS], BF16, tag="x_w4")

            dum_src = sp.tile([128, 8], BF16, tag="dumg", bufs=1)
            dum_idx = sp.tile([128, 1], I16, tag="dumi", bufs=1)
            dum_out = sp.tile([128, 32], BF16, tag="dumo", bufs=1)
            nc.vector.memset(dum_src[:], 0.0)
            nc.vector.memset(dum_idx[:], 0)
            nc.gpsimd.ap_gather(dum_out[:], dum_src[:], dum_idx[:],
                                channels=128, num_elems=4, d=2, num_idxs=16)

            load_batch(1)

            t3 = t_cat[:].rearrange("p (r c) -> p r c", r=SLAB_R, c=WS)
            nc.vector.memset(t3[:, :, 0:1], 0.0)
            nc.vector.memset(t3[:, :, 257:258], 0.0)

            # ---- phases 1+2 interleaved: 1x1 convs feed 3x3 conv ----
            def p1_tile(it):
                r0 = 2 * it
                b, rr = r0 // NRB, r0 % NRB
                if rr == 0 and b + 1 < NBATCH and (b + 1) not in fts:
                    load_batch(b + 1)
                ft = fts[b]
                ps = pp.tile([128, 2 * W], F32, tag="pst", name="pst")
                for ck in range(2):
                    nc.tensor.matmul(ps[0:T, :], wd_s[:, 0, ck, :],
                                     ft[:, 0, ck, rr:rr + 2, :],
                                     start=(ck == 0), stop=(ck == 1),
                                     tile_position=(0, 0),
                                     skip_group_check=True)
                    nc.tensor.matmul(ps[T:128, :], wd_s[:, 1, ck, :],
                                     ft[:, 1, ck, rr:rr + 2, :],
                                     start=(ck == 0), stop=(ck == 1),
                                     tile_position=(0, 64),
                                     skip_group_check=True)
                dst = bass.AP(tensor=t_cat.tensor, offset=r0 * WS + 1,
                              ap=[[SLAB_R * WS, 128], [WS, 2], [1, W]])
                nc.vector.tensor_copy(dst, ps[:].rearrange("p (r c) -> p r c",
                                                           r=2, c=W))

            x3 = x_sb[:].rearrange("p (r c) -> p r c", r=XH, c=WS)
            nc.vector.memset(x3[:, :, 0:1], 0.0)
            nc.vector.memset(x3[:, :, 257:258], 0.0)

            def p2_iter(it):
                jA = 2 * it
                jB = 30 + 2 * it
                ps = pp.tile([128, 2 * W], F32, tag="psx", name="psx")
                for tap in range(9):
                    dy, dx = tap // 3, tap % 3
                    rhsA = bass.AP(tensor=t_cat.tensor,
                                   offset=(jA + dy) * WS + dx,
                                   ap=[[SLAB_R * WS, 128], [WS, 2], [1, W]])
                    rhsB = bass.AP(tensor=t_cat.tensor,
                                   offset=(jB + dy) * WS + dx,
                                   ap=[[SLAB_R * WS, 128], [WS, 2], [1, W]])
                    nc.tensor.matmul(ps[0:T, :], wf1_s[:, tap, :], rhsA,
                                     start=(tap == 0), stop=(tap == 8),
                                     tile_position=(0, 0),
                                     skip_group_check=True)
                    nc.tensor.matmul(ps[T:128, :], wf1_s[:, tap, :], rhsB,
                                     start=(tap == 0), stop=(tap == 8),
                                     tile_position=(0, 64),
                                     skip_group_check=True)
                dstA = bass.AP(tensor=x_sb.tensor, offset=jA * WS + 1,
                               ap=[[XH * WS, T], [WS, 2], [1, W]])
                dstB = bass.AP(tensor=x_sb.tensor,
                               offset=T * (XH * WS) + jA * WS + 1,
                               ap=[[XH * WS, T], [WS, 2], [1, W]])
                nc.scalar.activation(dstA,
                                     ps[0:T].rearrange("p (r c) -> p r c", r=2, c=W),
                                     AF.Relu, bias=bn_s[0:T, 1], scale=bn_s[0:T, 0])
                nc.scalar.activation(dstB,
                                     ps[T:128].rearrange("p (r c) -> p r c", r=2, c=W),
                                     AF.Relu, bias=bn_s[T:128, 1], scale=bn_s[T:128, 0])

            for it in range(18):
                p1_tile(it)
            for it in range(18):
                p2_iter(it)
                if 18 + it < SLAB_R // 2:
                    p1_tile(18 + it)
                if it == 2:
                    nc.sync.dma_start(dsrc[:], dsrc_d[0, :, :])
                if it == 8:
                    nc.scalar.dma_start(dsrc2[:], dsrc_d[1, :, :])
            nc.vector.tensor_scalar_mul(x3[0:T, 0, :], x3[0:T, 0, :], mask_s[0:T, 0])
            nc.vector.tensor_scalar_mul(x3[T:128, 35, :], x3[T:128, 35, :],
                                        mask_s[T:128, 1])

            # ---- phase 3: 3x3 conv 64->2, two tiles concurrent via PE
            # column strips.  First 8 pairs cover flow rows 0..31 (rh0) so
            # the warp pipeline for rh0 can start while rh1 still computes.
            def p3_pair(iA, iB):
                # PE column strips: out PSUM start partition must equal the
                # tile-position column, so pos-1 writes partitions 64:66.
                tiles = [(iA, 0)] + ([(iB, 1)] if iB is not None else [])
                pss = []
                for i0, pos in tiles:
                    t_ = pp.tile([128, 2 * W], F32,
                                 tag=("psf" if pos == 0 else "psfB"),
                                 name="psf", bufs=2)
                    pss.append(t_[64 * pos:64 * pos + 2])
                for tap in range(9):
                    dy, dx = tap // 3, tap % 3
                    for (i0, pos), ps in zip(tiles, pss):
                        hf = 0 if i0 < 34 else 1
                        base = i0 + dy - 30 * hf
                        rhs = bass.AP(tensor=x_sb.tensor,
                                      offset=base * WS + dx,
                                      ap=[[XH * WS, 128], [WS, 2], [1, W]])
                        nc.tensor.matmul(ps, wf2_s[:, hf, tap, :], rhs,
                                         start=(tap == 0), stop=(tap == 8),
                                         tile_position=(0, 64 * pos),
                                         skip_group_check=True)
                for (i0, pos), ps in zip(tiles, pss):
                    # stage rows into the per-half SBUF accumulator; the
                    # DRAM write happens once per row-half (2 descriptors)
                    bt_v = bass.AP(tensor=btbig[i0 // 32].tensor,
                                   offset=i0 % 32,
                                   ap=[[8192, 2], [1, 2], [32, W]])
                    src = ps.rearrange("p (r c) -> p r c", r=2, c=W)
                    if pos == 0:
                        nc.vector.tensor_copy(bt_v, src)
                    else:
                        nc.scalar.copy(bt_v, src)

            p3_pairs_rh0 = [(2 * i, 16 + 2 * i) for i in range(8)]
            p3_pairs_rh1 = [(32, 34)] + [(36 + 4 * i, 38 + 4 * i)
                                         for i in range(7)]
            # one buffer, reused for rh1 after rh0's DRAM write (WAR-tracked)
            btbig = {0: bp.tile([2, W * 32], BF16, tag="btbig", name="btb0"),
                     1: bp.tile([2, W * 32], BF16, tag="btbig", name="btb1")}

            # ---- phase 4/5: flow -> CL + index math + gathers, by row half ----
            cl_fx = bp.tile([128, 128], BF16, tag="clfx")
            cl_fy = bp.tile([128, 128], BF16, tag="clfy")

            def cl(tag):
                tt = bp.tile([128, 128], F32, tag=tag, name=tag)
                return tt

            ix = cl("ix"); iy = cl("iy"); tmp = cl("tmp")
            x0i = bp.tile([128, 128], I16, tag="x0i")
            y0i = bp.tile([128, 128], I16, tag="y0i")
            x0f = cl("x0f"); y0f = cl("y0f")
            ef = cl("ef")
            eidx = bp.tile([128, 128], I16, tag="eidx")
            gatall = bp.tile([128, 2 * NIDX * DCH], BF16, tag="tcat_gat")
            _qs = [nc.sync, nc.scalar]
            _qi = 0

            def cl_load(rh):
                # one DMA per (ch, w): contiguous 32-row runs from the
                # (ch, col, row) flow file into CL partitions
                for ch, dtile in ((0, cl_fx), (1, cl_fy)):
                    for w in range(2):
                        dst = bass.AP(tensor=dtile.tensor,
                                      offset=64 * w + 32 * rh,
                                      ap=[[128, 128], [1, 32]])
                        srcp = bass.AP(
                            tensor=flow_dramh[rh],
                            offset=ch * W * 32 + 16 * w * 32,
                            ap=[[32 * 32, 8], [32, 16], [1, 32]])
                        _qs[(ch + w) % 2].dma_start(dst, srcp)

            def idx_math(rh):
                # both w column-halves in one 2D-sliced op set
                V = nc.vector

                def S(t):
                    return bass.AP(tensor=t.tensor, offset=32 * rh,
                                   ap=[[128, 128], [64, 2], [1, 32]])

                def S16(t):
                    return bass.AP(tensor=t.tensor, offset=32 * rh,
                                   ap=[[128, 128], [64, 2], [1, 32]])

                V.tensor_scalar_mul(S(ix), S(cl_fx), 0.5)
                V.tensor_tensor(S(ix), S(ix), S(bx_s), ALU.add)
                V.tensor_scalar_mul(S(iy), S(cl_fy), 0.5)
                V.tensor_tensor(S(iy), S(iy), S(by_s), ALU.add)
                V.tensor_copy(S16(x0i), S(ix))
                V.tensor_copy(S(x0f), S16(x0i))
                V.tensor_tensor(S(tmp), S(x0f), S(ix), ALU.is_gt)
                V.tensor_tensor(S(x0f), S(x0f), S(tmp), ALU.subtract)
                V.tensor_copy(S16(y0i), S(iy))
                V.tensor_copy(S(y0f), S16(y0i))
                V.tensor_tensor(S(tmp), S(y0f), S(iy), ALU.is_gt)
                V.tensor_tensor(S(y0f), S(y0f), S(tmp), ALU.subtract)
                V.tensor_scalar_mul(S(ef), S(y0f), float(XS))
                V.tensor_tensor(S(ef), S(ef), S(x0f), ALU.add)
                V.tensor_scalar(S(ef), S(ef), 0.0, float(LNUM - XS - 2),
                                ALU.max, ALU.min)
                V.tensor_copy(S16(eidx), S(ef))

            def warp_front(rh):
                nc.sync.dma_start(flow_dramh[rh][:], btbig[rh][:])
                cl_load(rh)
                idx_math(rh)
                for w in range(2):
                    sl = slice(64 * w + 32 * rh, 64 * w + 32 * rh + 32)
                    ds = dsrc if w == 0 else dsrc2
                    off = w * (NIDX * DCH) + rh * 4096
                    nc.gpsimd.ap_gather(
                        gatall[:, off:off + 4096], ds[:],
                        eidx[:, sl],
                        channels=128, num_elems=LNUM, d=DCH, num_idxs=512)

            for a, b in p3_pairs_rh0:
                p3_pair(a, b)
            warp_front(0)
            for a, b in p3_pairs_rh1:
                p3_pair(a, b)
            warp_front(1)

            # ---- weights math (full tensors) ----
            fx = cl("fx"); fy = cl("fy")
            nc.vector.tensor_tensor(fx[:], ix[:], x0f[:], ALU.subtract)
            nc.vector.tensor_tensor(fy[:], iy[:], y0f[:], ALU.subtract)
            vx0 = cl("vx0"); vx1 = cl("vx1"); vy0 = cl("vy0"); vy1 = cl("vy1")
            xp1 = cl("ix"); yp1 = cl("iy")   # reuse dead buffers (WAR-tracked)
            nc.vector.tensor_scalar_add(xp1[:], x0f[:], 1.0)
            nc.vector.tensor_scalar_add(yp1[:], y0f[:], 1.0)
            for vt, src_f in ((vx0, x0f), (vx1, xp1)):
                nc.vector.tensor_tensor(vt[:], src_f[:], xlo_s[:], ALU.is_ge)
                nc.vector.tensor_tensor(tmp[:], src_f[:], xhi_s[:], ALU.is_le)
                nc.vector.tensor_tensor(vt[:], vt[:], tmp[:], ALU.mult)
            for vt, src_f in ((vy0, y0f), (vy1, yp1)):
                nc.vector.tensor_tensor(vt[:], src_f[:], ylo_s[:], ALU.is_ge)
                nc.vector.tensor_tensor(tmp[:], src_f[:], yhi_s[:], ALU.is_le)
                nc.vector.tensor_tensor(vt[:], vt[:], tmp[:], ALU.mult)
            gx0 = cl("x0f"); gx1 = cl("y0f"); gy0 = cl("vx0"); gy1 = cl("vx1")
            nc.vector.tensor_scalar(tmp[:], fx[:], -1.0, 1.0, ALU.mult, ALU.add)
            nc.vector.tensor_tensor(gx0[:], tmp[:], vx0[:], ALU.mult)
            nc.vector.tensor_tensor(gx1[:], fx[:], vx1[:], ALU.mult)
            nc.vector.tensor_scalar(tmp[:], fy[:], -1.0, 1.0, ALU.mult, ALU.add)
            nc.vector.tensor_tensor(gy0[:], tmp[:], vy0[:], ALU.mult)
            nc.vector.tensor_tensor(gy1[:], fy[:], vy1[:], ALU.mult)
            wsall = bp.tile([128, 4, 128], BF16, tag="wsall")
            nc.vector.tensor_tensor(wsall[:, 0, :], gx0[:], gy0[:], ALU.mult)
            nc.vector.tensor_tensor(wsall[:, 1, :], gx1[:], gy0[:], ALU.mult)
            nc.vector.tensor_tensor(wsall[:, 2, :], gx0[:], gy1[:], ALU.mult)
            nc.vector.tensor_tensor(wsall[:, 3, :], gx1[:], gy1[:], ALU.mult)
            # ---- phase 6: weight planes -> w_g (SBUF->SBUF) -> W4 ----
            w_g = bp.tile([8, 4, 2048], BF16, tag="wg")
            for s in range(4):
                (nc.sync if s % 2 else nc.scalar).dma_start(
                    bass.AP(tensor=w_g.tensor, offset=s * 2048,
                            ap=[[4 * 2048, 8], [128, 16], [1, 128]]),
                    wsall[:, s, :])
            # W4 [128, 4, 2048] j-ordered (j = 32r + 16w + m over full 64 rows)
            w4 = bp.tile([128, 4 * 2048], F32, tag="x_w4")
            for s in range(4):
                for c4 in (2, 3, 0, 1):   # call-1 chunks first (feed gpsimd)
                    pw = pp.tile([128, 512], F32, tag="pst", name="pw")
                    nc.tensor.matmul(pw[:], emat_s[:], w_g[:, s, 512 * c4:512 * (c4 + 1)],
                                     start=True, stop=True)
                    # pw free = (m-part: 128,4)(r: 2,64)(w: 1,2) for m in [4c4, 4c4+4)
                    dstw = bass.AP(tensor=w4.tensor,
                                   offset=s * 2048 + 4 * c4,
                                   ap=[[4 * 2048, 128], [1, 4], [1024, 2], [16, 64]])
                    src_w = pw[:].rearrange("p (m w r) -> p m w r", m=4, w=2, r=64)
                    nc.scalar.copy(dstw, src_w)

            # Two independent combine chains: call 0 on VectorE, call 1 on
            # GpSimd, each with its own scratch so they run concurrently.
            # pls reuse the gather-source buffers (dead after the last gather)
            pls_c = [bp.tile([128, 4, NIDX], BF16, tag=("dsrc", "dsrc2")[c],
                             name=f"pls{c}")
                     for c in range(2)]
            bb_c = [bp.tile([128, 2, NIDX], F32, tag=f"bbc{c}", name=f"bbc{c}")
                    for c in range(2)]
            def combine(call, slot, eng, pls):
                bbt = bb_c[call]
                for s in range(4):
                    g_v = bass.AP(tensor=gatall.tensor,
                                  offset=call * NIDX * DCH + 4 * slot + s,
                                  ap=[[2 * NIDX * DCH, 128], [DCH, NIDX]])
                    eng.tensor_tensor(
                        pls[:, s, :], g_v,
                        w4[:, (s * 2048 + 1024 * call):(s * 2048 + 1024 * call + NIDX)],
                        ALU.mult)
                eng.tensor_tensor(pls[:, 0, :], pls[:, 0, :], pls[:, 1, :],
                                  ALU.add)
                eng.tensor_tensor(pls[:, 2, :], pls[:, 2, :], pls[:, 3, :],
                                  ALU.add)
                eng.tensor_tensor(bbt[:, slot, :], pls[:, 0, :], pls[:, 2, :],
                                  ALU.add)
                nl = 16 if slot == 0 else 3
                for G in range(8):
                    dst = bass.AP(
                        tensor=out_d,
                        offset=(16 * slot) * 64 * W + 32 * G + 16 * call,
                        ap=[[64 * W, nl], [W, 64], [1, 16]])
                    srcb = bass.AP(
                        tensor=bbt.tensor,
                        offset=(16 * G) * (2 * NIDX) + slot * NIDX,
                        ap=[[2 * NIDX, nl], [16, 64], [1, 16]])
                    (nc.scalar if eng is nc.gpsimd else nc.sync).dma_start(
                        dst, srcb)

            # gpsimd takes one quarter (it is ~1.5x slower per op and pays
            # a ucode lib swap); vector takes the other three.
            combine(1, 0, nc.gpsimd, pls_c[1])
            combine(0, 0, nc.vector, pls_c[0])
            combine(0, 1, nc.vector, pls_c[0])
            combine(1, 1, nc.vector, pls_c[0])
    nc.finalize()
    return nc


# ======================= host-side prep =======================

def _feat_slab(feat_b, h):
    """feat_b (256, 128, 256) f32 -> (2, 128, 68, 256) bf16 slab for half h."""
    r0 = 64 * h - 2
    slab = np.zeros((CIN, SLAB_R, W), np.float32)
    lo, hi = max(r0, 0), min(r0 + SLAB_R, H)
    slab[:, lo - r0:hi - r0, :] = feat_b[:, lo:hi, :]
    return np.ascontiguousarray(
        slab.reshape(2, 128, SLAB_R, W).astype(BF))


def _host_constants(q, h):
    R0 = 64 * h
    # CL layout: p = 16G + m, f = 64w + r; pixel (row R0+r, col 32G+16w+m)
    p = np.arange(128)[:, None]
    f = np.arange(128)[None, :]
    G = p // 16
    m = p % 16
    r = f % 64
    w = f // 64
    col = 32 * G + 16 * w + m
    row = R0 + r
    ix_base = col + col / (W - 1.0) - 0.5
    iy_base = row + row / (H - 1.0) - 0.5
    colbase = 32 * G + 16 * w - COLB
    rowbase = R0 - ROWB
    bx = np.broadcast_to(ix_base - colbase, (128, 128)).astype(np.float32).copy()
    by = np.broadcast_to(iy_base - rowbase, (128, 128)).astype(np.float32).copy()
    xlo = np.broadcast_to(0.0 - colbase, (128, 128)).astype(np.float32).copy()
    xhi = np.broadcast_to((W - 1.0) - colbase, (128, 128)).astype(np.float32).copy()
    ylo = np.full((128, 128), 0.0 - rowbase, np.float32)
    yhi = np.full((128, 128), (H - 1.0) - rowbase, np.float32)
    return bx, by, xlo, xhi, ylo, yhi


def _dsrc_build(pred_imgs, h):
    """pred_imgs: (19, 128, 256) f32. Returns (2, 128, LNUM*8) f32 gather
    source; call = col-half w, slab = rows [R0-6, R0+70) x 26-col band."""
    R0 = 64 * h
    padded = np.zeros((CCLS, H + 16, W + 16), np.float32)
    padded[:, 8:8 + H, 8:8 + W] = pred_imgs
    out = np.zeros((2, 128, LNUM, DCH), np.float32)
    rowbase = R0 - ROWB
    for call in range(2):
        for G in range(8):
            colbase = 32 * G + 16 * call - COLB
            for l in range(16):
                for slot in range(2):
                    img = l + 16 * slot
                    if img >= CCLS:
                        img = l
                    for j2 in range(2):
                        for j1 in range(2):
                            win = padded[img,
                                         8 + rowbase + j2: 8 + rowbase + j2 + YS,
                                         8 + colbase + j1: 8 + colbase + j1 + XS]
                            out[call, 16 * G + l, :, 4 * slot + 2 * j2 + j1] = \
                                win.reshape(-1)
    return out.reshape(2, 128, LNUM * DCH)


def make_inputs(core, t1_feature, t2_feature, t2_pred, w_down1, w_down2,
                w_flow1, bn_gamma, bn_beta, bn_mean, bn_var, w_flow2):
    q, h = core // 2, core % 2
    f1 = _feat_slab(t1_feature[q], h)
    f2 = _feat_slab(t2_feature[q], h)
    wd = np.stack([
        np.stack([w_down1[:, 128 * k:128 * (k + 1), 0, 0].T for k in range(2)]),
        np.stack([w_down2[:, 128 * k:128 * (k + 1), 0, 0].T for k in range(2)]),
    ]).transpose(2, 0, 1, 3).astype(BF).copy()        # (128,2,2,64)
    wf1 = np.stack([w_flow1[:, :, t // 3, t % 3].T for t in range(9)],
                   axis=1).astype(BF).copy()          # (128,9,64)
    wf2h = np.stack([w_flow2[:, :, t // 3, t % 3].T for t in range(9)],
                    axis=1).astype(BF)                # (64,9,2)
    z = np.zeros_like(wf2h)
    wf2 = np.stack([np.concatenate([wf2h, z], axis=0),
                    np.concatenate([z, wf2h], axis=0)],
                   axis=1).copy()                     # (128,2,9,2)
    scale = bn_gamma / np.sqrt(bn_var + 1e-5)
    bias = bn_beta - bn_mean * scale
    bn1 = np.stack([scale, bias], axis=1).reshape(T, 2, 1).astype(np.float32)
    bn = np.concatenate([bn1, bn1], axis=0)           # (128,2,1)
    mask = np.ones((128, 2, 1), np.float32)
    if h == 0:
        mask[0:T, 0] = 0.0   # x row 0 (half A) = image row -1
    else:
        mask[T:128, 1] = 0.0  # x half-B row 35 = x row 65 = image row 128
    bx, by, xlo, xhi, ylo, yhi = _host_constants(q, h)
    imgs = img_list(q)
    pred_imgs = np.stack([t2_pred[n, ch] for (n, ch) in imgs])
    dsrc = _dsrc_build(pred_imgs, h)
    emat = np.zeros((8, 128), BF)
    for Gi in range(8):
        emat[Gi, 16 * Gi:16 * (Gi + 1)] = 1.0
    return {
        "f1": f1, "f2": f2, "wd": wd, "wf1": wf1, "wf2": wf2,
        "bn": bn, "mask": mask, "bx": bx, "by": by, "ylo": ylo, "yhi": yhi,
        "xlo": xlo, "xhi": xhi, "emat": emat, "dsrc": dsrc.astype(BF),
    }


_NC_CACHE = {}


def kernel(**inputs):
    from concourse.bass_utils import run_bass_kernel_spmd
    if "nc" not in _NC_CACHE:
        _NC_CACHE["nc"] = build_nc()
    nc = _NC_CACHE["nc"]
    in_maps = [make_inputs(c, **inputs) for c in range(8)]
    res = run_bass_kernel_spmd(nc, in_maps, list(range(8)))
    out = np.zeros((NB, CCLS, H, W), np.float32)
    for c in range(8):
        q, h = c // 2, c % 2
        o = res.results[c]["out"]
        for i, (n, ch) in enumerate(img_list(q)):
            out[n, ch, 64 * h:64 * (h + 1), :] = o[i]
    return out



# revision 38
# speedup vs baseline: 1.4347x; 1.1889x over previous
"""Trainium2 Bass kernel for nn_AlignModule_full (8 NeuronCores, data-parallel).

Reference computation: two 1x1 convs -> concat -> 3x3 conv + BN + ReLU ->
3x3 conv -> flow -> bilinear grid_sample warp of t2_pred, where output
channel (n, ch) is warped with flow[(3n+ch) % 4] (torch flow.repeat
semantics faithfully ported by the reference).

Sharding: core c = (q, h), q = c//2 flow batch, h = c%2 row half.
Each core computes flow(q, rows 64h..64h+64) from batch-q features, then
warps the 19 (n, ch) images with (3n+ch)%4 == q for its row half, using
only its own flow. Zero cross-core communication.

Warp implementation: per-pixel bilinear gather via gpsimd ap_gather with a
host-built d=8 interleaved source: each index fetches the full 2x2 patch
for TWO image slots at once (19 images = 16 lanes x 2 slots).
"""
import sys

for _p in ('/opt/trn_rl_repo',):
    if _p not in sys.path:
        sys.path.append(_p)

import numpy as np
import ml_dtypes

import concourse.bass as bass
import concourse.bacc as bacc
import concourse.mybir as mybir
import concourse.tile as tile

F32 = mybir.dt.float32
BF16 = mybir.dt.bfloat16
I16 = mybir.dt.int16
AF = mybir.ActivationFunctionType
ALU = mybir.AluOpType

H, W, CIN, T, CCLS, NB = 128, 256, 256, 64, 19, 4
SLAB_R = 68          # feature slab rows
WS = 258             # padded width for t/x buffers
XR = 66              # x rows total
XH = 36              # x rows per partition-half (A: 0..36, B: 30..66)
YS, XS = 76, 26      # gather slab rows/cols per (group, call=col-half)
LNUM = YS * XS       # base positions per partition
DCH = 8              # interleave chunk: 2 slots x (2x2 patch)
NIDX = 1024          # gather indices per group per call
ROWB = 6             # slab row margin before first output row of the call
COLB = 5             # slab col margin before group col block

BF = ml_dtypes.bfloat16


def img_list(q):
    return [(n, ch) for n in range(NB) for ch in range(CCLS)
            if (3 * n + ch) % 4 == q]


def build_nc():
    nc = bacc.Bacc(None, target_bir_lowering=False, debug=False)
    P = nc.declare_dram_parameter
    f1_d = P("f1", [2, 128, SLAB_R, W], BF16, isOutput=False)
    f2_d = P("f2", [2, 128, SLAB_R, W], BF16, isOutput=False)
    wd_d = P("wd", [128, 2, 2, T], BF16, isOutput=False)
    wf1_d = P("wf1", [128, 9, T], BF16, isOutput=False)
    wf2_d = P("wf2", [128, 2, 9, 2], BF16, isOutput=False)
    bn_d = P("bn", [128, 2, 1], F32, isOutput=False)
    mask_d = P("mask", [128, 2, 1], F32, isOutput=False)
    bx_d = P("bx", [128, 128], F32, isOutput=False)
    by_d = P("by", [128, 128], F32, isOutput=False)
    ylo_d = P("ylo", [128, 128], F32, isOutput=False)
    yhi_d = P("yhi", [128, 128], F32, isOutput=False)
    xlo_d = P("xlo", [128, 128], F32, isOutput=False)
    xhi_d = P("xhi", [128, 128], F32, isOutput=False)
    emat_d = P("emat", [8, 128], BF16, isOutput=False)
    dsrc_d = P("dsrc", [2, 128, LNUM * DCH], BF16, isOutput=False)
    out_d = P("out", [CCLS, 64, W], F32, isOutput=True)

    flow_dramh = [nc.dram_tensor("flow_t0", [2, W, 32], BF16),
                  nc.dram_tensor("flow_t1", [2, W, 32], BF16)]  # (ch, col, row-half)

    NRB = 4               # feature rows per DMA batch
    NBATCH = (SLAB_R + NRB - 1) // NRB  # 12 (last batch 2 rows)

    with tile.TileContext(nc) as tc:
        with (
            tc.tile_pool(name="stream", bufs=3) as sp,
            tc.tile_pool(name="big", bufs=1) as bp,
            tc.tile_pool(name="psA", bufs=2, space="PSUM") as pp,
        ):
            # ---- feature batches: [128, feat, ck, NRB, W] bf16, 2-deep ring
            fts = {}

            def load_batch(b):
                r0 = NRB * b
                nr = min(NRB, SLAB_R - r0)
                ft = bp.tile([128, 2, 2, NRB, W], BF16, tag="fbatch",
                             name=f"fb{b}", bufs=3)
                for fi, fd in ((0, f1_d), (1, f2_d)):
                    src = bass.AP(tensor=fd, offset=r0 * W,
                                  ap=[[SLAB_R * W, 128], [128 * SLAB_R * W, 2],
                                      [W, nr], [1, W]])
                    (nc.sync if fi == 0 else nc.scalar).dma_start(
                        ft[:, fi, :, 0:nr, :], src)
                fts[b] = ft

            load_batch(0)

            # ---- constants (spread across both HWDGE queues) ----
            wd_s = bp.tile([128, 2, 2, T], BF16, tag="wd")
            wf1_s = bp.tile([128, 9, T], BF16, tag="wf1")
            wf2_s = bp.tile([128, 2, 9, 2], BF16, tag="wf2")
            bn_s = bp.tile([128, 2, 1], F32, tag="bn")
            mask_s = bp.tile([128, 2, 1], F32, tag="mask")
            xlo_s = bp.tile([128, 128], F32, tag="xlo")
            xhi_s = bp.tile([128, 128], F32, tag="xhi")
            emat_s = bp.tile([8, 128], BF16, tag="emat")
            bx_s = bp.tile([128, 128], F32, tag="bx")
            by_s = bp.tile([128, 128], F32, tag="by")
            ylo_s = bp.tile([128, 128], F32, tag="ylo")
            yhi_s = bp.tile([128, 128], F32, tag="yhi")
            for i, (t_, d_) in enumerate((
                    (wd_s, wd_d), (wf1_s, wf1_d), (wf2_s, wf2_d),
                    (bn_s, bn_d), (mask_s, mask_d), (xlo_s, xlo_d),
                    (xhi_s, xhi_d),
                    (emat_s, emat_d), (bx_s, bx_d), (by_s, by_d),
                    (ylo_s, ylo_d), (yhi_s, yhi_d))):
                (nc.sync if i % 2 else nc.scalar).dma_start(t_[:], d_[:])

            # ---- big shared tiles; gather sources loaded up front (SWDGE) ----
            t_cat = bp.tile([128, SLAB_R * WS], BF16, tag="tcat_gat")
            dsrc = bp.tile([128, LNUM * DCH], BF16, tag="dsrc")
            dsrc2 = bp.tile([128, LNUM * DCH], BF16, tag="dsrc2")
            # dsrc loads happen post-P1-solo on the HWDGE queues so startup
            # HBM bandwidth is all features.  GpSimd runs only ap_gather +
            # tensor_tensor, and a dummy gather below pre-loads the gather
            # ucode lib so no IRAM swap lands on the critical path.
            x_sb = bp.tile([128, XH * WS], BF16, tag="x_w4")

            dum_src = sp.tile([128, 8], BF16, tag="dumg", bufs=1)
            dum_idx = sp.tile([128, 1], I16, tag="dumi", bufs=1)
            dum_out = sp.tile([128, 32], BF16, tag="dumo", bufs=1)
            nc.vector.memset(dum_src[:], 0.0)
            nc.vector.memset(dum_idx[:], 0)
            nc.gpsimd.ap_gather(dum_out[:], dum_src[:], dum_idx[:],
                                channels=128, num_elems=4, d=2, num_idxs=16)

            load_batch(1)

            t3 = t_cat[:].rearrange("p (r c) -> p r c", r=SLAB_R, c=WS)
            nc.vector.memset(t3[:, :, 0:1], 0.0)
            nc.vector.memset(t3[:, :, 257:258], 0.0)

            # ---- phases 1+2 interleaved: 1x1 convs feed 3x3 conv ----
            def p1_tile(it):
                r0 = 2 * it
                b, rr = r0 // NRB, r0 % NRB
                if rr == 0 and b + 1 < NBATCH and (b + 1) not in fts:
                    load_batch(b + 1)
                ft = fts[b]
                ps = pp.tile([128, 2 * W], F32, tag="pst", name="pst")
                for ck in range(2):
                    nc.tensor.matmul(ps[0:T, :], wd_s[:, 0, ck, :],
                                     ft[:, 0, ck, rr:rr + 2, :],
                                     start=(ck == 0), stop=(ck == 1),
                                     tile_position=(0, 0),
                                     skip_group_check=True)
                    nc.tensor.matmul(ps[T:128, :], wd_s[:, 1, ck, :],
                                     ft[:, 1, ck, rr:rr + 2, :],
                                     start=(ck == 0), stop=(ck == 1),
                                     tile_position=(0, 64),
                                     skip_group_check=True)
                dst = bass.AP(tensor=t_cat.tensor, offset=r0 * WS + 1,
                              ap=[[SLAB_R * WS, 128], [WS, 2], [1, W]])
                nc.vector.tensor_copy(dst, ps[:].rearrange("p (r c) -> p r c",
                                                           r=2, c=W))

            x3 = x_sb[:].rearrange("p (r c) -> p r c", r=XH, c=WS)
            nc.vector.memset(x3[:, :, 0:1], 0.0)
            nc.vector.memset(x3[:, :, 257:258], 0.0)

            def p2_iter(it):
                jA = 2 * it
                jB = 30 + 2 * it
                ps = pp.tile([128, 2 * W], F32, tag="psx", name="psx")
                for tap in range(9):
                    dy, dx = tap // 3, tap % 3
                    rhsA = bass.AP(tensor=t_cat.tensor,
                                   offset=(jA + dy) * WS + dx,
                                   ap=[[SLAB_R * WS, 128], [WS, 2], [1, W]])
                    rhsB = bass.AP(tensor=t_cat.tensor,
                                   offset=(jB + dy) * WS + dx,
                                   ap=[[SLAB_R * WS, 128], [WS, 2], [1, W]])
                    nc.tensor.matmul(ps[0:T, :], wf1_s[:, tap, :], rhsA,
                                     start=(tap == 0), stop=(tap == 8),
                                     tile_position=(0, 0),
                                     skip_group_check=True)
                    nc.tensor.matmul(ps[T:128, :], wf1_s[:, tap, :], rhsB,
                                     start=(tap == 0), stop=(tap == 8),
                                     tile_position=(0, 64),
                                     skip_group_check=True)
                dstA = bass.AP(tensor=x_sb.tensor, offset=jA * WS + 1,
                               ap=[[XH * WS, T], [WS, 2], [1, W]])
                dstB = bass.AP(tensor=x_sb.tensor,
                               offset=T * (XH * WS) + jA * WS + 1,
                               ap=[[XH * WS, T], [WS, 2], [1, W]])
                nc.scalar.activation(dstA,
                                     ps[0:T].rearrange("p (r c) -> p r c", r=2, c=W),
                                     AF.Relu, bias=bn_s[0:T, 1], scale=bn_s[0:T, 0])
                nc.scalar.activation(dstB,
                                     ps[T:128].rearrange("p (r c) -> p r c", r=2, c=W),
                                     AF.Relu, bias=bn_s[T:128, 1], scale=bn_s[T:128, 0])

            for it in range(18):
                p1_tile(it)
            for it in range(18):
                p2_iter(it)
                if 18 + it < SLAB_R // 2:
                    p1_tile(18 + it)
                if it == 2:
                    nc.sync.dma_start(dsrc[:], dsrc_d[0, :, :])
                if it == 8:
                    nc.scalar.dma_start(dsrc2[:], dsrc_d[1, :, :])
            nc.vector.tensor_scalar_mul(x3[0:T, 0, :], x3[0:T, 0, :], mask_s[0:T, 0])
            nc.vector.tensor_scalar_mul(x3[T:128, 35, :], x3[T:128, 35, :],
                                        mask_s[T:128, 1])

            # ---- phase 3: 3x3 conv 64->2, two tiles concurrent via PE
            # column strips.  First 8 pairs cover flow rows 0..31 (rh0) so
            # the warp pipeline for rh0 can start while rh1 still computes.
            def p3_pair(iA, iB):
                # PE column strips: out PSUM start partition must equal the
                # tile-position column, so pos-1 writes partitions 64:66.
                tiles = [(iA, 0)] + ([(iB, 1)] if iB is not None else [])
                pss = []
                for i0, pos in tiles:
                    t_ = pp.tile([128, 2 * W], F32,
                                 tag=("psf" if pos == 0 else "psfB"),
                                 name="psf", bufs=2)
                    pss.append(t_[64 * pos:64 * pos + 2])
                for tap in range(9):
                    dy, dx = tap // 3, tap % 3
                    for (i0, pos), ps in zip(tiles, pss):
                        hf = 0 if i0 < 34 else 1
                        base = i0 + dy - 30 * hf
                        rhs = bass.AP(tensor=x_sb.tensor,
                                      offset=base * WS + dx,
                                      ap=[[XH * WS, 128], [WS, 2], [1, W]])
                        nc.tensor.matmul(ps, wf2_s[:, hf, tap, :], rhs,
                                         start=(tap == 0), stop=(tap == 8),
                                         tile_position=(0, 64 * pos),
                                         skip_group_check=True)
                for (i0, pos), ps in zip(tiles, pss):
                    # stage rows into the per-half SBUF accumulator; the
                    # DRAM write happens once per row-half (2 descriptors)
                    bt_v = bass.AP(tensor=btbig[i0 // 32].tensor,
                                   offset=i0 % 32,
                                   ap=[[8192, 2], [1, 2], [32, W]])
                    src = ps.rearrange("p (r c) -> p r c", r=2, c=W)
                    if pos == 0:
                        nc.vector.tensor_copy(bt_v, src)
                    else:
                        nc.scalar.copy(bt_v, src)

            p3_pairs_rh0 = [(2 * i, 16 + 2 * i) for i in range(8)]
            p3_pairs_rh1 = [(32, 34)] + [(36 + 4 * i, 38 + 4 * i)
                                         for i in range(7)]
            # one buffer, reused for rh1 after rh0's DRAM write, and later
            # reused again as w_g (tag ring, WAR-tracked)
            btbig = {0: bp.tile([2, W * 32], BF16, tag="wg", name="btb0"),
                     1: bp.tile([2, W * 32], BF16, tag="wg", name="btb1")}

            # ---- phase 4/5: flow -> CL + index math + gathers, by row half ----
            cl_fx = bp.tile([128, 128], BF16, tag="clfx")
            cl_fy = bp.tile([128, 128], BF16, tag="clfy")

            def cl(tag):
                tt = bp.tile([128, 128], F32, tag=tag, name=tag)
                return tt

            ix = cl("ix"); iy = cl("iy"); tmp = cl("tmp")
            x0i = bp.tile([128, 128], I16, tag="x0i")
            y0i = bp.tile([128, 128], I16, tag="y0i")
            x0f = cl("x0f"); y0f = cl("y0f")
            ef = cl("ef")
            eidx = bp.tile([128, 128], I16, tag="eidx")
            gatall = bp.tile([128, 2 * NIDX * DCH], BF16, tag="tcat_gat")
            _qs = [nc.sync, nc.scalar]
            _qi = 0

            def cl_load(rh):
                # one DMA per (ch, w): contiguous 32-row runs from the
                # (ch, col, row) flow file into CL partitions
                for ch, dtile in ((0, cl_fx), (1, cl_fy)):
                    for w in range(2):
                        dst = bass.AP(tensor=dtile.tensor,
                                      offset=64 * w + 32 * rh,
                                      ap=[[128, 128], [1, 32]])
                        srcp = bass.AP(
                            tensor=flow_dramh[rh],
                            offset=ch * W * 32 + 16 * w * 32,
                            ap=[[32 * 32, 8], [32, 16], [1, 32]])
                        _qs[(ch + w) % 2].dma_start(dst, srcp)

            def idx_math(rh):
                # both w column-halves in one 2D-sliced op set
                V = nc.vector

                def S(t):
                    return bass.AP(tensor=t.tensor, offset=32 * rh,
                                   ap=[[128, 128], [64, 2], [1, 32]])

                def S16(t):
                    return bass.AP(tensor=t.tensor, offset=32 * rh,
                                   ap=[[128, 128], [64, 2], [1, 32]])

                V.tensor_scalar_mul(S(ix), S(cl_fx), 0.5)
                V.tensor_tensor(S(ix), S(ix), S(bx_s), ALU.add)
                V.tensor_scalar_mul(S(iy), S(cl_fy), 0.5)
                V.tensor_tensor(S(iy), S(iy), S(by_s), ALU.add)
                V.tensor_copy(S16(x0i), S(ix))
                V.tensor_copy(S(x0f), S16(x0i))
                V.tensor_tensor(S(tmp), S(x0f), S(ix), ALU.is_gt)
                V.tensor_tensor(S(x0f), S(x0f), S(tmp), ALU.subtract)
                V.tensor_copy(S16(y0i), S(iy))
                V.tensor_copy(S(y0f), S16(y0i))
                V.tensor_tensor(S(tmp), S(y0f), S(iy), ALU.is_gt)
                V.tensor_tensor(S(y0f), S(y0f), S(tmp), ALU.subtract)
                V.tensor_scalar_mul(S(ef), S(y0f), float(XS))
                V.tensor_tensor(S(ef), S(ef), S(x0f), ALU.add)
                V.tensor_scalar(S(ef), S(ef), 0.0, float(LNUM - XS - 2),
                                ALU.max, ALU.min)
                V.tensor_copy(S16(eidx), S(ef))

            # ---- weights math (per row-half, overlapped with P3) ----
            fx = cl("fx"); fy = cl("fy")
            vx0 = cl("vx0"); vx1 = cl("vx1"); vy0 = cl("vy0"); vy1 = cl("vy1")
            xp1 = cl("xp1"); yp1 = cl("yp1")
            gx0 = cl("gx0"); gx1 = cl("gx1"); gy0 = cl("gy0"); gy1 = cl("gy1")
            wsall = bp.tile([128, 4, 128], BF16, tag="wsall")

            def weights_math(rh):
                V = nc.vector

                def S(t):
                    return bass.AP(tensor=t.tensor, offset=32 * rh,
                                   ap=[[128, 128], [64, 2], [1, 32]])

                def SW(s):
                    return bass.AP(tensor=wsall.tensor,
                                   offset=s * 128 + 32 * rh,
                                   ap=[[4 * 128, 128], [64, 2], [1, 32]])

                V.tensor_tensor(S(fx), S(ix), S(x0f), ALU.subtract)
                V.tensor_tensor(S(fy), S(iy), S(y0f), ALU.subtract)
                V.tensor_scalar_add(S(xp1), S(x0f), 1.0)
                V.tensor_scalar_add(S(yp1), S(y0f), 1.0)
                for vt, src_f in ((vx0, x0f), (vx1, xp1)):
                    V.tensor_tensor(S(vt), S(src_f), S(xlo_s), ALU.is_ge)
                    V.tensor_tensor(S(tmp), S(src_f), S(xhi_s), ALU.is_le)
                    V.tensor_tensor(S(vt), S(vt), S(tmp), ALU.mult)
                for vt, src_f in ((vy0, y0f), (vy1, yp1)):
                    V.tensor_tensor(S(vt), S(src_f), S(ylo_s), ALU.is_ge)
                    V.tensor_tensor(S(tmp), S(src_f), S(yhi_s), ALU.is_le)
                    V.tensor_tensor(S(vt), S(vt), S(tmp), ALU.mult)
                V.tensor_scalar(S(tmp), S(fx), -1.0, 1.0, ALU.mult, ALU.add)
                V.tensor_tensor(S(gx0), S(tmp), S(vx0), ALU.mult)
                V.tensor_tensor(S(gx1), S(fx), S(vx1), ALU.mult)
                V.tensor_scalar(S(tmp), S(fy), -1.0, 1.0, ALU.mult, ALU.add)
                V.tensor_tensor(S(gy0), S(tmp), S(vy0), ALU.mult)
                V.tensor_tensor(S(gy1), S(fy), S(vy1), ALU.mult)
                V.tensor_tensor(SW(0), S(gx0), S(gy0), ALU.mult)
                V.tensor_tensor(SW(1), S(gx1), S(gy0), ALU.mult)
                V.tensor_tensor(SW(2), S(gx0), S(gy1), ALU.mult)
                V.tensor_tensor(SW(3), S(gx1), S(gy1), ALU.mult)

            def warp_front(rh):
                nc.sync.dma_start(flow_dramh[rh][:], btbig[rh][:])
                cl_load(rh)
                idx_math(rh)
                for w in range(2):
                    sl = slice(64 * w + 32 * rh, 64 * w + 32 * rh + 32)
                    ds = dsrc if w == 0 else dsrc2
                    off = w * (NIDX * DCH) + rh * 4096
                    nc.gpsimd.ap_gather(
                        gatall[:, off:off + 4096], ds[:],
                        eidx[:, sl],
                        channels=128, num_elems=LNUM, d=DCH, num_idxs=512)
                weights_math(rh)

            for a, b in p3_pairs_rh0:
                p3_pair(a, b)
            warp_front(0)
            for a, b in p3_pairs_rh1:
                p3_pair(a, b)
            warp_front(1)

            # ---- phase 6: weight planes -> w_g (SBUF->SBUF) -> W4 ----
            w_g = bp.tile([8, 4, 2048], BF16, tag="wg")
            for s in range(4):
                (nc.sync if s % 2 else nc.scalar).dma_start(
                    bass.AP(tensor=w_g.tensor, offset=s * 2048,
                            ap=[[4 * 2048, 8], [128, 16], [1, 128]]),
                    wsall[:, s, :])
            # W4 [128, 4, 2048] j-ordered (j = 32r + 16w + m over full 64 rows)
            w4 = bp.tile([128, 4 * 2048], F32, tag="x_w4")
            for s in range(4):
                for c4 in (2, 3, 0, 1):   # call-1 chunks first (feed gpsimd)
                    pw = pp.tile([128, 512], F32, tag="pst", name="pw")
                    nc.tensor.matmul(pw[:], emat_s[:], w_g[:, s, 512 * c4:512 * (c4 + 1)],
                                     start=True, stop=True)
                    # pw free = (m-part: 128,4)(r: 2,64)(w: 1,2) for m in [4c4, 4c4+4)
                    dstw = bass.AP(tensor=w4.tensor,
                                   offset=s * 2048 + 4 * c4,
                                   ap=[[4 * 2048, 128], [1, 4], [1024, 2], [16, 64]])
                    src_w = pw[:].rearrange("p (m w r) -> p m w r", m=4, w=2, r=64)
                    nc.scalar.copy(dstw, src_w)

            # Two independent combine chains: call 0 on VectorE, call 1 on
            # GpSimd, each with its own scratch so they run concurrently.
            # pls reuse the gather-source buffers (dead after the last gather)
            pls_c = [bp.tile([128, 4, NIDX], BF16, tag=("dsrc", "dsrc2")[c],
                             name=f"pls{c}")
                     for c in range(2)]
            bb_c = [bp.tile([128, 2, NIDX], F32, tag=f"bbc{c}", name=f"bbc{c}")
                    for c in range(2)]
            def combine(call, slot, eng, pls):
                bbt = bb_c[call]
                for s in range(4):
                    g_v = bass.AP(tensor=gatall.tensor,
                                  offset=call * NIDX * DCH + 4 * slot + s,
                                  ap=[[2 * NIDX * DCH, 128], [DCH, NIDX]])
                    eng.tensor_tensor(
                        pls[:, s, :], g_v,
                        w4[:, (s * 2048 + 1024 * call):(s * 2048 + 1024 * call + NIDX)],
                        ALU.mult)
                eng.tensor_tensor(pls[:, 0, :], pls[:, 0, :], pls[:, 1, :],
                                  ALU.add)
                eng.tensor_tensor(pls[:, 2, :], pls[:, 2, :], pls[:, 3, :],
                                  ALU.add)
                eng.tensor_tensor(bbt[:, slot, :], pls[:, 0, :], pls[:, 2, :],
                                  ALU.add)
                nl = 16 if slot == 0 else 3
                for G in range(8):
                    dst = bass.AP(
                        tensor=out_d,
                        offset=(16 * slot) * 64 * W + 32 * G + 16 * call,
                        ap=[[64 * W, nl], [W, 64], [1, 16]])
                    srcb = bass.AP(
                        tensor=bbt.tensor,
                        offset=(16 * G) * (2 * NIDX) + slot * NIDX,
                        ap=[[2 * NIDX, nl], [16, 64], [1, 16]])
                    (nc.scalar if eng is nc.gpsimd else nc.sync).dma_start(
                        dst, srcb)

            # gpsimd takes one quarter (it is ~1.5x slower per op and pays
            # a ucode lib swap); vector takes the other three.
            combine(1, 0, nc.gpsimd, pls_c[1])
            combine(0, 0, nc.vector, pls_c[0])
            combine(0, 1, nc.vector, pls_c[0])
            combine(1, 1, nc.vector, pls_c[0])
    nc.finalize()
    return nc


# ======================= host-side prep =======================

def _feat_slab(feat_b, h):
    """feat_b (256, 128, 256) f32 -> (2, 128, 68, 256) bf16 slab for half h."""
    r0 = 64 * h - 2
    slab = np.zeros((CIN, SLAB_R, W), np.float32)
    lo, hi = max(r0, 0), min(r0 + SLAB_R, H)
    slab[:, lo - r0:hi - r0, :] = feat_b[:, lo:hi, :]
    return np.ascontiguousarray(
        slab.reshape(2, 128, SLAB_R, W).astype(BF))


def _host_constants(q, h):
    R0 = 64 * h
    # CL layout: p = 16G + m, f = 64w + r; pixel (row R0+r, col 32G+16w+m)
    p = np.arange(128)[:, None]
    f = np.arange(128)[None, :]
    G = p // 16
    m = p % 16
    r = f % 64
    w = f // 64
    col = 32 * G + 16 * w + m
    row = R0 + r
    ix_base = col + col / (W - 1.0) - 0.5
    iy_base = row + row / (H - 1.0) - 0.5
    colbase = 32 * G + 16 * w - COLB
    rowbase = R0 - ROWB
    bx = np.broadcast_to(ix_base - colbase, (128, 128)).astype(np.float32).copy()
    by = np.broadcast_to(iy_base - rowbase, (128, 128)).astype(np.float32).copy()
    xlo = np.broadcast_to(0.0 - colbase, (128, 128)).astype(np.float32).copy()
    xhi = np.broadcast_to((W - 1.0) - colbase, (128, 128)).astype(np.float32).copy()
    ylo = np.full((128, 128), 0.0 - rowbase, np.float32)
    yhi = np.full((128, 128), (H - 1.0) - rowbase, np.float32)
    return bx, by, xlo, xhi, ylo, yhi


def _dsrc_build(pred_imgs, h):
    """pred_imgs: (19, 128, 256) f32. Returns (2, 128, LNUM*8) f32 gather
    source; call = col-half w, slab = rows [R0-6, R0+70) x 26-col band."""
    R0 = 64 * h
    padded = np.zeros((CCLS, H + 16, W + 16), np.float32)
    padded[:, 8:8 + H, 8:8 + W] = pred_imgs
    out = np.zeros((2, 128, LNUM, DCH), np.float32)
    rowbase = R0 - ROWB
    for call in range(2):
        for G in range(8):
            colbase = 32 * G + 16 * call - COLB
            for l in range(16):
                for slot in range(2):
                    img = l + 16 * slot
                    if img >= CCLS:
                        img = l
                    for j2 in range(2):
                        for j1 in range(2):
                            win = padded[img,
                                         8 + rowbase + j2: 8 + rowbase + j2 + YS,
                                         8 + colbase + j1: 8 + colbase + j1 + XS]
                            out[call, 16 * G + l, :, 4 * slot + 2 * j2 + j1] = \
                                win.reshape(-1)
    return out.reshape(2, 128, LNUM * DCH)


def make_inputs(core, t1_feature, t2_feature, t2_pred, w_down1, w_down2,
                w_flow1, bn_gamma, bn_beta, bn_mean, bn_var, w_flow2):
    q, h = core // 2, core % 2
    f1 = _feat_slab(t1_feature[q], h)
    f2 = _feat_slab(t2_feature[q], h)
    wd = np.stack([
        np.stack([w_down1[:, 128 * k:128 * (k + 1), 0, 0].T for k in range(2)]),
        np.stack([w_down2[:, 128 * k:128 * (k + 1), 0, 0].T for k in range(2)]),
    ]).transpose(2, 0, 1, 3).astype(BF).copy()        # (128,2,2,64)
    wf1 = np.stack([w_flow1[:, :, t // 3, t % 3].T for t in range(9)],
                   axis=1).astype(BF).copy()          # (128,9,64)
    wf2h = np.stack([w_flow2[:, :, t // 3, t % 3].T for t in range(9)],
                    axis=1).astype(BF)                # (64,9,2)
    z = np.zeros_like(wf2h)
    wf2 = np.stack([np.concatenate([wf2h, z], axis=0),
                    np.concatenate([z, wf2h], axis=0)],
                   axis=1).copy()                     # (128,2,9,2)
    scale = bn_gamma / np.sqrt(bn_var + 1e-5)
    bias = bn_beta - bn_mean * scale
    bn1 = np.stack([scale, bias], axis=1).reshape(T, 2, 1).astype(np.float32)
    bn = np.concatenate([bn1, bn1], axis=0)           # (128,2,1)
    mask = np.ones((128, 2, 1), np.float32)
    if h == 0:
        mask[0:T, 0] = 0.0   # x row 0 (half A) = image row -1
    else:
        mask[T:128, 1] = 0.0  # x half-B row 35 = x row 65 = image row 128
    bx, by, xlo, xhi, ylo, yhi = _host_constants(q, h)
    imgs = img_list(q)
    pred_imgs = np.stack([t2_pred[n, ch] for (n, ch) in imgs])
    dsrc = _dsrc_build(pred_imgs, h)
    emat = np.zeros((8, 128), BF)
    for Gi in range(8):
        emat[Gi, 16 * Gi:16 * (Gi + 1)] = 1.0
    return {
        "f1": f1, "f2": f2, "wd": wd, "wf1": wf1, "wf2": wf2,
        "bn": bn, "mask": mask, "bx": bx, "by": by, "ylo": ylo, "yhi": yhi,
        "xlo": xlo, "xhi": xhi, "emat": emat, "dsrc": dsrc.astype(BF),
    }


_NC_CACHE = {}


def kernel(**inputs):
    from concourse.bass_utils import run_bass_kernel_spmd
    if "nc" not in _NC_CACHE:
        _NC_CACHE["nc"] = build_nc()
    nc = _NC_CACHE["nc"]
    in_maps = [make_inputs(c, **inputs) for c in range(8)]
    res = run_bass_kernel_spmd(nc, in_maps, list(range(8)))
    out = np.zeros((NB, CCLS, H, W), np.float32)
    for c in range(8):
        q, h = c // 2, c % 2
        o = res.results[c]["out"]
        for i, (n, ch) in enumerate(img_list(q)):
            out[n, ch, 64 * h:64 * (h + 1), :] = o[i]
    return out



# revision 48
# speedup vs baseline: 1.4400x; 1.0037x over previous
"""Trainium2 Bass kernel for nn_AlignModule_full (8 NeuronCores, data-parallel).

Reference computation: two 1x1 convs -> concat -> 3x3 conv + BN + ReLU ->
3x3 conv -> flow -> bilinear grid_sample warp of t2_pred, where output
channel (n, ch) is warped with flow[(3n+ch) % 4] (torch flow.repeat
semantics faithfully ported by the reference).

Sharding: core c = (q, h), q = c//2 flow batch, h = c%2 row half.
Each core computes flow(q, rows 64h..64h+64) from batch-q features, then
warps the 19 (n, ch) images with (3n+ch)%4 == q for its row half, using
only its own flow. Zero cross-core communication.

Warp implementation: per-pixel bilinear gather via gpsimd ap_gather with a
host-built d=8 interleaved source: each index fetches the full 2x2 patch
for TWO image slots at once (19 images = 16 lanes x 2 slots).
"""
import sys

for _p in ('/opt/trn_rl_repo',):
    if _p not in sys.path:
        sys.path.append(_p)

import numpy as np
import ml_dtypes

import concourse.bass as bass
import concourse.bacc as bacc
import concourse.mybir as mybir
import concourse.tile as tile

F32 = mybir.dt.float32
BF16 = mybir.dt.bfloat16
I16 = mybir.dt.int16
AF = mybir.ActivationFunctionType
ALU = mybir.AluOpType

H, W, CIN, T, CCLS, NB = 128, 256, 256, 64, 19, 4
SLAB_R = 68          # feature slab rows
WS = 258             # padded width for t/x buffers
XR = 66              # x rows total
XH = 36              # x rows per partition-half (A: 0..36, B: 30..66)
YS, XS = 76, 26      # gather slab rows/cols per (group, call=col-half)
LNUM = YS * XS       # base positions per partition
DCH = 8              # interleave chunk: 2 slots x (2x2 patch)
NIDX = 1024          # gather indices per group per call
ROWB = 6             # slab row margin before first output row of the call
COLB = 5             # slab col margin before group col block

BF = ml_dtypes.bfloat16


def img_list(q):
    return [(n, ch) for n in range(NB) for ch in range(CCLS)
            if (3 * n + ch) % 4 == q]


def build_nc():
    nc = bacc.Bacc(None, target_bir_lowering=False, debug=False)
    P = nc.declare_dram_parameter
    f1_d = P("f1", [2, 128, SLAB_R, W], BF16, isOutput=False)
    f2_d = P("f2", [2, 128, SLAB_R, W], BF16, isOutput=False)
    wd_d = P("wd", [128, 2, 2, T], BF16, isOutput=False)
    wf1_d = P("wf1", [128, 9, T], BF16, isOutput=False)
    wf2_d = P("wf2", [128, 2, 9, 2], BF16, isOutput=False)
    bn_d = P("bn", [128, 2, 1], F32, isOutput=False)
    mask_d = P("mask", [128, 2, 1], F32, isOutput=False)
    bx_d = P("bx", [128, 128], F32, isOutput=False)
    by_d = P("by", [128, 128], F32, isOutput=False)
    ylo_d = P("ylo", [128, 128], F32, isOutput=False)
    yhi_d = P("yhi", [128, 128], F32, isOutput=False)
    xlo_d = P("xlo", [128, 128], F32, isOutput=False)
    xhi_d = P("xhi", [128, 128], F32, isOutput=False)
    emat_d = P("emat", [8, 128], BF16, isOutput=False)
    dsrc_d = P("dsrc", [2, 128, LNUM * DCH], BF16, isOutput=False)
    out_d = P("out", [CCLS, 64, W], F32, isOutput=True)

    flow_dramh = [nc.dram_tensor("flow_t0", [2, W, 32], BF16),
                  nc.dram_tensor("flow_t1", [2, W, 32], BF16)]  # (ch, col, row-half)

    NRB = 4               # feature rows per DMA batch
    NBATCH = (SLAB_R + NRB - 1) // NRB  # 12 (last batch 2 rows)

    with tile.TileContext(nc) as tc:
        with (
            tc.tile_pool(name="stream", bufs=3) as sp,
            tc.tile_pool(name="big", bufs=1) as bp,
            tc.tile_pool(name="psA", bufs=2, space="PSUM") as pp,
        ):
            # ---- feature batches: [128, feat, ck, NRB, W] bf16, 2-deep ring
            fts = {}

            def load_batch(b):
                r0 = NRB * b
                nr = min(NRB, SLAB_R - r0)
                ft = bp.tile([128, 2, 2, NRB, W], BF16, tag="fbatch",
                             name=f"fb{b}", bufs=3)
                for fi, fd in ((0, f1_d), (1, f2_d)):
                    src = bass.AP(tensor=fd, offset=r0 * W,
                                  ap=[[SLAB_R * W, 128], [128 * SLAB_R * W, 2],
                                      [W, nr], [1, W]])
                    (nc.sync if fi == 0 else nc.scalar).dma_start(
                        ft[:, fi, :, 0:nr, :], src)
                fts[b] = ft

            load_batch(0)

            # ---- constants (spread across both HWDGE queues) ----
            wd_s = bp.tile([128, 2, 2, T], BF16, tag="wd")
            wf1_s = bp.tile([128, 9, T], BF16, tag="wf1")
            wf2_s = bp.tile([128, 2, 9, 2], BF16, tag="wf2")
            bn_s = bp.tile([128, 2, 1], F32, tag="bn")
            mask_s = bp.tile([128, 2, 1], F32, tag="mask")
            xlo_s = bp.tile([128, 128], F32, tag="xlo")
            xhi_s = bp.tile([128, 128], F32, tag="xhi")
            emat_s = bp.tile([8, 128], BF16, tag="emat")
            bx_s = bp.tile([128, 128], F32, tag="bx")
            by_s = bp.tile([128, 128], F32, tag="by")
            ylo_s = bp.tile([128, 128], F32, tag="ylo")
            yhi_s = bp.tile([128, 128], F32, tag="yhi")
            for i, (t_, d_) in enumerate((
                    (wd_s, wd_d), (wf1_s, wf1_d), (wf2_s, wf2_d),
                    (bn_s, bn_d), (mask_s, mask_d), (xlo_s, xlo_d),
                    (xhi_s, xhi_d),
                    (emat_s, emat_d), (bx_s, bx_d), (by_s, by_d),
                    (ylo_s, ylo_d), (yhi_s, yhi_d))):
                (nc.sync if i % 2 else nc.scalar).dma_start(t_[:], d_[:])

            # ---- big shared tiles; gather sources loaded up front (SWDGE) ----
            t_cat = bp.tile([128, SLAB_R * WS], BF16, tag="tcat_gat")
            dsrc = bp.tile([128, LNUM * DCH], BF16, tag="dsrc")
            dsrc2 = bp.tile([128, LNUM * DCH], BF16, tag="dsrc2")
            # dsrc loads happen post-P1-solo on the HWDGE queues so startup
            # HBM bandwidth is all features.  GpSimd runs only ap_gather +
            # tensor_tensor, and a dummy gather below pre-loads the gather
            # ucode lib so no IRAM swap lands on the critical path.
            x_sb = bp.tile([128, XH * WS], BF16, tag="x_w4")

            dum_src = sp.tile([128, 8], BF16, tag="dumg", bufs=1)
            dum_idx = sp.tile([128, 1], I16, tag="dumi", bufs=1)
            dum_out = sp.tile([128, 32], BF16, tag="dumo", bufs=1)
            nc.vector.memset(dum_src[:], 0.0)
            nc.vector.memset(dum_idx[:], 0)
            nc.gpsimd.ap_gather(dum_out[:], dum_src[:], dum_idx[:],
                                channels=128, num_elems=4, d=2, num_idxs=16)

            load_batch(1)

            t3 = t_cat[:].rearrange("p (r c) -> p r c", r=SLAB_R, c=WS)
            nc.vector.memset(t3[:, :, 0:1], 0.0)
            nc.vector.memset(t3[:, :, 257:258], 0.0)

            # ---- phases 1+2 interleaved: 1x1 convs feed 3x3 conv ----
            def p1_tile(it):
                r0 = 2 * it
                b, rr = r0 // NRB, r0 % NRB
                if rr == 0 and b + 1 < NBATCH and (b + 1) not in fts:
                    load_batch(b + 1)
                ft = fts[b]
                ps = pp.tile([128, 2 * W], F32, tag="pst", name="pst")
                for ck in range(2):
                    nc.tensor.matmul(ps[0:T, :], wd_s[:, 0, ck, :],
                                     ft[:, 0, ck, rr:rr + 2, :],
                                     start=(ck == 0), stop=(ck == 1),
                                     tile_position=(0, 0),
                                     skip_group_check=True)
                    nc.tensor.matmul(ps[T:128, :], wd_s[:, 1, ck, :],
                                     ft[:, 1, ck, rr:rr + 2, :],
                                     start=(ck == 0), stop=(ck == 1),
                                     tile_position=(0, 64),
                                     skip_group_check=True)
                dst = bass.AP(tensor=t_cat.tensor, offset=r0 * WS + 1,
                              ap=[[SLAB_R * WS, 128], [WS, 2], [1, W]])
                nc.vector.tensor_copy(dst, ps[:].rearrange("p (r c) -> p r c",
                                                           r=2, c=W))

            x3 = x_sb[:].rearrange("p (r c) -> p r c", r=XH, c=WS)
            nc.vector.memset(x3[:, :, 0:1], 0.0)
            nc.vector.memset(x3[:, :, 257:258], 0.0)

            def p2_iter(it):
                jA = 2 * it
                jB = 30 + 2 * it
                ps = pp.tile([128, 2 * W], F32, tag="psx", name="psx")
                for tap in range(9):
                    dy, dx = tap // 3, tap % 3
                    rhsA = bass.AP(tensor=t_cat.tensor,
                                   offset=(jA + dy) * WS + dx,
                                   ap=[[SLAB_R * WS, 128], [WS, 2], [1, W]])
                    rhsB = bass.AP(tensor=t_cat.tensor,
                                   offset=(jB + dy) * WS + dx,
                                   ap=[[SLAB_R * WS, 128], [WS, 2], [1, W]])
                    nc.tensor.matmul(ps[0:T, :], wf1_s[:, tap, :], rhsA,
                                     start=(tap == 0), stop=(tap == 8),
                                     tile_position=(0, 0),
                                     skip_group_check=True)
                    nc.tensor.matmul(ps[T:128, :], wf1_s[:, tap, :], rhsB,
                                     start=(tap == 0), stop=(tap == 8),
                                     tile_position=(0, 64),
                                     skip_group_check=True)
                dstA = bass.AP(tensor=x_sb.tensor, offset=jA * WS + 1,
                               ap=[[XH * WS, T], [WS, 2], [1, W]])
                dstB = bass.AP(tensor=x_sb.tensor,
                               offset=T * (XH * WS) + jA * WS + 1,
                               ap=[[XH * WS, T], [WS, 2], [1, W]])
                nc.scalar.activation(dstA,
                                     ps[0:T].rearrange("p (r c) -> p r c", r=2, c=W),
                                     AF.Relu, bias=bn_s[0:T, 1], scale=bn_s[0:T, 0])
                nc.scalar.activation(dstB,
                                     ps[T:128].rearrange("p (r c) -> p r c", r=2, c=W),
                                     AF.Relu, bias=bn_s[T:128, 1], scale=bn_s[T:128, 0])

            for it in range(18):
                p1_tile(it)
            for it in range(18):
                p2_iter(it)
                if 18 + it < SLAB_R // 2:
                    p1_tile(18 + it)
                if it == 2:
                    nc.sync.dma_start(dsrc[:], dsrc_d[0, :, :])
                if it == 5:
                    nc.sync.dma_start(dsrc2[:], dsrc_d[1, :, :])
            nc.vector.tensor_scalar_mul(x3[0:T, 0, :], x3[0:T, 0, :], mask_s[0:T, 0])
            nc.vector.tensor_scalar_mul(x3[T:128, 35, :], x3[T:128, 35, :],
                                        mask_s[T:128, 1])

            # ---- phase 3: 3x3 conv 64->2, two tiles concurrent via PE
            # column strips.  First 8 pairs cover flow rows 0..31 (rh0) so
            # the warp pipeline for rh0 can start while rh1 still computes.
            def p3_pair(iA, iB):
                # PE column strips: out PSUM start partition must equal the
                # tile-position column, so pos-1 writes partitions 64:66.
                tiles = [(iA, 0)] + ([(iB, 1)] if iB is not None else [])
                pss = []
                for i0, pos in tiles:
                    t_ = pp.tile([128, 2 * W], F32,
                                 tag=("psf" if pos == 0 else "psfB"),
                                 name="psf", bufs=2)
                    pss.append(t_[64 * pos:64 * pos + 2])
                for tap in range(9):
                    dy, dx = tap // 3, tap % 3
                    for (i0, pos), ps in zip(tiles, pss):
                        hf = 0 if i0 < 34 else 1
                        base = i0 + dy - 30 * hf
                        rhs = bass.AP(tensor=x_sb.tensor,
                                      offset=base * WS + dx,
                                      ap=[[XH * WS, 128], [WS, 2], [1, W]])
                        nc.tensor.matmul(ps, wf2_s[:, hf, tap, :], rhs,
                                         start=(tap == 0), stop=(tap == 8),
                                         tile_position=(0, 64 * pos),
                                         skip_group_check=True)
                for (i0, pos), ps in zip(tiles, pss):
                    # stage rows into the per-half SBUF accumulator; the
                    # DRAM write happens once per row-half (2 descriptors)
                    bt_v = bass.AP(tensor=btbig[i0 // 32].tensor,
                                   offset=i0 % 32,
                                   ap=[[8192, 2], [1, 2], [32, W]])
                    src = ps.rearrange("p (r c) -> p r c", r=2, c=W)
                    if pos == 0:
                        nc.vector.tensor_copy(bt_v, src)
                    else:
                        nc.scalar.copy(bt_v, src)

            p3_pairs_rh0 = [(2 * i, 16 + 2 * i) for i in range(8)]
            p3_pairs_rh1 = [(32, 34)] + [(36 + 4 * i, 38 + 4 * i)
                                         for i in range(7)]
            # one buffer, reused for rh1 after rh0's DRAM write, and later
            # reused again as w_g (tag ring, WAR-tracked)
            btbig = {0: bp.tile([2, W * 32], BF16, tag="wg", name="btb0"),
                     1: bp.tile([2, W * 32], BF16, tag="wg", name="btb1")}

            # ---- phase 4/5: flow -> CL + index math + gathers, by row half ----
            cl_fx = bp.tile([128, 128], BF16, tag="clfx")
            cl_fy = bp.tile([128, 128], BF16, tag="clfy")

            def cl(tag):
                tt = bp.tile([128, 128], F32, tag=tag, name=tag)
                return tt

            ix = cl("ix"); iy = cl("iy"); tmp = cl("tmp")
            x0i = bp.tile([128, 128], I16, tag="x0i")
            y0i = bp.tile([128, 128], I16, tag="y0i")
            x0f = cl("x0f"); y0f = cl("y0f")
            ef = cl("ef")
            eidx = bp.tile([128, 128], I16, tag="eidx")
            gatall = bp.tile([128, 2 * NIDX * DCH], BF16, tag="tcat_gat")
            _qs = [nc.sync, nc.scalar]
            _qi = 0

            def cl_load(rh):
                # one DMA per (ch, w): contiguous 32-row runs from the
                # (ch, col, row) flow file into CL partitions
                for ch, dtile in ((0, cl_fx), (1, cl_fy)):
                    for w in range(2):
                        dst = bass.AP(tensor=dtile.tensor,
                                      offset=64 * w + 32 * rh,
                                      ap=[[128, 128], [1, 32]])
                        srcp = bass.AP(
                            tensor=flow_dramh[rh],
                            offset=ch * W * 32 + 16 * w * 32,
                            ap=[[32 * 32, 8], [32, 16], [1, 32]])
                        _qs[(ch + w) % 2].dma_start(dst, srcp)

            def idx_math(rh):
                # both w column-halves in one 2D-sliced op set
                V = nc.vector

                def S(t):
                    return bass.AP(tensor=t.tensor, offset=32 * rh,
                                   ap=[[128, 128], [64, 2], [1, 32]])

                def S16(t):
                    return bass.AP(tensor=t.tensor, offset=32 * rh,
                                   ap=[[128, 128], [64, 2], [1, 32]])

                V.tensor_scalar_mul(S(ix), S(cl_fx), 0.5)
                V.tensor_tensor(S(ix), S(ix), S(bx_s), ALU.add)
                V.tensor_scalar_mul(S(iy), S(cl_fy), 0.5)
                V.tensor_tensor(S(iy), S(iy), S(by_s), ALU.add)
                V.tensor_copy(S16(x0i), S(ix))
                V.tensor_copy(S(x0f), S16(x0i))
                V.tensor_tensor(S(tmp), S(x0f), S(ix), ALU.is_gt)
                V.tensor_tensor(S(x0f), S(x0f), S(tmp), ALU.subtract)
                V.tensor_copy(S16(y0i), S(iy))
                V.tensor_copy(S(y0f), S16(y0i))
                V.tensor_tensor(S(tmp), S(y0f), S(iy), ALU.is_gt)
                V.tensor_tensor(S(y0f), S(y0f), S(tmp), ALU.subtract)
                V.tensor_scalar_mul(S(ef), S(y0f), float(XS))
                V.tensor_tensor(S(ef), S(ef), S(x0f), ALU.add)
                V.tensor_scalar(S(ef), S(ef), 0.0, float(LNUM - XS - 2),
                                ALU.max, ALU.min)
                V.tensor_copy(S16(eidx), S(ef))

            # ---- weights math (per row-half, overlapped with P3) ----
            fx = cl("fx"); fy = cl("fy")
            vx0 = cl("vx0"); vx1 = cl("vx1"); vy0 = cl("vy0"); vy1 = cl("vy1")
            xp1 = cl("xp1"); yp1 = cl("yp1")
            gx0 = cl("gx0"); gx1 = cl("gx1"); gy0 = cl("gy0"); gy1 = cl("gy1")
            wsall = bp.tile([128, 4, 128], BF16, tag="wsall")

            def weights_math(rh):
                V = nc.vector

                def S(t):
                    return bass.AP(tensor=t.tensor, offset=32 * rh,
                                   ap=[[128, 128], [64, 2], [1, 32]])

                def SW(s):
                    return bass.AP(tensor=wsall.tensor,
                                   offset=s * 128 + 32 * rh,
                                   ap=[[4 * 128, 128], [64, 2], [1, 32]])

                V.tensor_tensor(S(fx), S(ix), S(x0f), ALU.subtract)
                V.tensor_tensor(S(fy), S(iy), S(y0f), ALU.subtract)
                V.tensor_scalar_add(S(xp1), S(x0f), 1.0)
                V.tensor_scalar_add(S(yp1), S(y0f), 1.0)
                for vt, src_f in ((vx0, x0f), (vx1, xp1)):
                    V.tensor_tensor(S(vt), S(src_f), S(xlo_s), ALU.is_ge)
                    V.tensor_tensor(S(tmp), S(src_f), S(xhi_s), ALU.is_le)
                    V.tensor_tensor(S(vt), S(vt), S(tmp), ALU.mult)
                for vt, src_f in ((vy0, y0f), (vy1, yp1)):
                    V.tensor_tensor(S(vt), S(src_f), S(ylo_s), ALU.is_ge)
                    V.tensor_tensor(S(tmp), S(src_f), S(yhi_s), ALU.is_le)
                    V.tensor_tensor(S(vt), S(vt), S(tmp), ALU.mult)
                V.tensor_scalar(S(tmp), S(fx), -1.0, 1.0, ALU.mult, ALU.add)
                V.tensor_tensor(S(gx0), S(tmp), S(vx0), ALU.mult)
                V.tensor_tensor(S(gx1), S(fx), S(vx1), ALU.mult)
                V.tensor_scalar(S(tmp), S(fy), -1.0, 1.0, ALU.mult, ALU.add)
                V.tensor_tensor(S(gy0), S(tmp), S(vy0), ALU.mult)
                V.tensor_tensor(S(gy1), S(fy), S(vy1), ALU.mult)
                V.tensor_tensor(SW(0), S(gx0), S(gy0), ALU.mult)
                V.tensor_tensor(SW(1), S(gx1), S(gy0), ALU.mult)
                V.tensor_tensor(SW(2), S(gx0), S(gy1), ALU.mult)
                V.tensor_tensor(SW(3), S(gx1), S(gy1), ALU.mult)

            def warp_front(rh):
                if rh == 0:
                    nc.sync.dma_start(flow_dramh[rh][:], btbig[rh][:])
                cl_load(rh)
                idx_math(rh)
                for w in range(2):
                    sl = slice(64 * w + 32 * rh, 64 * w + 32 * rh + 32)
                    ds = dsrc if w == 0 else dsrc2
                    off = w * (NIDX * DCH) + rh * 4096
                    nc.gpsimd.ap_gather(
                        gatall[:, off:off + 4096], ds[:],
                        eidx[:, sl],
                        channels=128, num_elems=LNUM, d=DCH, num_idxs=512)
                weights_math(rh)

            # ---- phase 6 (per row-half): weight planes -> w_g -> W4 ----
            w_g = bp.tile([8, 4, 2048], BF16, tag="wg")
            w4 = bp.tile([128, 4 * 2048], F32, tag="x_w4")

            def w4_build(rh):
                for s in range(4):
                    for w in range(2):
                        dstg = bass.AP(tensor=w_g.tensor,
                                       offset=s * 2048 + 64 * w + 32 * rh,
                                       ap=[[4 * 2048, 8], [128, 16], [1, 32]])
                        ((nc.sync if (s + w) % 2 else nc.scalar)
                         .dma_start(dstg,
                                    wsall[:, s, 64 * w + 32 * rh:
                                          64 * w + 32 * rh + 32]))
                for s in range(4):
                    for c4 in range(4):
                        pw = pp.tile([128, 256], F32, tag="pst", name="pw")
                        rhsw = bass.AP(tensor=w_g.tensor,
                                       offset=s * 2048 + 4 * c4 * 128 + 32 * rh,
                                       ap=[[4 * 2048, 8], [128, 4], [64, 2],
                                           [1, 32]])
                        nc.tensor.matmul(pw[:], emat_s[:], rhsw,
                                         start=True, stop=True)
                        # pw free = (m 4)(w 2)(r' 32)
                        dstw = bass.AP(tensor=w4.tensor,
                                       offset=s * 2048 + 4 * c4 + 512 * rh,
                                       ap=[[4 * 2048, 128], [1, 4], [1024, 2],
                                           [16, 32]])
                        src_w = pw[:].rearrange("p (m w r) -> p m w r",
                                                m=4, w=2, r=32)
                        nc.scalar.copy(dstw, src_w)

            for a, b in p3_pairs_rh0:
                p3_pair(a, b)
            warp_front(0)
            for a, b in p3_pairs_rh1:
                p3_pair(a, b)
            # rh1 flow write must precede w_g writes (shared "wg" tag ring);
            # cl_load(1) goes ahead of the w4(0) copies on the HWDGE queues.
            nc.sync.dma_start(flow_dramh[1][:], btbig[1][:])
            cl_load(1)
            w4_build(0)
            idx_math(1)
            for w in range(2):
                sl = slice(64 * w + 32, 64 * w + 64)
                nc.gpsimd.ap_gather(
                    gatall[:, w * (NIDX * DCH) + 4096:
                           w * (NIDX * DCH) + 8192],
                    (dsrc if w == 0 else dsrc2)[:], eidx[:, sl],
                    channels=128, num_elems=LNUM, d=DCH, num_idxs=512)
            weights_math(1)
            w4_build(1)

            # Two independent combine chains: call 0 on VectorE, call 1 on
            # GpSimd, each with its own scratch so they run concurrently.
            # pls reuse dead buffers: the feature-batch ring (vector chain —
            # free long before the combine) and dsrc2 (gpsimd chain — its
            # WAR on the last gather is already implied by gpsimd FIFO).
            pls_c = [bp.tile([128, 4, NIDX], BF16, tag=("fbatch", "dsrc2")[c],
                             name=f"pls{c}", bufs=(3 if c == 0 else 1))
                     for c in range(2)]
            bb_c = [bp.tile([128, 2, NIDX], F32, tag=f"bbc{c}", name=f"bbc{c}")
                    for c in range(2)]
            def combine(call, slot, rh, eng, pls):
                bbt = bb_c[call]
                sl = slice(512 * rh, 512 * rh + 512)
                for s in range(4):
                    g_v = bass.AP(tensor=gatall.tensor,
                                  offset=call * NIDX * DCH + rh * 4096
                                  + 4 * slot + s,
                                  ap=[[2 * NIDX * DCH, 128], [DCH, 512]])
                    eng.tensor_tensor(
                        pls[:, s, sl], g_v,
                        w4[:, (s * 2048 + 1024 * call + 512 * rh):
                           (s * 2048 + 1024 * call + 512 * rh + 512)],
                        ALU.mult)
                eng.tensor_tensor(pls[:, 0, sl], pls[:, 0, sl], pls[:, 1, sl],
                                  ALU.add)
                eng.tensor_tensor(pls[:, 2, sl], pls[:, 2, sl], pls[:, 3, sl],
                                  ALU.add)
                eng.tensor_tensor(bbt[:, slot, sl], pls[:, 0, sl],
                                  pls[:, 2, sl], ALU.add)
                if rh == 1:
                    nl = 16 if slot == 0 else 3
                    for G in range(8):
                        dst = bass.AP(
                            tensor=out_d,
                            offset=(16 * slot) * 64 * W + 32 * G + 16 * call,
                            ap=[[64 * W, nl], [W, 64], [1, 16]])
                        srcb = bass.AP(
                            tensor=bbt.tensor,
                            offset=(16 * G) * (2 * NIDX) + slot * NIDX,
                            ap=[[2 * NIDX, nl], [16, 64], [1, 16]])
                        q = (nc.scalar if eng is nc.gpsimd
                             else (nc.sync if G % 2 else nc.scalar))
                        q.dma_start(dst, srcb)

            # gpsimd takes one quarter (it is ~1.5x slower per op and pays
            # a ucode lib swap); vector takes the other three.  rh0 halves
            # can start as soon as the rh0 gathers and W4 half are ready.
            for rh in range(2):
                combine(1, 0, rh, nc.gpsimd, pls_c[1])
                combine(0, 0, rh, nc.vector, pls_c[0])
                combine(0, 1, rh, nc.vector, pls_c[0])
                combine(1, 1, rh, nc.vector, pls_c[0])
    nc.finalize()
    return nc


# ======================= host-side prep =======================

def _feat_slab(feat_b, h):
    """feat_b (256, 128, 256) f32 -> (2, 128, 68, 256) bf16 slab for half h."""
    r0 = 64 * h - 2
    slab = np.zeros((CIN, SLAB_R, W), np.float32)
    lo, hi = max(r0, 0), min(r0 + SLAB_R, H)
    slab[:, lo - r0:hi - r0, :] = feat_b[:, lo:hi, :]
    return np.ascontiguousarray(
        slab.reshape(2, 128, SLAB_R, W).astype(BF))


def _host_constants(q, h):
    R0 = 64 * h
    # CL layout: p = 16G + m, f = 64w + r; pixel (row R0+r, col 32G+16w+m)
    p = np.arange(128)[:, None]
    f = np.arange(128)[None, :]
    G = p // 16
    m = p % 16
    r = f % 64
    w = f // 64
    col = 32 * G + 16 * w + m
    row = R0 + r
    ix_base = col + col / (W - 1.0) - 0.5
    iy_base = row + row / (H - 1.0) - 0.5
    colbase = 32 * G + 16 * w - COLB
    rowbase = R0 - ROWB
    bx = np.broadcast_to(ix_base - colbase, (128, 128)).astype(np.float32).copy()
    by = np.broadcast_to(iy_base - rowbase, (128, 128)).astype(np.float32).copy()
    xlo = np.broadcast_to(0.0 - colbase, (128, 128)).astype(np.float32).copy()
    xhi = np.broadcast_to((W - 1.0) - colbase, (128, 128)).astype(np.float32).copy()
    ylo = np.full((128, 128), 0.0 - rowbase, np.float32)
    yhi = np.full((128, 128), (H - 1.0) - rowbase, np.float32)
    return bx, by, xlo, xhi, ylo, yhi


def _dsrc_build(pred_imgs, h):
    """pred_imgs: (19, 128, 256) f32. Returns (2, 128, LNUM*8) f32 gather
    source; call = col-half w, slab = rows [R0-6, R0+70) x 26-col band."""
    R0 = 64 * h
    padded = np.zeros((CCLS, H + 16, W + 16), np.float32)
    padded[:, 8:8 + H, 8:8 + W] = pred_imgs
    out = np.zeros((2, 128, LNUM, DCH), np.float32)
    rowbase = R0 - ROWB
    for call in range(2):
        for G in range(8):
            colbase = 32 * G + 16 * call - COLB
            for l in range(16):
                for slot in range(2):
                    img = l + 16 * slot
                    if img >= CCLS:
                        img = l
                    for j2 in range(2):
                        for j1 in range(2):
                            win = padded[img,
                                         8 + rowbase + j2: 8 + rowbase + j2 + YS,
                                         8 + colbase + j1: 8 + colbase + j1 + XS]
                            out[call, 16 * G + l, :, 4 * slot + 2 * j2 + j1] = \
                                win.reshape(-1)
    return out.reshape(2, 128, LNUM * DCH)


def make_inputs(core, t1_feature, t2_feature, t2_pred, w_down1, w_down2,
                w_flow1, bn_gamma, bn_beta, bn_mean, bn_var, w_flow2):
    q, h = core // 2, core % 2
    f1 = _feat_slab(t1_feature[q], h)
    f2 = _feat_slab(t2_feature[q], h)
    wd = np.stack([
        np.stack([w_down1[:, 128 * k:128 * (k + 1), 0, 0].T for k in range(2)]),
        np.stack([w_down2[:, 128 * k:128 * (k + 1), 0, 0].T for k in range(2)]),
    ]).transpose(2, 0, 1, 3).astype(BF).copy()        # (128,2,2,64)
    wf1 = np.stack([w_flow1[:, :, t // 3, t % 3].T for t in range(9)],
                   axis=1).astype(BF).copy()          # (128,9,64)
    wf2h = np.stack([w_flow2[:, :, t // 3, t % 3].T for t in range(9)],
                    axis=1).astype(BF)                # (64,9,2)
    z = np.zeros_like(wf2h)
    wf2 = np.stack([np.concatenate([wf2h, z], axis=0),
                    np.concatenate([z, wf2h], axis=0)],
                   axis=1).copy()                     # (128,2,9,2)
    scale = bn_gamma / np.sqrt(bn_var + 1e-5)
    bias = bn_beta - bn_mean * scale
    bn1 = np.stack([scale, bias], axis=1).reshape(T, 2, 1).astype(np.float32)
    bn = np.concatenate([bn1, bn1], axis=0)           # (128,2,1)
    mask = np.ones((128, 2, 1), np.float32)
    if h == 0:
        mask[0:T, 0] = 0.0   # x row 0 (half A) = image row -1
    else:
        mask[T:128, 1] = 0.0  # x half-B row 35 = x row 65 = image row 128
    bx, by, xlo, xhi, ylo, yhi = _host_constants(q, h)
    imgs = img_list(q)
    pred_imgs = np.stack([t2_pred[n, ch] for (n, ch) in imgs])
    dsrc = _dsrc_build(pred_imgs, h)
    emat = np.zeros((8, 128), BF)
    for Gi in range(8):
        emat[Gi, 16 * Gi:16 * (Gi + 1)] = 1.0
    return {
        "f1": f1, "f2": f2, "wd": wd, "wf1": wf1, "wf2": wf2,
        "bn": bn, "mask": mask, "bx": bx, "by": by, "ylo": ylo, "yhi": yhi,
        "xlo": xlo, "xhi": xhi, "emat": emat, "dsrc": dsrc.astype(BF),
    }


_NC_CACHE = {}


def kernel(**inputs):
    from concourse.bass_utils import run_bass_kernel_spmd
    if "nc" not in _NC_CACHE:
        _NC_CACHE["nc"] = build_nc()
    nc = _NC_CACHE["nc"]
    in_maps = [make_inputs(c, **inputs) for c in range(8)]
    res = run_bass_kernel_spmd(nc, in_maps, list(range(8)))
    out = np.zeros((NB, CCLS, H, W), np.float32)
    for c in range(8):
        q, h = c // 2, c % 2
        o = res.results[c]["out"]
        for i, (n, ch) in enumerate(img_list(q)):
            out[n, ch, 64 * h:64 * (h + 1), :] = o[i]
    return out



# revision 53
# speedup vs baseline: 1.4420x; 1.0014x over previous
"""Trainium2 Bass kernel for nn_AlignModule_full (8 NeuronCores, data-parallel).

Reference computation: two 1x1 convs -> concat -> 3x3 conv + BN + ReLU ->
3x3 conv -> flow -> bilinear grid_sample warp of t2_pred, where output
channel (n, ch) is warped with flow[(3n+ch) % 4] (torch flow.repeat
semantics faithfully ported by the reference).

Sharding: core c = (q, h), q = c//2 flow batch, h = c%2 row half.
Each core computes flow(q, rows 64h..64h+64) from batch-q features, then
warps the 19 (n, ch) images with (3n+ch)%4 == q for its row half, using
only its own flow. Zero cross-core communication.

Warp implementation: per-pixel bilinear gather via gpsimd ap_gather with a
host-built d=8 interleaved source: each index fetches the full 2x2 patch
for TWO image slots at once (19 images = 16 lanes x 2 slots).
"""
import sys

for _p in ('/opt/trn_rl_repo',):
    if _p not in sys.path:
        sys.path.append(_p)

import numpy as np
import ml_dtypes

import concourse.bass as bass
import concourse.bacc as bacc
import concourse.mybir as mybir
import concourse.tile as tile

F32 = mybir.dt.float32
BF16 = mybir.dt.bfloat16
I16 = mybir.dt.int16
AF = mybir.ActivationFunctionType
ALU = mybir.AluOpType

H, W, CIN, T, CCLS, NB = 128, 256, 256, 64, 19, 4
SLAB_R = 68          # feature slab rows
WS = 258             # padded width for t/x buffers
XR = 66              # x rows total
XH = 36              # x rows per partition-half (A: 0..36, B: 30..66)
YS, XS = 76, 26      # gather slab rows/cols per (group, call=col-half)
LNUM = YS * XS       # base positions per partition
DCH = 8              # interleave chunk: 2 slots x (2x2 patch)
NIDX = 1024          # gather indices per group per call
ROWB = 6             # slab row margin before first output row of the call
COLB = 5             # slab col margin before group col block

BF = ml_dtypes.bfloat16


def img_list(q):
    return [(n, ch) for n in range(NB) for ch in range(CCLS)
            if (3 * n + ch) % 4 == q]


def build_nc():
    nc = bacc.Bacc(None, target_bir_lowering=False, debug=False)
    P = nc.declare_dram_parameter
    f1_d = P("f1", [2, 128, SLAB_R, W], BF16, isOutput=False)
    f2_d = P("f2", [2, 128, SLAB_R, W], BF16, isOutput=False)
    wd_d = P("wd", [128, 2, 2, T], BF16, isOutput=False)
    wf1_d = P("wf1", [128, 9, T], BF16, isOutput=False)
    wf2_d = P("wf2", [128, 2, 9, 2], BF16, isOutput=False)
    bn_d = P("bn", [128, 2, 1], F32, isOutput=False)
    mask_d = P("mask", [128, 2, 1], F32, isOutput=False)
    bx_d = P("bx", [128, 128], F32, isOutput=False)
    by_d = P("by", [128, 128], F32, isOutput=False)
    ylo_d = P("ylo", [128, 128], F32, isOutput=False)
    yhi_d = P("yhi", [128, 128], F32, isOutput=False)
    xlo_d = P("xlo", [128, 128], F32, isOutput=False)
    xhi_d = P("xhi", [128, 128], F32, isOutput=False)
    emat_d = P("emat", [8, 128], BF16, isOutput=False)
    dsrc_d = P("dsrc", [2, 128, LNUM * DCH], BF16, isOutput=False)
    out_d = P("out", [CCLS, 64, W], F32, isOutput=True)

    # flow row-quarters (ch, col, 16 rows) -- small enough to round-trip
    # through DRAM with 2-descriptor DMAs as soon as 16 rows are done
    flow_dramq = [nc.dram_tensor(f"flow_q{q}", [2, W, 16], BF16)
                  for q in range(4)]

    NRB = 4               # feature rows per DMA batch
    NBATCH = (SLAB_R + NRB - 1) // NRB  # 12 (last batch 2 rows)

    with tile.TileContext(nc) as tc:
        with (
            tc.tile_pool(name="stream", bufs=3) as sp,
            tc.tile_pool(name="big", bufs=1) as bp,
            tc.tile_pool(name="psA", bufs=2, space="PSUM") as pp,
        ):
            # ---- feature batches: [128, feat, ck, NRB, W] bf16, 2-deep ring
            fts = {}

            def load_batch(b):
                r0 = NRB * b
                nr = min(NRB, SLAB_R - r0)
                ft = bp.tile([128, 2, 2, NRB, W], BF16, tag="fbatch",
                             name=f"fb{b}", bufs=3)
                for fi, fd in ((0, f1_d), (1, f2_d)):
                    src = bass.AP(tensor=fd, offset=r0 * W,
                                  ap=[[SLAB_R * W, 128], [128 * SLAB_R * W, 2],
                                      [W, nr], [1, W]])
                    (nc.sync if fi == 0 else nc.scalar).dma_start(
                        ft[:, fi, :, 0:nr, :], src)
                fts[b] = ft

            load_batch(0)

            # ---- constants (spread across both HWDGE queues) ----
            wd_s = bp.tile([128, 2, 2, T], BF16, tag="wd")
            wf1_s = bp.tile([128, 9, T], BF16, tag="wf1")
            wf2_s = bp.tile([128, 2, 9, 2], BF16, tag="wf2")
            bn_s = bp.tile([128, 2, 1], F32, tag="bn")
            mask_s = bp.tile([128, 2, 1], F32, tag="mask")
            xlo_s = bp.tile([128, 128], F32, tag="xlo")
            xhi_s = bp.tile([128, 128], F32, tag="xhi")
            emat_s = bp.tile([8, 128], BF16, tag="emat")
            bx_s = bp.tile([128, 128], F32, tag="bx")
            by_s = bp.tile([128, 128], F32, tag="by")
            ylo_s = bp.tile([128, 128], F32, tag="ylo")
            yhi_s = bp.tile([128, 128], F32, tag="yhi")
            for i, (t_, d_) in enumerate((
                    (wd_s, wd_d), (wf1_s, wf1_d), (wf2_s, wf2_d),
                    (bn_s, bn_d), (mask_s, mask_d), (xlo_s, xlo_d),
                    (xhi_s, xhi_d),
                    (emat_s, emat_d), (bx_s, bx_d), (by_s, by_d),
                    (ylo_s, ylo_d), (yhi_s, yhi_d))):
                (nc.sync if i % 2 else nc.scalar).dma_start(t_[:], d_[:])

            # ---- big shared tiles; gather sources loaded up front (SWDGE) ----
            t_cat = bp.tile([128, SLAB_R * WS], BF16, tag="tcat_gat")
            dsrc = bp.tile([128, LNUM * DCH], BF16, tag="dsrc")
            dsrc2 = bp.tile([128, LNUM * DCH], BF16, tag="dsrc2")
            # dsrc loads happen post-P1-solo on the HWDGE queues so startup
            # HBM bandwidth is all features.  GpSimd runs only ap_gather +
            # tensor_tensor, and a dummy gather below pre-loads the gather
            # ucode lib so no IRAM swap lands on the critical path.
            x_sb = bp.tile([128, XH * WS], BF16, tag="x_w4")

            dum_src = sp.tile([128, 8], BF16, tag="dumg", bufs=1)
            dum_idx = sp.tile([128, 1], I16, tag="dumi", bufs=1)
            dum_out = sp.tile([128, 32], BF16, tag="dumo", bufs=1)
            nc.vector.memset(dum_src[:], 0.0)
            nc.vector.memset(dum_idx[:], 0)
            nc.gpsimd.ap_gather(dum_out[:], dum_src[:], dum_idx[:],
                                channels=128, num_elems=4, d=2, num_idxs=16)

            load_batch(1)

            t3 = t_cat[:].rearrange("p (r c) -> p r c", r=SLAB_R, c=WS)
            nc.vector.memset(t3[:, :, 0:1], 0.0)
            nc.vector.memset(t3[:, :, 257:258], 0.0)

            # ---- phases 1+2 interleaved: 1x1 convs feed 3x3 conv ----
            def p1_tile(it):
                r0 = 2 * it
                b, rr = r0 // NRB, r0 % NRB
                if rr == 0 and b + 1 < NBATCH and (b + 1) not in fts:
                    load_batch(b + 1)
                ft = fts[b]
                ps = pp.tile([128, 2 * W], F32, tag="pst", name="pst")
                for ck in range(2):
                    nc.tensor.matmul(ps[0:T, :], wd_s[:, 0, ck, :],
                                     ft[:, 0, ck, rr:rr + 2, :],
                                     start=(ck == 0), stop=(ck == 1),
                                     tile_position=(0, 0),
                                     skip_group_check=True)
                    nc.tensor.matmul(ps[T:128, :], wd_s[:, 1, ck, :],
                                     ft[:, 1, ck, rr:rr + 2, :],
                                     start=(ck == 0), stop=(ck == 1),
                                     tile_position=(0, 64),
                                     skip_group_check=True)
                dst = bass.AP(tensor=t_cat.tensor, offset=r0 * WS + 1,
                              ap=[[SLAB_R * WS, 128], [WS, 2], [1, W]])
                nc.vector.tensor_copy(dst, ps[:].rearrange("p (r c) -> p r c",
                                                           r=2, c=W))

            x3 = x_sb[:].rearrange("p (r c) -> p r c", r=XH, c=WS)
            nc.vector.memset(x3[:, :, 0:1], 0.0)
            nc.vector.memset(x3[:, :, 257:258], 0.0)

            def p2_iter(it):
                jA = 2 * it
                jB = 30 + 2 * it
                ps = pp.tile([128, 2 * W], F32, tag="psx", name="psx")
                for tap in range(9):
                    dy, dx = tap // 3, tap % 3
                    rhsA = bass.AP(tensor=t_cat.tensor,
                                   offset=(jA + dy) * WS + dx,
                                   ap=[[SLAB_R * WS, 128], [WS, 2], [1, W]])
                    rhsB = bass.AP(tensor=t_cat.tensor,
                                   offset=(jB + dy) * WS + dx,
                                   ap=[[SLAB_R * WS, 128], [WS, 2], [1, W]])
                    nc.tensor.matmul(ps[0:T, :], wf1_s[:, tap, :], rhsA,
                                     start=(tap == 0), stop=(tap == 8),
                                     tile_position=(0, 0),
                                     skip_group_check=True)
                    nc.tensor.matmul(ps[T:128, :], wf1_s[:, tap, :], rhsB,
                                     start=(tap == 0), stop=(tap == 8),
                                     tile_position=(0, 64),
                                     skip_group_check=True)
                dstA = bass.AP(tensor=x_sb.tensor, offset=jA * WS + 1,
                               ap=[[XH * WS, T], [WS, 2], [1, W]])
                dstB = bass.AP(tensor=x_sb.tensor,
                               offset=T * (XH * WS) + jA * WS + 1,
                               ap=[[XH * WS, T], [WS, 2], [1, W]])
                nc.scalar.activation(dstA,
                                     ps[0:T].rearrange("p (r c) -> p r c", r=2, c=W),
                                     AF.Relu, bias=bn_s[0:T, 1], scale=bn_s[0:T, 0])
                nc.scalar.activation(dstB,
                                     ps[T:128].rearrange("p (r c) -> p r c", r=2, c=W),
                                     AF.Relu, bias=bn_s[T:128, 1], scale=bn_s[T:128, 0])

            for it in range(18):
                p1_tile(it)
            for it in range(18):
                p2_iter(it)
                if 18 + it < SLAB_R // 2:
                    p1_tile(18 + it)
                if it == 2:
                    nc.sync.dma_start(dsrc[:], dsrc_d[0, :, :])
                if it == 5:
                    nc.sync.dma_start(dsrc2[:], dsrc_d[1, :, :])
            nc.vector.tensor_scalar_mul(x3[0:T, 0, :], x3[0:T, 0, :], mask_s[0:T, 0])
            nc.vector.tensor_scalar_mul(x3[T:128, 35, :], x3[T:128, 35, :],
                                        mask_s[T:128, 1])

            # ---- phase 3: 3x3 conv 64->2, two tiles concurrent via PE
            # column strips.  First 8 pairs cover flow rows 0..31 (rh0) so
            # the warp pipeline for rh0 can start while rh1 still computes.
            def p3_pair(iA, iB):
                # PE column strips: out PSUM start partition must equal the
                # tile-position column, so pos-1 writes partitions 64:66.
                tiles = [(iA, 0)] + ([(iB, 1)] if iB is not None else [])
                pss = []
                for i0, pos in tiles:
                    t_ = pp.tile([128, 2 * W], F32,
                                 tag=("psf" if pos == 0 else "psfB"),
                                 name="psf", bufs=2)
                    pss.append(t_[64 * pos:64 * pos + 2])
                for tap in range(9):
                    dy, dx = tap // 3, tap % 3
                    for (i0, pos), ps in zip(tiles, pss):
                        hf = 0 if i0 < 34 else 1
                        base = i0 + dy - 30 * hf
                        rhs = bass.AP(tensor=x_sb.tensor,
                                      offset=base * WS + dx,
                                      ap=[[XH * WS, 128], [WS, 2], [1, W]])
                        nc.tensor.matmul(ps, wf2_s[:, hf, tap, :], rhs,
                                         start=(tap == 0), stop=(tap == 8),
                                         tile_position=(0, 64 * pos),
                                         skip_group_check=True)
                for (i0, pos), ps in zip(tiles, pss):
                    # stage rows into the per-quarter SBUF accumulator; the
                    # DRAM write happens once per quarter (2 descriptors)
                    bt_v = bass.AP(tensor=btbig[i0 // 16].tensor,
                                   offset=i0 % 16,
                                   ap=[[16 * W, 2], [1, 2], [16, W]])
                    src = ps.rearrange("p (r c) -> p r c", r=2, c=W)
                    if pos == 0:
                        nc.vector.tensor_copy(bt_v, src)
                    else:
                        nc.scalar.copy(bt_v, src)

            # tiles i0 in quarter q: {16q, 16q+2, .., 16q+14}; pair within
            # the quarter so its flow completes after 4 pairs
            p3_pairs_q = [[(16 * q + 2 * i, 16 * q + 8 + 2 * i)
                           for i in range(4)] for q in range(4)]
            # one buffer reused across quarters and finally as w_g
            # (tag ring, WAR-tracked)
            btbig = {q: bp.tile([2, W * 16], BF16, tag="wg", name=f"btb{q}")
                     for q in range(4)}

            # ---- phase 4/5: flow -> CL + index math + gathers, by row half ----
            cl_fx = bp.tile([128, 128], BF16, tag="clfx")
            cl_fy = bp.tile([128, 128], BF16, tag="clfy")

            def cl(tag):
                tt = bp.tile([128, 128], F32, tag=tag, name=tag)
                return tt

            ix = cl("ix"); iy = cl("iy"); tmp = cl("tmp")
            x0i = bp.tile([128, 128], I16, tag="x0i")
            y0i = bp.tile([128, 128], I16, tag="y0i")
            x0f = cl("x0f"); y0f = cl("y0f")
            ef = cl("ef")
            eidx = bp.tile([128, 128], I16, tag="eidx")
            gatall = bp.tile([128, 2 * NIDX * DCH], BF16, tag="tcat_gat")
            _qs = [nc.sync, nc.scalar]
            _qi = 0

            def cl_load(q):
                # one DMA per (ch, w): contiguous 16-row runs from the
                # (ch, col, row) flow quarter-file into CL partitions
                for ch, dtile in ((0, cl_fx), (1, cl_fy)):
                    for w in range(2):
                        dst = bass.AP(tensor=dtile.tensor,
                                      offset=64 * w + 16 * q,
                                      ap=[[128, 128], [1, 16]])
                        srcp = bass.AP(
                            tensor=flow_dramq[q],
                            offset=ch * W * 16 + 16 * w * 16,
                            ap=[[32 * 16, 8], [16, 16], [1, 16]])
                        _qs[(ch + w) % 2].dma_start(dst, srcp)

            def idx_math(q):
                # both w column-halves in one 2D-sliced op set
                V = nc.vector

                def S(t):
                    return bass.AP(tensor=t.tensor, offset=16 * q,
                                   ap=[[128, 128], [64, 2], [1, 16]])

                S16 = S

                V.tensor_scalar_mul(S(ix), S(cl_fx), 0.5)
                V.tensor_tensor(S(ix), S(ix), S(bx_s), ALU.add)
                V.tensor_scalar_mul(S(iy), S(cl_fy), 0.5)
                V.tensor_tensor(S(iy), S(iy), S(by_s), ALU.add)
                V.tensor_copy(S16(x0i), S(ix))
                V.tensor_copy(S(x0f), S16(x0i))
                V.tensor_tensor(S(tmp), S(x0f), S(ix), ALU.is_gt)
                V.tensor_tensor(S(x0f), S(x0f), S(tmp), ALU.subtract)
                V.tensor_copy(S16(y0i), S(iy))
                V.tensor_copy(S(y0f), S16(y0i))
                V.tensor_tensor(S(tmp), S(y0f), S(iy), ALU.is_gt)
                V.tensor_tensor(S(y0f), S(y0f), S(tmp), ALU.subtract)
                V.tensor_scalar_mul(S(ef), S(y0f), float(XS))
                V.tensor_tensor(S(ef), S(ef), S(x0f), ALU.add)
                V.tensor_scalar(S(ef), S(ef), 0.0, float(LNUM - XS - 2),
                                ALU.max, ALU.min)
                V.tensor_copy(S16(eidx), S(ef))

            # ---- weights math (per row-half, overlapped with P3) ----
            fx = cl("fx"); fy = cl("fy")
            vx0 = cl("vx0"); vx1 = cl("vx1"); vy0 = cl("vy0"); vy1 = cl("vy1")
            xp1 = cl("xp1"); yp1 = cl("yp1")
            gx0 = cl("gx0"); gx1 = cl("gx1"); gy0 = cl("gy0"); gy1 = cl("gy1")
            wsall = bp.tile([128, 4, 128], BF16, tag="wsall")

            def weights_math(rh):
                V = nc.vector

                def S(t):
                    return bass.AP(tensor=t.tensor, offset=32 * rh,
                                   ap=[[128, 128], [64, 2], [1, 32]])

                def SW(s):
                    return bass.AP(tensor=wsall.tensor,
                                   offset=s * 128 + 32 * rh,
                                   ap=[[4 * 128, 128], [64, 2], [1, 32]])

                V.tensor_tensor(S(fx), S(ix), S(x0f), ALU.subtract)
                V.tensor_tensor(S(fy), S(iy), S(y0f), ALU.subtract)
                V.tensor_scalar_add(S(xp1), S(x0f), 1.0)
                V.tensor_scalar_add(S(yp1), S(y0f), 1.0)
                for vt, src_f in ((vx0, x0f), (vx1, xp1)):
                    V.tensor_tensor(S(vt), S(src_f), S(xlo_s), ALU.is_ge)
                    V.tensor_tensor(S(tmp), S(src_f), S(xhi_s), ALU.is_le)
                    V.tensor_tensor(S(vt), S(vt), S(tmp), ALU.mult)
                for vt, src_f in ((vy0, y0f), (vy1, yp1)):
                    V.tensor_tensor(S(vt), S(src_f), S(ylo_s), ALU.is_ge)
                    V.tensor_tensor(S(tmp), S(src_f), S(yhi_s), ALU.is_le)
                    V.tensor_tensor(S(vt), S(vt), S(tmp), ALU.mult)
                V.tensor_scalar(S(tmp), S(fx), -1.0, 1.0, ALU.mult, ALU.add)
                V.tensor_tensor(S(gx0), S(tmp), S(vx0), ALU.mult)
                V.tensor_tensor(S(gx1), S(fx), S(vx1), ALU.mult)
                V.tensor_scalar(S(tmp), S(fy), -1.0, 1.0, ALU.mult, ALU.add)
                V.tensor_tensor(S(gy0), S(tmp), S(vy0), ALU.mult)
                V.tensor_tensor(S(gy1), S(fy), S(vy1), ALU.mult)
                V.tensor_tensor(SW(0), S(gx0), S(gy0), ALU.mult)
                V.tensor_tensor(SW(1), S(gx1), S(gy0), ALU.mult)
                V.tensor_tensor(SW(2), S(gx0), S(gy1), ALU.mult)
                V.tensor_tensor(SW(3), S(gx1), S(gy1), ALU.mult)

            def warp_q(q):
                nc.sync.dma_start(flow_dramq[q][:], btbig[q][:])
                cl_load(q)
                idx_math(q)
                for w in range(2):
                    sl = slice(64 * w + 16 * q, 64 * w + 16 * q + 16)
                    ds = dsrc if w == 0 else dsrc2
                    off = w * (NIDX * DCH) + 2048 * q
                    nc.gpsimd.ap_gather(
                        gatall[:, off:off + 2048], ds[:],
                        eidx[:, sl],
                        channels=128, num_elems=LNUM, d=DCH, num_idxs=256)

            # ---- phase 6 (per row-half): weight planes -> w_g -> W4 ----
            w_g = bp.tile([8, 4, 2048], BF16, tag="wg")
            w4 = bp.tile([128, 4 * 2048], F32, tag="x_w4")

            def w4_build(rh):
                for s in range(4):
                    for w in range(2):
                        dstg = bass.AP(tensor=w_g.tensor,
                                       offset=s * 2048 + 64 * w + 32 * rh,
                                       ap=[[4 * 2048, 8], [128, 16], [1, 32]])
                        ((nc.sync if (s + w) % 2 else nc.scalar)
                         .dma_start(dstg,
                                    wsall[:, s, 64 * w + 32 * rh:
                                          64 * w + 32 * rh + 32]))
                for s in range(4):
                    for c4 in range(4):
                        pw = pp.tile([128, 256], F32, tag="pst", name="pw")
                        rhsw = bass.AP(tensor=w_g.tensor,
                                       offset=s * 2048 + 4 * c4 * 128 + 32 * rh,
                                       ap=[[4 * 2048, 8], [128, 4], [64, 2],
                                           [1, 32]])
                        nc.tensor.matmul(pw[:], emat_s[:], rhsw,
                                         start=True, stop=True)
                        # pw free = (m 4)(w 2)(r' 32)
                        dstw = bass.AP(tensor=w4.tensor,
                                       offset=s * 2048 + 4 * c4 + 512 * rh,
                                       ap=[[4 * 2048, 128], [1, 4], [1024, 2],
                                           [16, 32]])
                        src_w = pw[:].rearrange("p (m w r) -> p m w r",
                                                m=4, w=2, r=32)
                        nc.scalar.copy(dstw, src_w)

            for q in range(4):
                for a, b in p3_pairs_q[q]:
                    p3_pair(a, b)
                warp_q(q)
            weights_math(0)
            w4_build(0)
            weights_math(1)
            w4_build(1)

            # Two independent combine chains: call 0 on VectorE, call 1 on
            # GpSimd, each with its own scratch so they run concurrently.
            # pls reuse dead buffers: the feature-batch ring (vector chain —
            # free long before the combine) and dsrc2 (gpsimd chain — its
            # WAR on the last gather is already implied by gpsimd FIFO).
            pls_c = [bp.tile([128, 4, NIDX], BF16, tag=("fbatch", "dsrc2")[c],
                             name=f"pls{c}", bufs=(3 if c == 0 else 1))
                     for c in range(2)]
            bb_c = [bp.tile([128, 2, NIDX], F32, tag=f"bbc{c}", name=f"bbc{c}")
                    for c in range(2)]
            def combine(call, slot, rh, eng, pls):
                bbt = bb_c[call]
                sl = slice(512 * rh, 512 * rh + 512)
                for s in range(4):
                    g_v = bass.AP(tensor=gatall.tensor,
                                  offset=call * NIDX * DCH + rh * 4096
                                  + 4 * slot + s,
                                  ap=[[2 * NIDX * DCH, 128], [DCH, 512]])
                    eng.tensor_tensor(
                        pls[:, s, sl], g_v,
                        w4[:, (s * 2048 + 1024 * call + 512 * rh):
                           (s * 2048 + 1024 * call + 512 * rh + 512)],
                        ALU.mult)
                eng.tensor_tensor(pls[:, 0, sl], pls[:, 0, sl], pls[:, 1, sl],
                                  ALU.add)
                eng.tensor_tensor(pls[:, 2, sl], pls[:, 2, sl], pls[:, 3, sl],
                                  ALU.add)
                eng.tensor_tensor(bbt[:, slot, sl], pls[:, 0, sl],
                                  pls[:, 2, sl], ALU.add)
                if rh == 1:
                    nl = 16 if slot == 0 else 3
                    for G in range(8):
                        dst = bass.AP(
                            tensor=out_d,
                            offset=(16 * slot) * 64 * W + 32 * G + 16 * call,
                            ap=[[64 * W, nl], [W, 64], [1, 16]])
                        srcb = bass.AP(
                            tensor=bbt.tensor,
                            offset=(16 * G) * (2 * NIDX) + slot * NIDX,
                            ap=[[2 * NIDX, nl], [16, 64], [1, 16]])
                        q = (nc.scalar if eng is nc.gpsimd
                             else (nc.sync if G % 2 else nc.scalar))
                        q.dma_start(dst, srcb)

            # gpsimd takes one quarter (it is ~1.5x slower per op and pays
            # a ucode lib swap); vector takes the other three.  rh0 halves
            # can start as soon as the rh0 gathers and W4 half are ready.
            for rh in range(2):
                combine(1, 0, rh, nc.gpsimd, pls_c[1])
                combine(0, 0, rh, nc.vector, pls_c[0])
                combine(0, 1, rh, nc.vector, pls_c[0])
                combine(1, 1, rh, nc.vector, pls_c[0])
    nc.finalize()
    return nc


# ======================= host-side prep =======================

def _feat_slab(feat_b, h):
    """feat_b (256, 128, 256) f32 -> (2, 128, 68, 256) bf16 slab for half h."""
    r0 = 64 * h - 2
    slab = np.zeros((CIN, SLAB_R, W), np.float32)
    lo, hi = max(r0, 0), min(r0 + SLAB_R, H)
    slab[:, lo - r0:hi - r0, :] = feat_b[:, lo:hi, :]
    return np.ascontiguousarray(
        slab.reshape(2, 128, SLAB_R, W).astype(BF))


def _host_constants(q, h):
    R0 = 64 * h
    # CL layout: p = 16G + m, f = 64w + r; pixel (row R0+r, col 32G+16w+m)
    p = np.arange(128)[:, None]
    f = np.arange(128)[None, :]
    G = p // 16
    m = p % 16
    r = f % 64
    w = f // 64
    col = 32 * G + 16 * w + m
    row = R0 + r
    ix_base = col + col / (W - 1.0) - 0.5
    iy_base = row + row / (H - 1.0) - 0.5
    colbase = 32 * G + 16 * w - COLB
    rowbase = R0 - ROWB
    bx = np.broadcast_to(ix_base - colbase, (128, 128)).astype(np.float32).copy()
    by = np.broadcast_to(iy_base - rowbase, (128, 128)).astype(np.float32).copy()
    xlo = np.broadcast_to(0.0 - colbase, (128, 128)).astype(np.float32).copy()
    xhi = np.broadcast_to((W - 1.0) - colbase, (128, 128)).astype(np.float32).copy()
    ylo = np.full((128, 128), 0.0 - rowbase, np.float32)
    yhi = np.full((128, 128), (H - 1.0) - rowbase, np.float32)
    return bx, by, xlo, xhi, ylo, yhi


def _dsrc_build(pred_imgs, h):
    """pred_imgs: (19, 128, 256) f32. Returns (2, 128, LNUM*8) f32 gather
    source; call = col-half w, slab = rows [R0-6, R0+70) x 26-col band."""
    R0 = 64 * h
    padded = np.zeros((CCLS, H + 16, W + 16), np.float32)
    padded[:, 8:8 + H, 8:8 + W] = pred_imgs
    out = np.zeros((2, 128, LNUM, DCH), np.float32)
    rowbase = R0 - ROWB
    for call in range(2):
        for G in range(8):
            colbase = 32 * G + 16 * call - COLB
            for l in range(16):
                for slot in range(2):
                    img = l + 16 * slot
                    if img >= CCLS:
                        img = l
                    for j2 in range(2):
                        for j1 in range(2):
                            win = padded[img,
                                         8 + rowbase + j2: 8 + rowbase + j2 + YS,
                                         8 + colbase + j1: 8 + colbase + j1 + XS]
                            out[call, 16 * G + l, :, 4 * slot + 2 * j2 + j1] = \
                                win.reshape(-1)
    return out.reshape(2, 128, LNUM * DCH)


def make_inputs(core, t1_feature, t2_feature, t2_pred, w_down1, w_down2,
                w_flow1, bn_gamma, bn_beta, bn_mean, bn_var, w_flow2):
    q, h = core // 2, core % 2
    f1 = _feat_slab(t1_feature[q], h)
    f2 = _feat_slab(t2_feature[q], h)
    wd = np.stack([
        np.stack([w_down1[:, 128 * k:128 * (k + 1), 0, 0].T for k in range(2)]),
        np.stack([w_down2[:, 128 * k:128 * (k + 1), 0, 0].T for k in range(2)]),
    ]).transpose(2, 0, 1, 3).astype(BF).copy()        # (128,2,2,64)
    wf1 = np.stack([w_flow1[:, :, t // 3, t % 3].T for t in range(9)],
                   axis=1).astype(BF).copy()          # (128,9,64)
    wf2h = np.stack([w_flow2[:, :, t // 3, t % 3].T for t in range(9)],
                    axis=1).astype(BF)                # (64,9,2)
    z = np.zeros_like(wf2h)
    wf2 = np.stack([np.concatenate([wf2h, z], axis=0),
                    np.concatenate([z, wf2h], axis=0)],
                   axis=1).copy()                     # (128,2,9,2)
    scale = bn_gamma / np.sqrt(bn_var + 1e-5)
    bias = bn_beta - bn_mean * scale
    bn1 = np.stack([scale, bias], axis=1).reshape(T, 2, 1).astype(np.float32)
    bn = np.concatenate([bn1, bn1], axis=0)           # (128,2,1)
    mask = np.ones((128, 2, 1), np.float32)
    if h == 0:
        mask[0:T, 0] = 0.0   # x row 0 (half A) = image row -1
    else:
        mask[T:128, 1] = 0.0  # x half-B row 35 = x row 65 = image row 128
    bx, by, xlo, xhi, ylo, yhi = _host_constants(q, h)
    imgs = img_list(q)
    pred_imgs = np.stack([t2_pred[n, ch] for (n, ch) in imgs])
    dsrc = _dsrc_build(pred_imgs, h)
    emat = np.zeros((8, 128), BF)
    for Gi in range(8):
        emat[Gi, 16 * Gi:16 * (Gi + 1)] = 1.0
    return {
        "f1": f1, "f2": f2, "wd": wd, "wf1": wf1, "wf2": wf2,
        "bn": bn, "mask": mask, "bx": bx, "by": by, "ylo": ylo, "yhi": yhi,
        "xlo": xlo, "xhi": xhi, "emat": emat, "dsrc": dsrc.astype(BF),
    }


_NC_CACHE = {}


def kernel(**inputs):
    from concourse.bass_utils import run_bass_kernel_spmd
    if "nc" not in _NC_CACHE:
        _NC_CACHE["nc"] = build_nc()
    nc = _NC_CACHE["nc"]
    in_maps = [make_inputs(c, **inputs) for c in range(8)]
    res = run_bass_kernel_spmd(nc, in_maps, list(range(8)))
    out = np.zeros((NB, CCLS, H, W), np.float32)
    for c in range(8):
        q, h = c // 2, c % 2
        o = res.results[c]["out"]
        for i, (n, ch) in enumerate(img_list(q)):
            out[n, ch, 64 * h:64 * (h + 1), :] = o[i]
    return out

